# revision 73
# baseline (speedup 1.0000x reference)
"""Trainium2 Bass kernel for nn_Big_MPNN (gnn_message_passing).

Self-contained: hardcodes shapes/sharding. Data-parallel over the batch dim
across 8 NeuronCores (16 graphs per core), weights replicated; no collectives.

Node layout: the host pairs graphs to BALANCE per-type counts (local search
minimizing sum of per-type max counts over pairs), then sorts nodes by GRU
atom-type within each pair. Each pair occupies exactly U = sum(caps) columns
(no dead padding); per-type capacities are uniform across all pairs/cores so
every per-type GRU matmul reads a static strided access pattern.

Per-core dataflow (3 passes), transposed activations [D=128 part, cols],
all f16 except PSUM/biases/final cast.  Pairs are processed in groups of two;
per (layer, bond) wave one 2-bank PSUM tile holds both pairs and is drained
by a single ReLU op, load-balanced between ACT and DVE (GPSIMD cannot read
PSUM; it gets the SBUF-only f16 GRU elementwise ops instead).  Each group's
layer-7 flip (chunks 128/128/rem -> normal-layout xb) and aggregation
m^T = xb^T g^T are split into small matmul quanta and paced between the NEXT
group's MLP wave tiles, so the tensor engine never runs long drain-free
stretches.  GRU pieces merge both GRU universes into single wide elementwise
ops (messages in one [128, 2, NP] tile; tanh and the blend each issued once
per piece over [2, ncols]); each piece is expanded into 9 dependency-ordered
stage closures dripped two per wave so no engine's in-order stream blocks on
an op whose inputs resolve late (ACT/DVE have no exec-queue lookahead).  The
final pass drains its last pieces stage-interleaved with a DVE-only tail and
ships y in two DMA halves; pieces must be ISSUED before a DMA that reads
their columns (issue order defines RAW vs WAR for the dependency tracker).
Host unpads/unpermutes the f16 result.
"""

import numpy as np

import concourse.bass as bass
import concourse.bacc as bacc
import concourse.tile as tile
import concourse.mybir as mybir

F32 = mybir.dt.float32
F16 = mybir.dt.float16
AF = mybir.ActivationFunctionType
ALU = mybir.AluOpType

M = 8                      # cores
B, N, FEAT, D = 128, 128, 75, 128
NB, NL, NT = 7, 8, 6       # bonds, mlp layers, gru type slots
PASSES = 3
BG = B // M                # graphs per core
NPAIR = BG // 2            # graph pairs per core (8)
TOP_ATOMS = [6.0, 7.0, 8.0, 9.0, 0.0]


def _pair_graphs(cnt):
    """Pair the B graphs to minimize sum_t max_pairs(count_t).  cnt: [B, NT]."""
    P = B // 2
    order = np.argsort(cnt[:, NT - 1], kind="stable")
    pairs = np.stack([order[:P], order[:P - 1:-1]], 1)
    rng = np.random.default_rng(12345)

    def obj(pr):
        pc = cnt[pr[:, 0]] + cnt[pr[:, 1]]
        s = np.sort(pc, 0)[::-1]
        return s[0].sum() * 1000 + s[1].sum() * 10 + s[2].sum()

    cur = pairs.copy()
    co = obj(cur)
    best, bo = cur.copy(), co
    for _ in range(150000):
        i, j = rng.integers(0, P, 2)
        if i == j:
            continue
        trial = cur.copy()
        a1, b1 = trial[i]
        a2, b2 = trial[j]
        if rng.integers(0, 2) == 0:
            trial[i] = (a1, a2)
            trial[j] = (b1, b2)
        else:
            trial[i] = (a1, b2)
            trial[j] = (a2, b1)
        to = obj(trial)
        if to <= co:
            cur, co = trial, to
            if to < bo:
                best, bo = trial.copy(), to
    return best


def _prepare(g, h, msg_W, gru_Wih, gru_Whh, gru_bih, gru_bhh):
    g = np.ascontiguousarray(np.asarray(g, np.float32))
    h = np.ascontiguousarray(np.asarray(h, np.float32))
    msg_W = np.asarray(msg_W, np.float32)
    gru_Wih = np.asarray(gru_Wih, np.float32).reshape(2, NT, 3, D, D)
    gru_Whh = np.asarray(gru_Whh, np.float32).reshape(2, NT, 3, D, D)
    gru_bih = np.asarray(gru_bih, np.float32).reshape(2, NT, 3, D)
    gru_bhh = np.asarray(gru_bhh, np.float32).reshape(2, NT, 3, D)

    atoms = h[:, :, 0]
    tid = np.full((B, N), NT - 1, np.int32)
    for i, a in enumerate(TOP_ATOMS):
        tid[atoms == np.float32(a)] = i
    cnt = np.stack([(tid == t).sum(1) for t in range(NT)], 1).astype(np.int64)

    pairs = _pair_graphs(cnt)                       # [64, 2] graph ids
    pc = cnt[pairs[:, 0]] + cnt[pairs[:, 1]]
    caps = tuple(int(c) for c in pc.max(axis=0))
    U = sum(caps)
    assert 256 < U <= 384, f"caps {caps} sum {U} out of supported range"
    rem = U - 256
    NP = NPAIR * U
    offs = np.cumsum([0] + list(caps))[:-1]

    # replicated weights, partition-major f16 layouts
    mwT = np.transpose(msg_W, (3, 0, 1, 2))         # [din, k, l, dout]
    mwT06 = np.ascontiguousarray(mwT[:, :, :NL - 1]).astype(np.float16)
    mw8T = np.ascontiguousarray(mwT[:, :, NL - 1]).astype(np.float16)
    wihT = np.ascontiguousarray(
        np.transpose(gru_Wih, (4, 0, 1, 2, 3))).astype(np.float16)
    whhT = np.ascontiguousarray(
        np.transpose(gru_Whh, (4, 0, 1, 2, 3))).astype(np.float16)
    brz = np.ascontiguousarray(
        np.transpose(gru_bih[:, :, :2] + gru_bhh[:, :, :2], (3, 0, 1, 2)))
    brzM = np.ascontiguousarray(
        np.transpose(brz, (1, 2, 3, 0)).reshape(1, 2 * NT * 2, D)
    ).astype(np.float16)
    ones1 = np.ones((1, 512), np.float16)
    binn = np.ascontiguousarray(np.transpose(gru_bih[:, :, 2], (2, 0, 1)))
    bhnn = np.ascontiguousarray(np.transpose(gru_bhh[:, :, 2], (2, 0, 1)))

    h_t = np.concatenate([h, np.zeros((B, N, D - FEAT), np.float32)], axis=2)

    in_maps = []
    placements = []     # per core: (gids [BG], pos [BG, N])
    for c in range(M):
        gids = pairs[c * NPAIR:(c + 1) * NPAIR].reshape(-1)
        pos = np.zeros((BG, N), np.int64)
        x0 = np.zeros((NP, D), np.float32)
        gPa = np.zeros((128, NPAIR, 2, NB, U), np.float32)
        gPr = np.zeros((rem, NPAIR, NB, U), np.float32)
        for p in range(NPAIR):
            ga, gb = gids[2 * p], gids[2 * p + 1]
            tp = np.concatenate([tid[ga], tid[gb]])            # [256]
            hp = np.concatenate([h_t[ga], h_t[gb]], axis=0)    # [256, D]
            ppos = np.zeros(2 * N, np.int64)
            for t in range(NT):
                idx = np.flatnonzero(tp == t)
                ppos[idx] = offs[t] + np.arange(len(idx))
            pos[2 * p] = p * U + ppos[:N]
            pos[2 * p + 1] = p * U + ppos[N:]
            x0[p * U + ppos] = hp
            # dense pair block: big[m_row, k, n_col] = g[graph, k, n, m]
            big = np.zeros((U, NB, U), np.float32)
            for gi, gr in enumerate((ga, gb)):
                lg = ppos[gi * N:(gi + 1) * N]
                blk = np.transpose(g[gr], (2, 0, 1))           # [m, k, n]
                big[np.ix_(lg, np.arange(NB), lg)] = blk
            gPa[:, p, 0] = np.transpose(big[:128], (0, 1, 2))
            gPa[:, p, 1] = big[128:256]
            gPr[:, p] = big[256:U]
        placements.append((gids.copy(), pos))
        in_maps.append(dict(
            gPa=gPa.astype(np.float16),
            gPr=gPr.astype(np.float16),
            x0=np.ascontiguousarray(x0.T).astype(np.float16),
            mwT06=mwT06, mw8T=mw8T, wihT=wihT, whhT=whhT,
            brz=brz, binn=binn, bhnn=bhnn,
            brzM=brzM, ones1=ones1,
        ))
    meta = (caps, U)
    return in_maps, meta, placements


class _Balancer:
    """Greedy per-engine load balancer for drain/elementwise ops."""

    def __init__(self, nc):
        self.nc = nc
        self.load = {"A": 0.0, "D": 0.0, "P": 0.0}

    def _cost(self, e, op, cols, psum_src, f16_sbuf):
        # Exact TimelineSim engine-busy costs: processing = cols*cycle_t +
        # max-over-operands(2*access_cycles)/2 * cycle_t (SBUF dst dominates).
        if e == "A":
            return cols * 0.8333 + 185.0
        if e == "D":
            if f16_sbuf:
                return cols * 0.521 + 60.0
            return cols * 1.0417 + 125.0
        eff = 0.42 if op in ("add", "sub", "mul") else 0.6
        return cols * 0.8333 / eff + 131.0

    def pick(self, op, cols, psum_src=True, f16_sbuf=False, allow=("A", "D")):
        cand = [(self.load[e] + self._cost(e, op, cols, psum_src, f16_sbuf), e)
                for e in allow]
        _, e = min(cand)
        self.load[e] += self._cost(e, op, cols, psum_src, f16_sbuf)
        return e

    def charge(self, e, op, cols, psum_src=True, f16_sbuf=False):
        self.load[e] += self._cost(e, op, cols, psum_src, f16_sbuf)

    # PSUM sources: GPSIMD has no PSUM access -> ACT/DVE only.
    def relu(self, out, ps, cols):
        e = self.pick("relu", cols)
        if e == "A":
            self.nc.scalar.activation(out, ps, AF.Relu)
        else:
            self.nc.vector.tensor_scalar_max(out, ps, 0.0)

    def copy(self, out, ps, cols):
        e = self.pick("copy", cols)
        if e == "A":
            self.nc.scalar.copy(out, ps)
        else:
            self.nc.vector.tensor_copy(out, ps)

    def stt(self, out, in0, scal, in1, op0, op1, cols):
        self.charge("D", "stt", cols)
        self.nc.vector.scalar_tensor_tensor(out, in0, scal, in1,
                                            op0=op0, op1=op1)

    # SBUF-only f16 elementwise: DVE or Pool.
    def tt(self, op, out, a, b, cols, f16_sbuf=True, allow=("P",)):
        e = self.pick(op, cols, psum_src=False, f16_sbuf=f16_sbuf, allow=allow)
        eng = self.nc.vector if e == "D" else self.nc.gpsimd
        getattr(eng, "tensor_" + op)(out, a, b)


def _build(meta):
    caps, U = meta
    rem = U - 256
    NP = NPAIR * U
    nc = bacc.Bacc("TRN2", target_bir_lowering=False, debug=False, num_devices=M)

    gPa_d = nc.dram_tensor("gPa", [128, NPAIR, 2, NB, U], F16, kind="ExternalInput")
    gPr_d = nc.dram_tensor("gPr", [rem, NPAIR, NB, U], F16, kind="ExternalInput")
    x0_d = nc.dram_tensor("x0", [128, NP], F16, kind="ExternalInput")
    mwT06_d = nc.dram_tensor("mwT06", [128, NB, NL - 1, 128], F16, kind="ExternalInput")
    mw8T_d = nc.dram_tensor("mw8T", [128, NB, 128], F16, kind="ExternalInput")
    wih_d = nc.dram_tensor("wihT", [128, 2, NT, 3, 128], F16, kind="ExternalInput")
    whh_d = nc.dram_tensor("whhT", [128, 2, NT, 3, 128], F16, kind="ExternalInput")
    brz_d = nc.dram_tensor("brz", [128, 2, NT, 2], F32, kind="ExternalInput")
    brzM_d = nc.dram_tensor("brzM", [1, 2 * NT * 2, 128], F16, kind="ExternalInput")
    ones_d = nc.dram_tensor("ones1", [1, 512], F16, kind="ExternalInput")
    binn_d = nc.dram_tensor("binn", [128, 2, NT], F32, kind="ExternalInput")
    bhnn_d = nc.dram_tensor("bhnn", [128, 2, NT], F32, kind="ExternalInput")
    y_d = nc.dram_tensor("y", [128, NP], F16, kind="ExternalOutput")

    # GRU pieces: (type, col-offset, pair0, n_pairs); issued after pair p0+npr-1
    # The final pass splits the second half into npr=2 pieces so the tail only
    # waits on the last two pairs' aggregation.
    pieces_at = {pr: [] for pr in range(NPAIR)}
    pieces_at_final = {pr: [] for pr in range(NPAIR)}
    off = 0
    for t in range(NT):
        if caps[t] == 0:
            continue
        npr = min(4, max(1, 256 // caps[t]))
        while NPAIR % npr:
            npr -= 1
        for p0 in range(0, NPAIR, npr):
            pieces_at[p0 + npr - 1].append((t, off, p0, npr))
            if p0 < NPAIR // 2 or npr <= 2:
                pieces_at_final[p0 + npr - 1].append((t, off, p0, npr))
            else:
                for q0 in range(p0, p0 + npr, 2):
                    pieces_at_final[q0 + 1].append((t, off, q0, 2))
        off += caps[t]

    with tile.TileContext(nc) as tc:
        with (
            tc.tile_pool(name="const", bufs=1) as cp,
            tc.tile_pool(name="xp", bufs=2) as xp,
            tc.tile_pool(name="mlp", bufs=24) as mp,
            tc.tile_pool(name="x7p", bufs=10) as x7p,
            tc.tile_pool(name="xbp", bufs=3) as xbp,
            tc.tile_pool(name="gtp", bufs=5) as gtp,
            tc.tile_pool(name="mtp", bufs=2) as mtp,
            tc.tile_pool(name="gates", bufs=32) as ggp,
            tc.tile_pool(name="mps", bufs=3, space="PSUM") as mpsp,
            tc.tile_pool(name="ps", bufs=2, space="PSUM") as psp,
        ):
            bal = _Balancer(nc)

            x_cur = xp.tile([128, NP], F16, tag="x")
            mwT06 = cp.tile([128, NB, NL - 1, 128], F16, tag="mwT06")
            nc.sync.dma_start(x_cur[:, 0:U], x0_d.ap()[:, 0:U])
            nc.sync.dma_start(mwT06[:, :, 0:1, :], mwT06_d.ap()[:, :, 0:1, :])
            nc.sync.dma_start(x_cur[:, U:3 * U], x0_d.ap()[:, U:3 * U])
            nc.sync.dma_start(mwT06[:, :, 1:, :], mwT06_d.ap()[:, :, 1:, :])
            nc.sync.dma_start(x_cur[:, 3 * U:6 * U], x0_d.ap()[:, 3 * U:6 * U])
            nc.sync.dma_start(x_cur[:, 6 * U:], x0_d.ap()[:, 6 * U:])

            gtiles = {}
            for pn in (0, 1):
                gta0 = gtp.tile([128, 2, NB, U], F16, tag="gta")
                nc.sync.dma_start(gta0[:], gPa_d.ap()[:, pn])
                gtr0 = gtp.tile([64, NB, U], F16, tag="gtr")
                o = 32 * (pn % 2)
                nc.sync.dma_start(gtr0[o:o + rem], gPr_d.ap()[:, pn])
                gtiles[pn] = (gta0, gtr0)

            mw8T = cp.tile([128, NB, 128], F16, tag="mw8T")
            wih = cp.tile([128, 2, NT, 3, 128], F16, tag="wih")
            whh = cp.tile([128, 2, NT, 3, 128], F16, tag="whh")
            brz = cp.tile([128, 2, NT, 2], F32, tag="brz")
            brzM = cp.tile([1, 2 * NT * 2, 128], F16, tag="brzM")
            ones1 = cp.tile([1, 512], F16, tag="ones1")
            binn = cp.tile([128, 2, NT], F32, tag="binn")
            bhnn = cp.tile([128, 2, NT], F32, tag="bhnn")
            nc.sync.dma_start(mw8T[:], mw8T_d.ap())
            nc.sync.dma_start(wih[:], wih_d.ap())
            nc.sync.dma_start(whh[:], whh_d.ap())
            nc.sync.dma_start(brz[:], brz_d.ap())
            nc.sync.dma_start(brzM[:], brzM_d.ap())
            nc.sync.dma_start(ones1[:], ones_d.ap())
            nc.sync.dma_start(binn[:], binn_d.ap())
            nc.sync.dma_start(bhnn[:], bhnn_d.ap())

            def seg(tile_, t_off, p0, npr, w):
                return tile_[:].rearrange("d (pr u) -> d pr u", u=U)[
                    :, p0:p0 + npr, t_off:t_off + w]

            def seg_m(m2, u, t_off, p0, npr, w):
                return m2[:, u, :].rearrange("d (pr u2) -> d pr u2", u2=U)[
                    :, p0:p0 + npr, t_off:t_off + w]

            def seg2(m2, t_off, p0, npr, w):
                """4D view of the merged [128, 2, NP] message tile:
                [128, u, pair, col]."""
                return m2[:].rearrange("d u (pr u2) -> d u pr u2", u2=U)[
                    :, :, p0:p0 + npr, t_off:t_off + w]

            def piece_stages(args, fast_tail):
                """Stage closures for ONE GRU piece.  Dripping one stage per
                wave keeps each engine's in-order stream free of ops whose
                dependencies resolve late (head-of-line blocking: ACT/DVE have
                no exec-queue lookahead, so a stalled op blocks later ones)."""
                (xc, xn, m2, piece) = args
                t, t_off, p0, npr = piece
                w = caps[t]
                ncols = npr * w
                s = dict(xs=seg(xc, t_off, p0, npr, w),
                         ms=[seg_m(m2, 0, t_off, p0, npr, w),
                             seg_m(m2, 1, t_off, p0, npr, w)],
                         ms2=seg2(m2, t_off, p0, npr, w))
                tail = ("D",) if fast_tail else ("P",)

                def st_mm():
                    s["prz"], s["pn2"] = [], []
                    for u in range(2):
                        prz = psp.tile([128, 2, 256], F32, tag="ps",
                                       name="prz")
                        pool2 = mpsp if fast_tail else psp
                        pn2 = pool2.tile([128, 2, 256], F32,
                                         tag="mps" if fast_tail else "ps",
                                         name="pn2")
                        for gi in range(2):
                            nc.tensor.matmul(prz[:, gi, :ncols],
                                             wih[:, u, t, gi, :], s["xs"],
                                             start=True, stop=False)
                            nc.tensor.matmul(prz[:, gi, :ncols],
                                             whh[:, u, t, gi, :], s["ms"][u],
                                             start=False, stop=False)
                            row = (u * NT + t) * 2 + gi
                            nc.tensor.matmul(prz[:, gi, :ncols],
                                             brzM[0:1, row, :],
                                             ones1[0:1, :ncols],
                                             start=False, stop=True)
                        nc.tensor.matmul(pn2[:, 0, :ncols], wih[:, u, t, 2, :],
                                         s["xs"], start=True, stop=True)
                        nc.tensor.matmul(pn2[:, 1, :ncols], whh[:, u, t, 2, :],
                                         s["ms"][u], start=True, stop=True)
                        s["prz"].append(prz)
                        s["pn2"].append(pn2)
                    s["rzb"] = ggp.tile([128, 2, 2, 256], F16,
                                        tag="gt4", name="rzb", bufs=6)

                def st_sig(u):
                    nc.scalar.activation(s["rzb"][:, u, :, :ncols],
                                         s["prz"][u][:, :, :ncols],
                                         AF.Sigmoid)
                    bal.charge("A", "act", 2 * ncols)

                def st_t1():
                    t12 = ggp.tile([128, 2, 256], F16, tag="gt2", name="t12",
                                   bufs=17)
                    s["t12"] = t12
                    for u in range(2):
                        bal.stt(t12[:, u, :ncols], s["pn2"][u][:, 1, :ncols],
                                bhnn[:, u, t:t + 1], s["rzb"][:, u, 0, :ncols],
                                ALU.add, ALU.mult, ncols)

                def st_na():
                    na2 = ggp.tile([128, 2, 256], F16, tag="gt2", name="na2",
                                   bufs=17)
                    s["na2"] = na2
                    for u in range(2):
                        bal.stt(na2[:, u, :ncols], s["pn2"][u][:, 0, :ncols],
                                binn[:, u, t:t + 1], s["t12"][:, u, :ncols],
                                ALU.add, ALU.add, ncols)

                def st_tanh():
                    n2 = ggp.tile([128, 2, 256], F16, tag="gt2", name="n2",
                                  bufs=17)
                    nc.scalar.activation(n2[:, :, :ncols],
                                         s["na2"][:, :, :ncols], AF.Tanh)
                    bal.charge("A", "act", 2 * ncols, psum_src=False)
                    s["n2"] = n2

                def st_d2():
                    d2 = ggp.tile([128, 2, 256], F16, tag="gt2", name="d2",
                                  bufs=17)
                    n2v = s["n2"][:, :, :ncols].rearrange(
                        "d u (pr w) -> d u pr w", w=w)
                    d2v = d2[:, :, :ncols].rearrange(
                        "d u (pr w) -> d u pr w", w=w)
                    bal.tt("sub", d2v, s["ms2"], n2v, 2 * ncols, allow=tail)
                    s["d2"] = d2

                def st_e2():
                    e2 = ggp.tile([128, 2, 256], F16, tag="gt2", name="e2",
                                  bufs=17)
                    bal.tt("mul", e2[:, :, :ncols],
                           s["rzb"][:, :, 1, :ncols],
                           s["d2"][:, :, :ncols], 2 * ncols, allow=tail)
                    s["e2"] = e2

                def st_hu():
                    hu2 = ggp.tile([128, 2, 256], F16, tag="gt2", name="hu2",
                                   bufs=17)
                    bal.tt("add", hu2[:, :, :ncols], s["n2"][:, :, :ncols],
                           s["e2"][:, :, :ncols], 2 * ncols, allow=tail)
                    hv = hu2[:, :, :ncols].rearrange(
                        "d u (pr w) -> d u pr w", w=w)
                    bal.tt("add", seg(xn, t_off, p0, npr, w),
                           hv[:, 0], hv[:, 1],
                           ncols, allow=("D",) if fast_tail else ("P",))

                return [st_mm, lambda: st_sig(0), lambda: st_sig(1),
                        st_t1, st_na, st_tanh, st_d2, st_e2, st_hu]

            def issue_pieces(batch, fast_tail):
                """Issue whole pieces, stage-interleaved across the batch."""
                stl = [piece_stages(a, fast_tail) for a in batch]
                for i in range(max(len(sl) for sl in stl)):
                    for sl in stl:
                        if i < len(sl):
                            sl[i]()

            def flip_quanta(pr, i, x7t, xb):
                """Per-pair layer-8 flip quanta, chunks 0-1 only (the rem
                chunk of BOTH pairs goes into one shared group tile)."""
                qs = []
                for k0, kn in ((0, 2), (2, 2), (4, 2), (6, 1)):
                    def fq(k0=k0, kn=kn):
                        kk = list(range(k0, k0 + kn))
                        ps3 = mpsp.tile([128, len(kk), 2, 128], F32, tag="mps",
                                        name="ps3")
                        for j, k in enumerate(kk):
                            nc.tensor.matmul(ps3[:, j, 0, :],
                                             x7t[k][:, i, 0:128],
                                             mw8T[:, k, :],
                                             start=True, stop=True)
                            nc.tensor.matmul(ps3[:, j, 1, :],
                                             x7t[k][:, i, 128:256],
                                             mw8T[:, k, :],
                                             start=True, stop=True)
                        bal.copy(xb[:, k0:k0 + len(kk), :, :], ps3[:],
                                 len(kk) * 256)
                    qs.append(fq)
                return qs

            def c2_quantum(grp, x7t, xbc2):
                """Both pairs' rem-chunk flips into ONE PSUM tile (pair 1 at
                partition 32 via PE col-tiling, auto-derived from the out AP's
                base partition) and a single 896-col drain."""
                def cq():
                    ps3c = mpsp.tile([128, NB, 128], F32, tag="mps",
                                     name="ps3c")
                    for i in range(len(grp)):
                        off = 32 * i
                        for k in range(NB):
                            nc.tensor.matmul(ps3c[off:off + rem, k, :],
                                             x7t[k][:, i, 256:U],
                                             mw8T[:, k, :],
                                             start=True, stop=True)
                    bal.copy(xbc2[0:32 + rem, :, :], ps3c[0:32 + rem, :, :],
                             NB * 128)
                return [cq]

            def agg_quanta(pr, i, xb, xbc2, xc, xn, m2, pat):
                """Aggregation quanta for one pair; mc=2 reads the shared rem
                tile at base partition 32*i (gtr rows DMA'd to match)."""
                cell = {}

                def ps_():
                    if 'ps' not in cell:
                        cell['ps'] = (psp.tile([128, U], F32, tag="ps",
                                               name="ps_n"),
                                      psp.tile([128, U], F32, tag="ps",
                                               name="ps_u"))
                    return cell['ps']

                off = 32 * i
                qs = []
                steps = [(mc, k) for mc in range(3) for k in range(NB)]
                chunks = [steps[j:j + 4] for j in range(0, len(steps), 4)]
                for ci, ch in enumerate(chunks):
                    def aq(ch=ch, lastq=(ci == len(chunks) - 1)):
                        ps_n, ps_u = ps_()
                        gta, gtr = gtiles[pr]
                        for (mc, k) in ch:
                            dst = ps_u if k == NB - 1 else ps_n
                            start = mc == 0 and k in (0, NB - 1)
                            stop = mc == 2 and k in (NB - 2, NB - 1)
                            if mc < 2:
                                nc.tensor.matmul(dst[:], xb[:, k, mc, :],
                                                 gta[:, mc, k, :],
                                                 start=start, stop=stop)
                            else:
                                nc.tensor.matmul(dst[:],
                                                 xbc2[off:off + rem, k, :],
                                                 gtr[off:off + rem, k, :],
                                                 start=start, stop=stop)
                        if lastq:
                            sl = slice(pr * U, (pr + 1) * U)
                            bal.copy(m2[:, 0, sl], ps_n[:], U)
                            bal.copy(m2[:, 1, sl], ps_u[:], U)
                            for piece in pat[pr]:
                                pending.append((xc, xn, m2, piece))
                    qs.append(aq)
                return qs

            from collections import deque
            pending = []        # GRU pieces awaiting issue
            fillers = deque()   # flip/agg quanta awaiting interleave
            stq = deque()       # piece stages dripped one per wave
            GROUPS = ((0, 1), (2, 3), (4, 5), (6, 7))
            for p in range(PASSES):
                last = p == PASSES - 1
                pat = pieces_at
                x_next = xp.tile([128, NP], F16, tag="x")
                m2 = mtp.tile([128, 2, NP], F16, tag="m2")

                for pg, grp in enumerate(GROUPS):
                    G = len(grp)
                    # prefetch next group's adjacency (one group ahead)
                    if pg + 1 < len(GROUPS):
                        nxt = [(p, pn_) for pn_ in GROUPS[pg + 1]]
                    else:
                        nxt = [(p + 1, pn_) for pn_ in GROUPS[0]]
                    for pp, pn in nxt:
                        if pp < PASSES:
                            gta = gtp.tile([128, 2, NB, U], F16, tag="gta")
                            nc.sync.dma_start(gta[:], gPa_d.ap()[:, pn])
                            gtr = gtp.tile([64, NB, U], F16, tag="gtr")
                            o = 32 * (pn % 2)
                            nc.sync.dma_start(gtr[o:o + rem], gPr_d.ap()[:, pn])
                            gtiles[pn] = (gta, gtr)

                    # all still-pending pieces must land before this group's
                    # first wave reads their output columns
                    while pending:
                        issue_pieces([pending.pop(0)], False)

                    # ---- bond MLPs: G pairs per PSUM tile, waves over bonds;
                    # the previous group's flips/aggs and older GRU pieces are
                    # interleaved between waves to keep every engine fed ----
                    curs = [[x_cur[:, pr * U:(pr + 1) * U]] * NB for pr in grp]
                    x7t = [None] * NB
                    per_slot = len(fillers) / ((NL - 1) * NB)
                    credit = 0.0
                    for l in range(NL - 1):
                        outs = [[] for _ in grp]
                        for k in range(NB):
                            if l == NL - 2:
                                nt_ = x7p.tile([128, G, U], F16, tag="x7")
                            else:
                                nt_ = mp.tile([128, G, U], F16, tag="mlp")
                            if l == NL - 2:
                                x7t[k] = nt_
                            ps = mpsp.tile([128, G, 512], F32, tag="mps")
                            for j in range(G):
                                nc.tensor.matmul(ps[:, j, :U],
                                                 mwT06[:, k, l, :],
                                                 curs[j][k],
                                                 start=True, stop=True)
                            bal.relu(nt_[:], ps[:, :, :U], G * U)
                            for j in range(G):
                                outs[j].append(nt_[:, j, :])
                            credit += per_slot
                            for _ in range(2):
                                if stq:
                                    stq.popleft()()
                            while credit >= 1.0 and fillers:
                                fillers.popleft()()
                                credit -= 1.0
                        curs = outs
                        while pending:
                            stq.extend(piece_stages(pending.pop(0), False))

                    while fillers:
                        fillers.popleft()()
                    while stq:
                        stq.popleft()()
                    xbs = [xbp.tile([128, NB, 2, 128], F16, tag="xb",
                                    name="xb") for _ in grp]
                    xbc2 = xbp.tile([64, NB, 128], F16, tag="xbc2", bufs=2,
                                    name="xbc2")
                    for j, pr in enumerate(grp):
                        fillers.extend(flip_quanta(pr, j, x7t, xbs[j]))
                    fillers.extend(c2_quantum(grp, x7t, xbc2))
                    for j, pr in enumerate(grp):
                        fillers.extend(
                            agg_quanta(pr, j, xbs[j], xbc2, x_cur, x_next,
                                       m2, pat))

                    if last and pg == len(GROUPS) - 1:
                        # pairs 0-5: make sure every piece write is issued
                        # BEFORE the DMA reads those columns (issue order
                        # defines RAW vs WAR for the dependency tracker)
                        while pending:
                            issue_pieces([pending.pop(0)], False)
                        nc.sync.dma_start(y_d.ap()[:, 0:4 * U],
                                          x_next[:, 0:4 * U])

                x_cur = x_next

            while fillers:
                fillers.popleft()()
            while pending:
                issue_pieces(pending[:2], True)
                pending = pending[2:]
            nc.sync.dma_start(y_d.ap()[:, 4 * U:], x_cur[:, 4 * U:])

    nc.compile()
    return nc


def _make_runner(nc):
    import jax
    from jax.experimental.shard_map import shard_map
    from jax.sharding import Mesh, PartitionSpec, NamedSharding
    from concourse.bass2jax import (install_neuronx_cc_hook, _bass_exec_p,
                                    partition_id_tensor)

    install_neuronx_cc_hook()
    partition_name = (nc.partition_id_tensor.name
                      if nc.partition_id_tensor else None)
    in_names, out_names, out_avals, zero_outs = [], [], [], []
    for alloc in nc.m.functions[0].allocations:
        if not isinstance(alloc, mybir.MemoryLocationSet):
            continue
        name = alloc.memorylocations[0].name
        if alloc.kind == "ExternalInput":
            if name != partition_name:
                in_names.append(name)
        elif alloc.kind == "ExternalOutput":
            out_names.append(name)
            shape = tuple(alloc.tensor_shape)
            dtype = mybir.dt.np(alloc.dtype)
            out_avals.append(jax.core.ShapedArray(shape, dtype))
            zero_outs.append(np.zeros(shape, dtype))
    n_params = len(in_names)
    all_names = in_names + out_names
    if partition_name is not None:
        all_names = all_names + [partition_name]

    def _body(*args):
        operands = list(args)
        if partition_name is not None:
            operands.append(partition_id_tensor())
        outs = _bass_exec_p.bind(
            *operands,
            out_avals=tuple(out_avals),
            in_names=tuple(all_names),
            out_names=tuple(out_names),
            lowering_input_output_aliases=(),
            sim_require_finite=True,
            sim_require_nnan=True,
            nc=nc,
        )
        return tuple(outs)

    devices = jax.devices()[:M]
    mesh = Mesh(np.asarray(devices), ("core",))
    specs = (PartitionSpec("core"),) * (n_params + len(out_names))
    fn = jax.jit(shard_map(_body, mesh=mesh,
                           in_specs=specs,
                           out_specs=(PartitionSpec("core"),) * len(out_names)),
                 keep_unused=True)

    def put(in_maps):
        sh = NamedSharding(mesh, PartitionSpec("core"))
        args = []
        for name in in_names:
            cat = np.concatenate([np.asarray(im[name]) for im in in_maps], axis=0)
            args.append(jax.device_put(cat, sh))
        for z in zero_outs:
            cat = np.concatenate([z] * M, axis=0)
            args.append(jax.device_put(cat, sh))
        return args

    def run(args):
        outs = fn(*args)
        outs = [np.asarray(o) for o in outs]
        per_core = []
        for c in range(M):
            per_core.append({
                name: outs[i].reshape(M, *out_avals[i].shape)[c]
                for i, name in enumerate(out_names)})
        return per_core

    return put, run


_CACHE = {}


def _get_runner(meta):
    if meta not in _CACHE:
        nc = _build(meta)
        _CACHE[meta] = (_make_runner(nc), nc)
    return _CACHE[meta]


def _assemble(per_core, placements):
    out = np.empty((B, N, D), np.float32)
    for c in range(M):
        y = np.asarray(per_core[c]["y"], np.float32)   # [D, NP] padded transposed
        gids, pos = placements[c]
        out[gids] = y.T[pos]
    return out


def kernel(g, h, msg_W, gru_Wih, gru_Whh, gru_bih, gru_bhh):
    in_maps, meta, placements = _prepare(g, h, msg_W, gru_Wih, gru_Whh,
                                         gru_bih, gru_bhh)
    (put, run), _nc = _get_runner(meta)
    args = put(in_maps)
    per_core = run(args)
    return _assemble(per_core, placements)


# exposed for test.py
def get_nc_and_runner(g, h, msg_W, gru_Wih, gru_Whh, gru_bih, gru_bhh):
    in_maps, meta, placements = _prepare(g, h, msg_W, gru_Wih, gru_Whh,
                                         gru_bih, gru_bhh)
    (put, run), nc = _get_runner(meta)
    return in_maps, put, run, nc, placements



# revision 78
# speedup vs baseline: 1.0078x; 1.0078x over previous
"""Trainium2 Bass kernel for nn_Big_MPNN (gnn_message_passing).

Self-contained: hardcodes shapes/sharding. Data-parallel over the batch dim
across 8 NeuronCores (16 graphs per core), weights replicated; no collectives.

Node layout: the host pairs graphs to BALANCE per-type counts (local search
minimizing sum of per-type max counts over pairs), then sorts nodes by GRU
atom-type within each pair. Each pair occupies exactly U = sum(caps) columns
(no dead padding); per-type capacities are uniform across all pairs/cores so
every per-type GRU matmul reads a static strided access pattern.

Per-core dataflow (3 passes), transposed activations [D=128 part, cols],
all f16 except PSUM/biases/final cast.  Pairs are processed in groups of two;
per (layer, bond) wave one 2-bank PSUM tile holds both pairs and is drained
by a single ReLU op, load-balanced between ACT and DVE (GPSIMD cannot read
PSUM; it gets the SBUF-only f16 GRU elementwise ops instead).  Each group's
layer-7 flip (chunks 128/128/rem -> normal-layout xb) and aggregation
m^T = xb^T g^T are split into small matmul quanta and paced between the NEXT
group's MLP wave tiles, so the tensor engine never runs long drain-free
stretches.  GRU pieces merge both GRU universes into single wide elementwise
ops (messages in one [128, 2, NP] tile; tanh and the blend each issued once
per piece over [2, ncols]); each piece is expanded into 9 dependency-ordered
stage closures dripped two per wave so no engine's in-order stream blocks on
an op whose inputs resolve late (ACT/DVE have no exec-queue lookahead).  The
final pass drains its last pieces stage-interleaved with a DVE-only tail and
ships y in two DMA halves; pieces must be ISSUED before a DMA that reads
their columns (issue order defines RAW vs WAR for the dependency tracker).
Host unpads/unpermutes the f16 result.
"""

import numpy as np

import concourse.bass as bass
import concourse.bacc as bacc
import concourse.tile as tile
import concourse.mybir as mybir

F32 = mybir.dt.float32
F16 = mybir.dt.float16
AF = mybir.ActivationFunctionType
ALU = mybir.AluOpType

M = 8                      # cores
B, N, FEAT, D = 128, 128, 75, 128
NB, NL, NT = 7, 8, 6       # bonds, mlp layers, gru type slots
PASSES = 3
BG = B // M                # graphs per core
NPAIR = BG // 2            # graph pairs per core (8)
TOP_ATOMS = [6.0, 7.0, 8.0, 9.0, 0.0]


def _pair_graphs(cnt):
    """Pair the B graphs to minimize sum_t max_pairs(count_t).  cnt: [B, NT]."""
    P = B // 2
    order = np.argsort(cnt[:, NT - 1], kind="stable")
    pairs = np.stack([order[:P], order[:P - 1:-1]], 1)
    rng = np.random.default_rng(12345)

    def obj(pr):
        pc = cnt[pr[:, 0]] + cnt[pr[:, 1]]
        s = np.sort(pc, 0)[::-1]
        return s[0].sum() * 1000 + s[1].sum() * 10 + s[2].sum()

    cur = pairs.copy()
    co = obj(cur)
    best, bo = cur.copy(), co
    for _ in range(150000):
        i, j = rng.integers(0, P, 2)
        if i == j:
            continue
        trial = cur.copy()
        a1, b1 = trial[i]
        a2, b2 = trial[j]
        if rng.integers(0, 2) == 0:
            trial[i] = (a1, a2)
            trial[j] = (b1, b2)
        else:
            trial[i] = (a1, b2)
            trial[j] = (a2, b1)
        to = obj(trial)
        if to <= co:
            cur, co = trial, to
            if to < bo:
                best, bo = trial.copy(), to
    return best


def _prepare(g, h, msg_W, gru_Wih, gru_Whh, gru_bih, gru_bhh):
    g = np.ascontiguousarray(np.asarray(g, np.float32))
    h = np.ascontiguousarray(np.asarray(h, np.float32))
    msg_W = np.asarray(msg_W, np.float32)
    gru_Wih = np.asarray(gru_Wih, np.float32).reshape(2, NT, 3, D, D)
    gru_Whh = np.asarray(gru_Whh, np.float32).reshape(2, NT, 3, D, D)
    gru_bih = np.asarray(gru_bih, np.float32).reshape(2, NT, 3, D)
    gru_bhh = np.asarray(gru_bhh, np.float32).reshape(2, NT, 3, D)

    atoms = h[:, :, 0]
    tid = np.full((B, N), NT - 1, np.int32)
    for i, a in enumerate(TOP_ATOMS):
        tid[atoms == np.float32(a)] = i
    cnt = np.stack([(tid == t).sum(1) for t in range(NT)], 1).astype(np.int64)

    pairs = _pair_graphs(cnt)                       # [64, 2] graph ids
    pc = cnt[pairs[:, 0]] + cnt[pairs[:, 1]]
    caps = tuple(int(c) for c in pc.max(axis=0))
    U = sum(caps)
    assert 256 < U <= 384, f"caps {caps} sum {U} out of supported range"
    rem = U - 256
    NP = NPAIR * U
    offs = np.cumsum([0] + list(caps))[:-1]

    # replicated weights, partition-major f16 layouts
    mwT = np.transpose(msg_W, (3, 0, 1, 2))         # [din, k, l, dout]
    mwT06 = np.ascontiguousarray(mwT[:, :, :NL - 1]).astype(np.float16)
    mw8T = np.ascontiguousarray(mwT[:, :, NL - 1]).astype(np.float16)
    wihT = np.ascontiguousarray(
        np.transpose(gru_Wih, (4, 0, 1, 2, 3))).astype(np.float16)
    whhT = np.ascontiguousarray(
        np.transpose(gru_Whh, (4, 0, 1, 2, 3))).astype(np.float16)
    brz = np.ascontiguousarray(
        np.transpose(gru_bih[:, :, :2] + gru_bhh[:, :, :2], (3, 0, 1, 2)))
    brzM = np.ascontiguousarray(
        np.transpose(brz, (1, 2, 3, 0)).reshape(1, 2 * NT * 2, D)
    ).astype(np.float16)
    ones1 = np.ones((1, 512), np.float16)
    binn = np.ascontiguousarray(np.transpose(gru_bih[:, :, 2], (2, 0, 1)))
    bhnn = np.ascontiguousarray(np.transpose(gru_bhh[:, :, 2], (2, 0, 1)))

    h_t = np.concatenate([h, np.zeros((B, N, D - FEAT), np.float32)], axis=2)

    in_maps = []
    placements = []     # per core: (gids [BG], pos [BG, N])
    for c in range(M):
        gids = pairs[c * NPAIR:(c + 1) * NPAIR].reshape(-1)
        pos = np.zeros((BG, N), np.int64)
        x0 = np.zeros((NP, D), np.float32)
        gPa = np.zeros((128, NPAIR, 2, NB, U), np.float32)
        gPr = np.zeros((rem, NPAIR, NB, U), np.float32)
        for p in range(NPAIR):
            ga, gb = gids[2 * p], gids[2 * p + 1]
            tp = np.concatenate([tid[ga], tid[gb]])            # [256]
            hp = np.concatenate([h_t[ga], h_t[gb]], axis=0)    # [256, D]
            ppos = np.zeros(2 * N, np.int64)
            for t in range(NT):
                idx = np.flatnonzero(tp == t)
                ppos[idx] = offs[t] + np.arange(len(idx))
            pos[2 * p] = p * U + ppos[:N]
            pos[2 * p + 1] = p * U + ppos[N:]
            x0[p * U + ppos] = hp
            # dense pair block: big[m_row, k, n_col] = g[graph, k, n, m]
            big = np.zeros((U, NB, U), np.float32)
            for gi, gr in enumerate((ga, gb)):
                lg = ppos[gi * N:(gi + 1) * N]
                blk = np.transpose(g[gr], (2, 0, 1))           # [m, k, n]
                big[np.ix_(lg, np.arange(NB), lg)] = blk
            gPa[:, p, 0] = np.transpose(big[:128], (0, 1, 2))
            gPa[:, p, 1] = big[128:256]
            gPr[:, p] = big[256:U]
        placements.append((gids.copy(), pos))
        in_maps.append(dict(
            gPa=gPa.astype(np.float16),
            gPr=gPr.astype(np.float16),
            x0=np.ascontiguousarray(x0.T).astype(np.float16),
            mwT06=mwT06, mw8T=mw8T, wihT=wihT, whhT=whhT,
            brz=brz, binn=binn, bhnn=bhnn,
            brzM=brzM, ones1=ones1,
        ))
    meta = (caps, U)
    return in_maps, meta, placements


class _Balancer:
    """Greedy per-engine load balancer for drain/elementwise ops."""

    def __init__(self, nc):
        self.nc = nc
        self.load = {"A": 0.0, "D": 0.0, "P": 0.0}

    def _cost(self, e, op, cols, psum_src, f16_sbuf):
        # Exact TimelineSim engine-busy costs: processing = cols*cycle_t +
        # max-over-operands(2*access_cycles)/2 * cycle_t (SBUF dst dominates).
        if e == "A":
            return cols * 0.8333 + 185.0
        if e == "D":
            if f16_sbuf:
                return cols * 0.521 + 60.0
            return cols * 1.0417 + 125.0
        eff = 0.42 if op in ("add", "sub", "mul") else 0.6
        return cols * 0.8333 / eff + 131.0

    def pick(self, op, cols, psum_src=True, f16_sbuf=False, allow=("A", "D")):
        cand = [(self.load[e] + self._cost(e, op, cols, psum_src, f16_sbuf), e)
                for e in allow]
        _, e = min(cand)
        self.load[e] += self._cost(e, op, cols, psum_src, f16_sbuf)
        return e

    def charge(self, e, op, cols, psum_src=True, f16_sbuf=False):
        self.load[e] += self._cost(e, op, cols, psum_src, f16_sbuf)

    # PSUM sources: GPSIMD has no PSUM access -> ACT/DVE only.
    def relu(self, out, ps, cols):
        e = self.pick("relu", cols)
        if e == "A":
            self.nc.scalar.activation(out, ps, AF.Relu)
        else:
            self.nc.vector.tensor_scalar_max(out, ps, 0.0)

    def copy(self, out, ps, cols):
        e = self.pick("copy", cols)
        if e == "A":
            self.nc.scalar.copy(out, ps)
        else:
            self.nc.vector.tensor_copy(out, ps)

    def stt(self, out, in0, scal, in1, op0, op1, cols):
        self.charge("D", "stt", cols)
        self.nc.vector.scalar_tensor_tensor(out, in0, scal, in1,
                                            op0=op0, op1=op1)

    # SBUF-only f16 elementwise: DVE or Pool.
    def tt(self, op, out, a, b, cols, f16_sbuf=True, allow=("P",)):
        e = self.pick(op, cols, psum_src=False, f16_sbuf=f16_sbuf, allow=allow)
        eng = self.nc.vector if e == "D" else self.nc.gpsimd
        getattr(eng, "tensor_" + op)(out, a, b)


def _build(meta):
    caps, U = meta
    rem = U - 256
    NP = NPAIR * U
    nc = bacc.Bacc("TRN2", target_bir_lowering=False, debug=False, num_devices=M)

    gPa_d = nc.dram_tensor("gPa", [128, NPAIR, 2, NB, U], F16, kind="ExternalInput")
    gPr_d = nc.dram_tensor("gPr", [rem, NPAIR, NB, U], F16, kind="ExternalInput")
    x0_d = nc.dram_tensor("x0", [128, NP], F16, kind="ExternalInput")
    mwT06_d = nc.dram_tensor("mwT06", [128, NB, NL - 1, 128], F16, kind="ExternalInput")
    mw8T_d = nc.dram_tensor("mw8T", [128, NB, 128], F16, kind="ExternalInput")
    wih_d = nc.dram_tensor("wihT", [128, 2, NT, 3, 128], F16, kind="ExternalInput")
    whh_d = nc.dram_tensor("whhT", [128, 2, NT, 3, 128], F16, kind="ExternalInput")
    brz_d = nc.dram_tensor("brz", [128, 2, NT, 2], F32, kind="ExternalInput")
    brzM_d = nc.dram_tensor("brzM", [1, 2 * NT * 2, 128], F16, kind="ExternalInput")
    ones_d = nc.dram_tensor("ones1", [1, 512], F16, kind="ExternalInput")
    binn_d = nc.dram_tensor("binn", [128, 2, NT], F32, kind="ExternalInput")
    bhnn_d = nc.dram_tensor("bhnn", [128, 2, NT], F32, kind="ExternalInput")
    y_d = nc.dram_tensor("y", [128, NP], F16, kind="ExternalOutput")

    # GRU pieces: (type, col-offset, pair0, n_pairs); issued after pair p0+npr-1
    # The final pass splits the second half into npr=2 pieces so the tail only
    # waits on the last two pairs' aggregation.
    pieces_at = {pr: [] for pr in range(NPAIR)}
    pieces_at_final = {pr: [] for pr in range(NPAIR)}
    off = 0
    for t in range(NT):
        if caps[t] == 0:
            continue
        npr = min(4, max(1, 256 // caps[t]))
        while NPAIR % npr:
            npr -= 1
        for p0 in range(0, NPAIR, npr):
            pieces_at[p0 + npr - 1].append((t, off, p0, npr))
            if p0 < NPAIR // 2 or npr <= 2:
                pieces_at_final[p0 + npr - 1].append((t, off, p0, npr))
            else:
                for q0 in range(p0, p0 + npr, 2):
                    pieces_at_final[q0 + 1].append((t, off, q0, 2))
        off += caps[t]

    with tile.TileContext(nc) as tc:
        with (
            tc.tile_pool(name="const", bufs=1) as cp,
            tc.tile_pool(name="xp", bufs=2) as xp,
            tc.tile_pool(name="mlp", bufs=24) as mp,
            tc.tile_pool(name="x7p", bufs=10) as x7p,
            tc.tile_pool(name="xbp", bufs=3) as xbp,
            tc.tile_pool(name="gtp", bufs=5) as gtp,
            tc.tile_pool(name="mtp", bufs=2) as mtp,
            tc.tile_pool(name="gates", bufs=32) as ggp,
            tc.tile_pool(name="mps", bufs=3, space="PSUM") as mpsp,
            tc.tile_pool(name="ps", bufs=2, space="PSUM") as psp,
        ):
            bal = _Balancer(nc)

            x_cur = xp.tile([128, NP], F16, tag="x")
            mwT06 = cp.tile([128, NB, NL - 1, 128], F16, tag="mwT06")
            nc.sync.dma_start(x_cur[:, 0:U], x0_d.ap()[:, 0:U])
            nc.sync.dma_start(mwT06[:, :, 0:1, :], mwT06_d.ap()[:, :, 0:1, :])
            nc.sync.dma_start(x_cur[:, U:3 * U], x0_d.ap()[:, U:3 * U])
            nc.sync.dma_start(mwT06[:, :, 1:, :], mwT06_d.ap()[:, :, 1:, :])
            nc.sync.dma_start(x_cur[:, 3 * U:6 * U], x0_d.ap()[:, 3 * U:6 * U])
            nc.sync.dma_start(x_cur[:, 6 * U:], x0_d.ap()[:, 6 * U:])

            gtiles = {}
            for pn in (0, 1):
                gta0 = gtp.tile([128, 2, NB, U], F16, tag="gta")
                nc.sync.dma_start(gta0[:], gPa_d.ap()[:, pn])
                gtr0 = gtp.tile([64, NB, U], F16, tag="gtr")
                o = 32 * (pn % 2)
                nc.sync.dma_start(gtr0[o:o + rem], gPr_d.ap()[:, pn])
                gtiles[pn] = (gta0, gtr0)

            mw8T = cp.tile([128, NB, 128], F16, tag="mw8T")
            wih = cp.tile([128, 2, NT, 3, 128], F16, tag="wih")
            whh = cp.tile([128, 2, NT, 3, 128], F16, tag="whh")
            brz = cp.tile([128, 2, NT, 2], F32, tag="brz")
            brzM = cp.tile([1, 2 * NT * 2, 128], F16, tag="brzM")
            ones1 = cp.tile([1, 512], F16, tag="ones1")
            binn = cp.tile([128, 2, NT], F32, tag="binn")
            bhnn = cp.tile([128, 2, NT], F32, tag="bhnn")
            nc.sync.dma_start(mw8T[:], mw8T_d.ap())
            nc.sync.dma_start(wih[:], wih_d.ap())
            nc.sync.dma_start(whh[:], whh_d.ap())
            nc.sync.dma_start(brz[:], brz_d.ap())
            nc.sync.dma_start(brzM[:], brzM_d.ap())
            nc.sync.dma_start(ones1[:], ones_d.ap())
            nc.sync.dma_start(binn[:], binn_d.ap())
            nc.sync.dma_start(bhnn[:], bhnn_d.ap())

            def seg(tile_, t_off, p0, npr, w):
                return tile_[:].rearrange("d (pr u) -> d pr u", u=U)[
                    :, p0:p0 + npr, t_off:t_off + w]

            def seg_m(m2, u, t_off, p0, npr, w):
                return m2[:, u, :].rearrange("d (pr u2) -> d pr u2", u2=U)[
                    :, p0:p0 + npr, t_off:t_off + w]

            def seg2(m2, t_off, p0, npr, w):
                """4D view of the merged [128, 2, NP] message tile:
                [128, u, pair, col]."""
                return m2[:].rearrange("d u (pr u2) -> d u pr u2", u2=U)[
                    :, :, p0:p0 + npr, t_off:t_off + w]

            def piece_stages(args, fast_tail):
                """Stage closures for ONE GRU piece.  Dripping one stage per
                wave keeps each engine's in-order stream free of ops whose
                dependencies resolve late (head-of-line blocking: ACT/DVE have
                no exec-queue lookahead, so a stalled op blocks later ones)."""
                (xc, xn, m2, piece) = args
                t, t_off, p0, npr = piece
                w = caps[t]
                ncols = npr * w
                s = dict(xs=seg(xc, t_off, p0, npr, w),
                         ms=[seg_m(m2, 0, t_off, p0, npr, w),
                             seg_m(m2, 1, t_off, p0, npr, w)],
                         ms2=seg2(m2, t_off, p0, npr, w))
                tail = ("D",) if fast_tail else ("P",)

                def st_mm():
                    s["prz"], s["pn2"] = [], []
                    for u in range(2):
                        prz = psp.tile([128, 2, 256], F32, tag="ps",
                                       name="prz")
                        pool2 = mpsp if fast_tail else psp
                        pn2 = pool2.tile([128, 2, 256], F32,
                                         tag="mps" if fast_tail else "ps",
                                         name="pn2")
                        for gi in range(2):
                            nc.tensor.matmul(prz[:, gi, :ncols],
                                             wih[:, u, t, gi, :], s["xs"],
                                             start=True, stop=False)
                            nc.tensor.matmul(prz[:, gi, :ncols],
                                             whh[:, u, t, gi, :], s["ms"][u],
                                             start=False, stop=False)
                            row = (u * NT + t) * 2 + gi
                            nc.tensor.matmul(prz[:, gi, :ncols],
                                             brzM[0:1, row, :],
                                             ones1[0:1, :ncols],
                                             start=False, stop=True)
                        nc.tensor.matmul(pn2[:, 0, :ncols], wih[:, u, t, 2, :],
                                         s["xs"], start=True, stop=True)
                        nc.tensor.matmul(pn2[:, 1, :ncols], whh[:, u, t, 2, :],
                                         s["ms"][u], start=True, stop=True)
                        s["prz"].append(prz)
                        s["pn2"].append(pn2)
                    s["rzb"] = ggp.tile([128, 2, 2, 256], F16,
                                        tag="gt4", name="rzb", bufs=6)

                def st_sig(u):
                    nc.scalar.activation(s["rzb"][:, u, :, :ncols],
                                         s["prz"][u][:, :, :ncols],
                                         AF.Sigmoid)
                    bal.charge("A", "act", 2 * ncols)

                def st_t1():
                    t12 = ggp.tile([128, 2, 256], F16, tag="gt2", name="t12",
                                   bufs=17)
                    s["t12"] = t12
                    for u in range(2):
                        bal.stt(t12[:, u, :ncols], s["pn2"][u][:, 1, :ncols],
                                bhnn[:, u, t:t + 1], s["rzb"][:, u, 0, :ncols],
                                ALU.add, ALU.mult, ncols)

                def st_na():
                    na2 = ggp.tile([128, 2, 256], F16, tag="gt2", name="na2",
                                   bufs=17)
                    s["na2"] = na2
                    for u in range(2):
                        bal.stt(na2[:, u, :ncols], s["pn2"][u][:, 0, :ncols],
                                binn[:, u, t:t + 1], s["t12"][:, u, :ncols],
                                ALU.add, ALU.add, ncols)

                def st_tanh():
                    n2 = ggp.tile([128, 2, 256], F16, tag="gt2", name="n2",
                                  bufs=17)
                    nc.scalar.activation(n2[:, :, :ncols],
                                         s["na2"][:, :, :ncols], AF.Tanh)
                    bal.charge("A", "act", 2 * ncols, psum_src=False)
                    s["n2"] = n2

                def st_d2():
                    d2 = ggp.tile([128, 2, 256], F16, tag="gt2", name="d2",
                                  bufs=17)
                    n2v = s["n2"][:, :, :ncols].rearrange(
                        "d u (pr w) -> d u pr w", w=w)
                    d2v = d2[:, :, :ncols].rearrange(
                        "d u (pr w) -> d u pr w", w=w)
                    bal.tt("sub", d2v, s["ms2"], n2v, 2 * ncols, allow=tail)
                    s["d2"] = d2

                def st_e2():
                    e2 = ggp.tile([128, 2, 256], F16, tag="gt2", name="e2",
                                  bufs=17)
                    bal.tt("mul", e2[:, :, :ncols],
                           s["rzb"][:, :, 1, :ncols],
                           s["d2"][:, :, :ncols], 2 * ncols, allow=tail)
                    s["e2"] = e2

                def st_hu():
                    hu2 = ggp.tile([128, 2, 256], F16, tag="gt2", name="hu2",
                                   bufs=17)
                    bal.tt("add", hu2[:, :, :ncols], s["n2"][:, :, :ncols],
                           s["e2"][:, :, :ncols], 2 * ncols, allow=tail)
                    hv = hu2[:, :, :ncols].rearrange(
                        "d u (pr w) -> d u pr w", w=w)
                    bal.tt("add", seg(xn, t_off, p0, npr, w),
                           hv[:, 0], hv[:, 1],
                           ncols, allow=("D",) if fast_tail else ("P",))

                return [st_mm, lambda: st_sig(0), lambda: st_sig(1),
                        st_t1, st_na, st_tanh, st_d2, st_e2, st_hu]

            def issue_pieces(batch, fast_tail):
                """Issue whole pieces, stage-interleaved across the batch."""
                stl = [piece_stages(a, fast_tail) for a in batch]
                for i in range(max(len(sl) for sl in stl)):
                    for sl in stl:
                        if i < len(sl):
                            sl[i]()

            def flip_quanta(pr, i, x7t, xb):
                """Per-pair layer-8 flip quanta, chunks 0-1 only (the rem
                chunk of BOTH pairs goes into one shared group tile)."""
                qs = []
                for k0, kn in ((0, 2), (2, 2), (4, 2), (6, 1)):
                    def fq(k0=k0, kn=kn):
                        kk = list(range(k0, k0 + kn))
                        ps3 = mpsp.tile([128, len(kk), 2, 128], F32, tag="mps",
                                        name="ps3")
                        for j, k in enumerate(kk):
                            nc.tensor.matmul(ps3[:, j, 0, :],
                                             x7t[k][:, i, 0:128],
                                             mw8T[:, k, :],
                                             start=True, stop=True)
                            nc.tensor.matmul(ps3[:, j, 1, :],
                                             x7t[k][:, i, 128:256],
                                             mw8T[:, k, :],
                                             start=True, stop=True)
                        bal.copy(xb[:, k0:k0 + len(kk), :, :], ps3[:],
                                 len(kk) * 256)
                    qs.append(fq)
                return qs

            def c2_quantum(grp, x7t, xbc2):
                """Both pairs' rem-chunk flips into ONE PSUM tile (pair 1 at
                partition 32 via PE col-tiling, auto-derived from the out AP's
                base partition) and a single 896-col drain."""
                def cq():
                    ps3c = mpsp.tile([128, NB, 128], F32, tag="mps",
                                     name="ps3c")
                    for i in range(len(grp)):
                        off = 32 * i
                        for k in range(NB):
                            nc.tensor.matmul(ps3c[off:off + rem, k, :],
                                             x7t[k][:, i, 256:U],
                                             mw8T[:, k, :],
                                             start=True, stop=True)
                    bal.copy(xbc2[0:32 + rem, :, :], ps3c[0:32 + rem, :, :],
                             NB * 128)
                return [cq]

            def agg_quanta(pr, i, xb, xbc2, xc, xn, m2, pat):
                """Aggregation quanta for one pair; mc=2 reads the shared rem
                tile at base partition 32*i (gtr rows DMA'd to match)."""
                cell = {}

                def ps_():
                    if 'ps' not in cell:
                        cell['ps'] = (psp.tile([128, U], F32, tag="ps",
                                               name="ps_n"),
                                      psp.tile([128, U], F32, tag="ps",
                                               name="ps_u"))
                    return cell['ps']

                off = 32 * i
                qs = []
                steps = [(mc, k) for mc in range(3) for k in range(NB)]
                chunks = [steps[j:j + 4] for j in range(0, len(steps), 4)]
                for ci, ch in enumerate(chunks):
                    def aq(ch=ch, lastq=(ci == len(chunks) - 1)):
                        ps_n, ps_u = ps_()
                        gta, gtr = gtiles[pr]
                        for (mc, k) in ch:
                            dst = ps_u if k == NB - 1 else ps_n
                            start = mc == 0 and k in (0, NB - 1)
                            stop = mc == 2 and k in (NB - 2, NB - 1)
                            if mc < 2:
                                nc.tensor.matmul(dst[:], xb[:, k, mc, :],
                                                 gta[:, mc, k, :],
                                                 start=start, stop=stop)
                            else:
                                nc.tensor.matmul(dst[:],
                                                 xbc2[off:off + rem, k, :],
                                                 gtr[off:off + rem, k, :],
                                                 start=start, stop=stop)
                        if lastq:
                            sl = slice(pr * U, (pr + 1) * U)
                            bal.copy(m2[:, 0, sl], ps_n[:], U)
                            bal.copy(m2[:, 1, sl], ps_u[:], U)
                            for piece in pat[pr]:
                                pending.append((xc, xn, m2, piece))
                    qs.append(aq)
                return qs

            from collections import deque
            pending = []        # GRU pieces awaiting issue
            fillers = deque()   # flip/agg quanta awaiting interleave
            stq = deque()       # piece stages dripped one per wave
            GROUPS = ((0, 1), (2, 3), (4, 5), (6, 7))
            for p in range(PASSES):
                last = p == PASSES - 1
                pat = pieces_at
                x_next = xp.tile([128, NP], F16, tag="x")
                m2 = mtp.tile([128, 2, NP], F16, tag="m2")

                for pg, grp in enumerate(GROUPS):
                    G = len(grp)
                    # prefetch next group's adjacency (one group ahead)
                    if pg + 1 < len(GROUPS):
                        nxt = [(p, pn_) for pn_ in GROUPS[pg + 1]]
                    else:
                        nxt = [(p + 1, pn_) for pn_ in GROUPS[0]]
                    for pp, pn in nxt:
                        if pp < PASSES:
                            gta = gtp.tile([128, 2, NB, U], F16, tag="gta")
                            nc.sync.dma_start(gta[:], gPa_d.ap()[:, pn])
                            gtr = gtp.tile([64, NB, U], F16, tag="gtr")
                            o = 32 * (pn % 2)
                            nc.sync.dma_start(gtr[o:o + rem], gPr_d.ap()[:, pn])
                            gtiles[pn] = (gta, gtr)

                    # all still-pending pieces must land before this group's
                    # first wave reads their output columns
                    while pending:
                        issue_pieces([pending.pop(0)], False)

                    # ---- bond MLPs: G pairs per PSUM tile, waves over bonds;
                    # the previous group's flips/aggs and older GRU pieces are
                    # interleaved between waves to keep every engine fed ----
                    curs = [[x_cur[:, pr * U:(pr + 1) * U]] * NB for pr in grp]
                    x7t = [None] * NB
                    per_slot = len(fillers) / ((NL - 1) * NB + 4)
                    credit = 0.0
                    for l in range(NL - 1):
                        outs = [[] for _ in grp]
                        for k in range(NB):
                            if l == NL - 2:
                                nt_ = x7p.tile([128, G, U], F16, tag="x7")
                            else:
                                nt_ = mp.tile([128, G, U], F16, tag="mlp")
                            if l == NL - 2:
                                x7t[k] = nt_
                            ps = mpsp.tile([128, G, 512], F32, tag="mps")
                            for j in range(G):
                                nc.tensor.matmul(ps[:, j, :U],
                                                 mwT06[:, k, l, :],
                                                 curs[j][k],
                                                 start=True, stop=True)
                            bal.relu(nt_[:], ps[:, :, :U], G * U)
                            for j in range(G):
                                outs[j].append(nt_[:, j, :])
                            credit += per_slot
                            for _ in range(2):
                                if stq:
                                    stq.popleft()()
                            while credit >= 1.0 and fillers:
                                fillers.popleft()()
                                credit -= 1.0
                        curs = outs
                        while pending:
                            stq.extend(piece_stages(pending.pop(0), False))

                    while fillers:
                        fillers.popleft()()
                    while stq:
                        stq.popleft()()
                    xbs = [xbp.tile([128, NB, 2, 128], F16, tag="xb",
                                    name="xb") for _ in grp]
                    xbc2 = xbp.tile([64, NB, 128], F16, tag="xbc2", bufs=2,
                                    name="xbc2")
                    for j, pr in enumerate(grp):
                        fillers.extend(flip_quanta(pr, j, x7t, xbs[j]))
                    fillers.extend(c2_quantum(grp, x7t, xbc2))
                    for j, pr in enumerate(grp):
                        fillers.extend(
                            agg_quanta(pr, j, xbs[j], xbc2, x_cur, x_next,
                                       m2, pat))

                    if last and pg == len(GROUPS) - 1:
                        # pairs 0-5: make sure every piece write is issued
                        # BEFORE the DMA reads those columns (issue order
                        # defines RAW vs WAR for the dependency tracker)
                        while pending:
                            issue_pieces([pending.pop(0)], False)
                        nc.sync.dma_start(y_d.ap()[:, 0:4 * U],
                                          x_next[:, 0:4 * U])

                x_cur = x_next

            while fillers:
                fillers.popleft()()
            while pending:
                issue_pieces(pending[:2], True)
                pending = pending[2:]
            nc.sync.dma_start(y_d.ap()[:, 4 * U:], x_cur[:, 4 * U:])

    nc.compile()
    return nc


def _make_runner(nc):
    import jax
    from jax.experimental.shard_map import shard_map
    from jax.sharding import Mesh, PartitionSpec, NamedSharding
    from concourse.bass2jax import (install_neuronx_cc_hook, _bass_exec_p,
                                    partition_id_tensor)

    install_neuronx_cc_hook()
    partition_name = (nc.partition_id_tensor.name
                      if nc.partition_id_tensor else None)
    in_names, out_names, out_avals, zero_outs = [], [], [], []
    for alloc in nc.m.functions[0].allocations:
        if not isinstance(alloc, mybir.MemoryLocationSet):
            continue
        name = alloc.memorylocations[0].name
        if alloc.kind == "ExternalInput":
            if name != partition_name:
                in_names.append(name)
        elif alloc.kind == "ExternalOutput":
            out_names.append(name)
            shape = tuple(alloc.tensor_shape)
            dtype = mybir.dt.np(alloc.dtype)
            out_avals.append(jax.core.ShapedArray(shape, dtype))
            zero_outs.append(np.zeros(shape, dtype))
    n_params = len(in_names)
    all_names = in_names + out_names
    if partition_name is not None:
        all_names = all_names + [partition_name]

    def _body(*args):
        operands = list(args)
        if partition_name is not None:
            operands.append(partition_id_tensor())
        outs = _bass_exec_p.bind(
            *operands,
            out_avals=tuple(out_avals),
            in_names=tuple(all_names),
            out_names=tuple(out_names),
            lowering_input_output_aliases=(),
            sim_require_finite=True,
            sim_require_nnan=True,
            nc=nc,
        )
        return tuple(outs)

    devices = jax.devices()[:M]
    mesh = Mesh(np.asarray(devices), ("core",))
    specs = (PartitionSpec("core"),) * (n_params + len(out_names))
    fn = jax.jit(shard_map(_body, mesh=mesh,
                           in_specs=specs,
                           out_specs=(PartitionSpec("core"),) * len(out_names)),
                 keep_unused=True)

    def put(in_maps):
        sh = NamedSharding(mesh, PartitionSpec("core"))
        args = []
        for name in in_names:
            cat = np.concatenate([np.asarray(im[name]) for im in in_maps], axis=0)
            args.append(jax.device_put(cat, sh))
        for z in zero_outs:
            cat = np.concatenate([z] * M, axis=0)
            args.append(jax.device_put(cat, sh))
        return args

    def run(args):
        outs = fn(*args)
        outs = [np.asarray(o) for o in outs]
        per_core = []
        for c in range(M):
            per_core.append({
                name: outs[i].reshape(M, *out_avals[i].shape)[c]
                for i, name in enumerate(out_names)})
        return per_core

    return put, run


_CACHE = {}


def _get_runner(meta):
    if meta not in _CACHE:
        nc = _build(meta)
        _CACHE[meta] = (_make_runner(nc), nc)
    return _CACHE[meta]


def _assemble(per_core, placements):
    out = np.empty((B, N, D), np.float32)
    for c in range(M):
        y = np.asarray(per_core[c]["y"], np.float32)   # [D, NP] padded transposed
        gids, pos = placements[c]
        out[gids] = y.T[pos]
    return out


def kernel(g, h, msg_W, gru_Wih, gru_Whh, gru_bih, gru_bhh):
    in_maps, meta, placements = _prepare(g, h, msg_W, gru_Wih, gru_Whh,
                                         gru_bih, gru_bhh)
    (put, run), _nc = _get_runner(meta)
    args = put(in_maps)
    per_core = run(args)
    return _assemble(per_core, placements)


# exposed for test.py
def get_nc_and_runner(g, h, msg_W, gru_Wih, gru_Whh, gru_bih, gru_bhh):
    in_maps, meta, placements = _prepare(g, h, msg_W, gru_Wih, gru_Whh,
                                         gru_bih, gru_bhh)
    (put, run), nc = _get_runner(meta)
    return in_maps, put, run, nc, placements



# revision 81
# speedup vs baseline: 1.0095x; 1.0016x over previous
"""Trainium2 Bass kernel for nn_Big_MPNN (gnn_message_passing).

Self-contained: hardcodes shapes/sharding. Data-parallel over the batch dim
across 8 NeuronCores (16 graphs per core), weights replicated; no collectives.

Node layout: the host pairs graphs to BALANCE per-type counts (local search
minimizing sum of per-type max counts over pairs), then sorts nodes by GRU
atom-type within each pair. Each pair occupies exactly U = sum(caps) columns
(no dead padding); per-type capacities are uniform across all pairs/cores so
every per-type GRU matmul reads a static strided access pattern.

Per-core dataflow (3 passes), transposed activations [D=128 part, cols],
all f16 except PSUM/biases/final cast.  Pairs are processed in groups of two;
per (layer, bond) wave one 2-bank PSUM tile holds both pairs and is drained
by a single ReLU op, load-balanced between ACT and DVE (GPSIMD cannot read
PSUM; it gets the SBUF-only f16 GRU elementwise ops instead).  Each group's
layer-7 flip (chunks 128/128/rem -> normal-layout xb) and aggregation
m^T = xb^T g^T are split into small matmul quanta and paced between the NEXT
group's MLP wave tiles, so the tensor engine never runs long drain-free
stretches.  GRU pieces merge both GRU universes into single wide elementwise
ops (messages in one [128, 2, NP] tile; tanh and the blend each issued once
per piece over [2, ncols]); each piece is expanded into 9 dependency-ordered
stage closures dripped two per wave so no engine's in-order stream blocks on
an op whose inputs resolve late (ACT/DVE have no exec-queue lookahead).  The
final pass drains its last pieces stage-interleaved with a DVE-only tail and
ships y in two DMA halves; pieces must be ISSUED before a DMA that reads
their columns (issue order defines RAW vs WAR for the dependency tracker).
Host unpads/unpermutes the f16 result.
"""

import numpy as np

import concourse.bass as bass
import concourse.bacc as bacc
import concourse.tile as tile
import concourse.mybir as mybir

F32 = mybir.dt.float32
F16 = mybir.dt.float16
AF = mybir.ActivationFunctionType
ALU = mybir.AluOpType

M = 8                      # cores
B, N, FEAT, D = 128, 128, 75, 128
NB, NL, NT = 7, 8, 6       # bonds, mlp layers, gru type slots
PASSES = 3
BG = B // M                # graphs per core
NPAIR = BG // 2            # graph pairs per core (8)
TOP_ATOMS = [6.0, 7.0, 8.0, 9.0, 0.0]


def _pair_graphs(cnt):
    """Pair the B graphs to minimize sum_t max_pairs(count_t).  cnt: [B, NT]."""
    P = B // 2
    order = np.argsort(cnt[:, NT - 1], kind="stable")
    pairs = np.stack([order[:P], order[:P - 1:-1]], 1)
    rng = np.random.default_rng(12345)

    def obj(pr):
        pc = cnt[pr[:, 0]] + cnt[pr[:, 1]]
        s = np.sort(pc, 0)[::-1]
        return s[0].sum() * 1000 + s[1].sum() * 10 + s[2].sum()

    cur = pairs.copy()
    co = obj(cur)
    best, bo = cur.copy(), co
    for _ in range(150000):
        i, j = rng.integers(0, P, 2)
        if i == j:
            continue
        trial = cur.copy()
        a1, b1 = trial[i]
        a2, b2 = trial[j]
        if rng.integers(0, 2) == 0:
            trial[i] = (a1, a2)
            trial[j] = (b1, b2)
        else:
            trial[i] = (a1, b2)
            trial[j] = (a2, b1)
        to = obj(trial)
        if to <= co:
            cur, co = trial, to
            if to < bo:
                best, bo = trial.copy(), to
    return best


def _prepare(g, h, msg_W, gru_Wih, gru_Whh, gru_bih, gru_bhh):
    g = np.ascontiguousarray(np.asarray(g, np.float32))
    h = np.ascontiguousarray(np.asarray(h, np.float32))
    msg_W = np.asarray(msg_W, np.float32)
    gru_Wih = np.asarray(gru_Wih, np.float32).reshape(2, NT, 3, D, D)
    gru_Whh = np.asarray(gru_Whh, np.float32).reshape(2, NT, 3, D, D)
    gru_bih = np.asarray(gru_bih, np.float32).reshape(2, NT, 3, D)
    gru_bhh = np.asarray(gru_bhh, np.float32).reshape(2, NT, 3, D)

    atoms = h[:, :, 0]
    tid = np.full((B, N), NT - 1, np.int32)
    for i, a in enumerate(TOP_ATOMS):
        tid[atoms == np.float32(a)] = i
    cnt = np.stack([(tid == t).sum(1) for t in range(NT)], 1).astype(np.int64)

    pairs = _pair_graphs(cnt)                       # [64, 2] graph ids
    pc = cnt[pairs[:, 0]] + cnt[pairs[:, 1]]
    caps = tuple(int(c) for c in pc.max(axis=0))
    U = sum(caps)
    assert 256 < U <= 384, f"caps {caps} sum {U} out of supported range"
    rem = U - 256
    NP = NPAIR * U
    offs = np.cumsum([0] + list(caps))[:-1]

    # replicated weights, partition-major f16 layouts
    mwT = np.transpose(msg_W, (3, 0, 1, 2))         # [din, k, l, dout]
    mwT06 = np.ascontiguousarray(mwT[:, :, :NL - 1]).astype(np.float16)
    mw8T = np.ascontiguousarray(mwT[:, :, NL - 1]).astype(np.float16)
    wihT = np.ascontiguousarray(
        np.transpose(gru_Wih, (4, 0, 1, 2, 3))).astype(np.float16)
    whhT = np.ascontiguousarray(
        np.transpose(gru_Whh, (4, 0, 1, 2, 3))).astype(np.float16)
    brz = np.ascontiguousarray(
        np.transpose(gru_bih[:, :, :2] + gru_bhh[:, :, :2], (3, 0, 1, 2)))
    brzM = np.ascontiguousarray(
        np.transpose(brz, (1, 2, 3, 0)).reshape(1, 2 * NT * 2, D)
    ).astype(np.float16)
    ones1 = np.ones((1, 512), np.float16)
    binn = np.ascontiguousarray(np.transpose(gru_bih[:, :, 2], (2, 0, 1)))
    bhnn = np.ascontiguousarray(np.transpose(gru_bhh[:, :, 2], (2, 0, 1)))

    h_t = np.concatenate([h, np.zeros((B, N, D - FEAT), np.float32)], axis=2)

    in_maps = []
    placements = []     # per core: (gids [BG], pos [BG, N])
    for c in range(M):
        gids = pairs[c * NPAIR:(c + 1) * NPAIR].reshape(-1)
        pos = np.zeros((BG, N), np.int64)
        x0 = np.zeros((NP, D), np.float32)
        gPa = np.zeros((128, NPAIR, 2, NB, U), np.float32)
        gPr = np.zeros((rem, NPAIR, NB, U), np.float32)
        for p in range(NPAIR):
            ga, gb = gids[2 * p], gids[2 * p + 1]
            tp = np.concatenate([tid[ga], tid[gb]])            # [256]
            hp = np.concatenate([h_t[ga], h_t[gb]], axis=0)    # [256, D]
            ppos = np.zeros(2 * N, np.int64)
            for t in range(NT):
                idx = np.flatnonzero(tp == t)
                ppos[idx] = offs[t] + np.arange(len(idx))
            pos[2 * p] = p * U + ppos[:N]
            pos[2 * p + 1] = p * U + ppos[N:]
            x0[p * U + ppos] = hp
            # dense pair block: big[m_row, k, n_col] = g[graph, k, n, m]
            big = np.zeros((U, NB, U), np.float32)
            for gi, gr in enumerate((ga, gb)):
                lg = ppos[gi * N:(gi + 1) * N]
                blk = np.transpose(g[gr], (2, 0, 1))           # [m, k, n]
                big[np.ix_(lg, np.arange(NB), lg)] = blk
            gPa[:, p, 0] = np.transpose(big[:128], (0, 1, 2))
            gPa[:, p, 1] = big[128:256]
            gPr[:, p] = big[256:U]
        placements.append((gids.copy(), pos))
        in_maps.append(dict(
            gPa=gPa.astype(np.float16),
            gPr=gPr.astype(np.float16),
            x0=np.ascontiguousarray(x0.T).astype(np.float16),
            mwT06=mwT06, mw8T=mw8T, wihT=wihT, whhT=whhT,
            brz=brz, binn=binn, bhnn=bhnn,
            brzM=brzM, ones1=ones1,
        ))
    meta = (caps, U)
    return in_maps, meta, placements


class _Balancer:
    """Greedy per-engine load balancer for drain/elementwise ops."""

    def __init__(self, nc):
        self.nc = nc
        self.load = {"A": 0.0, "D": 0.0, "P": 0.0}

    def _cost(self, e, op, cols, psum_src, f16_sbuf):
        # Exact TimelineSim engine-busy costs: processing = cols*cycle_t +
        # max-over-operands(2*access_cycles)/2 * cycle_t (SBUF dst dominates).
        if e == "A":
            return cols * 0.8333 + 185.0
        if e == "D":
            if f16_sbuf:
                return cols * 0.521 + 60.0
            return cols * 1.0417 + 125.0
        eff = 0.42 if op in ("add", "sub", "mul") else 0.6
        return cols * 0.8333 / eff + 131.0

    def pick(self, op, cols, psum_src=True, f16_sbuf=False, allow=("A", "D")):
        cand = [(self.load[e] + self._cost(e, op, cols, psum_src, f16_sbuf), e)
                for e in allow]
        _, e = min(cand)
        self.load[e] += self._cost(e, op, cols, psum_src, f16_sbuf)
        return e

    def charge(self, e, op, cols, psum_src=True, f16_sbuf=False):
        self.load[e] += self._cost(e, op, cols, psum_src, f16_sbuf)

    # PSUM sources: GPSIMD has no PSUM access -> ACT/DVE only.
    def relu(self, out, ps, cols):
        e = self.pick("relu", cols)
        if e == "A":
            self.nc.scalar.activation(out, ps, AF.Relu)
        else:
            self.nc.vector.tensor_scalar_max(out, ps, 0.0)

    def copy(self, out, ps, cols):
        e = self.pick("copy", cols)
        if e == "A":
            self.nc.scalar.copy(out, ps)
        else:
            self.nc.vector.tensor_copy(out, ps)

    def stt(self, out, in0, scal, in1, op0, op1, cols):
        self.charge("D", "stt", cols)
        self.nc.vector.scalar_tensor_tensor(out, in0, scal, in1,
                                            op0=op0, op1=op1)

    # SBUF-only f16 elementwise: DVE or Pool.
    def tt(self, op, out, a, b, cols, f16_sbuf=True, allow=("P",)):
        e = self.pick(op, cols, psum_src=False, f16_sbuf=f16_sbuf, allow=allow)
        eng = self.nc.vector if e == "D" else self.nc.gpsimd
        getattr(eng, "tensor_" + op)(out, a, b)


def _build(meta):
    caps, U = meta
    rem = U - 256
    NP = NPAIR * U
    nc = bacc.Bacc("TRN2", target_bir_lowering=False, debug=False, num_devices=M)

    gPa_d = nc.dram_tensor("gPa", [128, NPAIR, 2, NB, U], F16, kind="ExternalInput")
    gPr_d = nc.dram_tensor("gPr", [rem, NPAIR, NB, U], F16, kind="ExternalInput")
    x0_d = nc.dram_tensor("x0", [128, NP], F16, kind="ExternalInput")
    mwT06_d = nc.dram_tensor("mwT06", [128, NB, NL - 1, 128], F16, kind="ExternalInput")
    mw8T_d = nc.dram_tensor("mw8T", [128, NB, 128], F16, kind="ExternalInput")
    wih_d = nc.dram_tensor("wihT", [128, 2, NT, 3, 128], F16, kind="ExternalInput")
    whh_d = nc.dram_tensor("whhT", [128, 2, NT, 3, 128], F16, kind="ExternalInput")
    brz_d = nc.dram_tensor("brz", [128, 2, NT, 2], F32, kind="ExternalInput")
    brzM_d = nc.dram_tensor("brzM", [1, 2 * NT * 2, 128], F16, kind="ExternalInput")
    ones_d = nc.dram_tensor("ones1", [1, 512], F16, kind="ExternalInput")
    binn_d = nc.dram_tensor("binn", [128, 2, NT], F32, kind="ExternalInput")
    bhnn_d = nc.dram_tensor("bhnn", [128, 2, NT], F32, kind="ExternalInput")
    y_d = nc.dram_tensor("y", [128, NP], F16, kind="ExternalOutput")

    # GRU pieces: (type, col-offset, pair0, n_pairs); issued after pair p0+npr-1
    # The final pass splits the second half into npr=2 pieces so the tail only
    # waits on the last two pairs' aggregation.
    pieces_at = {pr: [] for pr in range(NPAIR)}
    pieces_at_final = {pr: [] for pr in range(NPAIR)}
    off = 0
    for t in range(NT):
        if caps[t] == 0:
            continue
        npr = min(4, max(1, 256 // caps[t]))
        while NPAIR % npr:
            npr -= 1
        for p0 in range(0, NPAIR, npr):
            pieces_at[p0 + npr - 1].append((t, off, p0, npr))
            if p0 < NPAIR // 2 or npr <= 2:
                pieces_at_final[p0 + npr - 1].append((t, off, p0, npr))
            else:
                for q0 in range(p0, p0 + npr, 2):
                    pieces_at_final[q0 + 1].append((t, off, q0, 2))
        off += caps[t]

    with tile.TileContext(nc) as tc:
        with (
            tc.tile_pool(name="const", bufs=1) as cp,
            tc.tile_pool(name="xp", bufs=2) as xp,
            tc.tile_pool(name="mlp", bufs=24) as mp,
            tc.tile_pool(name="x7p", bufs=10) as x7p,
            tc.tile_pool(name="xbp", bufs=3) as xbp,
            tc.tile_pool(name="gtp", bufs=5) as gtp,
            tc.tile_pool(name="mtp", bufs=2) as mtp,
            tc.tile_pool(name="gates", bufs=32) as ggp,
            tc.tile_pool(name="mps", bufs=3, space="PSUM") as mpsp,
            tc.tile_pool(name="ps", bufs=2, space="PSUM") as psp,
        ):
            bal = _Balancer(nc)

            x_cur = xp.tile([128, NP], F16, tag="x")
            mwT06 = cp.tile([128, NB, NL - 1, 128], F16, tag="mwT06")
            nc.sync.dma_start(x_cur[:, 0:U], x0_d.ap()[:, 0:U])
            nc.sync.dma_start(mwT06[:, :, 0:1, :], mwT06_d.ap()[:, :, 0:1, :])
            nc.sync.dma_start(x_cur[:, U:3 * U], x0_d.ap()[:, U:3 * U])
            nc.sync.dma_start(mwT06[:, :, 1:, :], mwT06_d.ap()[:, :, 1:, :])
            nc.sync.dma_start(x_cur[:, 3 * U:6 * U], x0_d.ap()[:, 3 * U:6 * U])
            nc.sync.dma_start(x_cur[:, 6 * U:], x0_d.ap()[:, 6 * U:])

            gtiles = {}
            for pn in (0, 1):
                gta0 = gtp.tile([128, 2, NB, U], F16, tag="gta")
                nc.sync.dma_start(gta0[:], gPa_d.ap()[:, pn])
                gtr0 = gtp.tile([64, NB, U], F16, tag="gtr")
                o = 32 * (pn % 2)
                nc.sync.dma_start(gtr0[o:o + rem], gPr_d.ap()[:, pn])
                gtiles[pn] = (gta0, gtr0)

            mw8T = cp.tile([128, NB, 128], F16, tag="mw8T")
            wih = cp.tile([128, 2, NT, 3, 128], F16, tag="wih")
            whh = cp.tile([128, 2, NT, 3, 128], F16, tag="whh")
            brz = cp.tile([128, 2, NT, 2], F32, tag="brz")
            brzM = cp.tile([1, 2 * NT * 2, 128], F16, tag="brzM")
            ones1 = cp.tile([1, 512], F16, tag="ones1")
            binn = cp.tile([128, 2, NT], F32, tag="binn")
            bhnn = cp.tile([128, 2, NT], F32, tag="bhnn")
            nc.sync.dma_start(mw8T[:], mw8T_d.ap())
            nc.sync.dma_start(wih[:], wih_d.ap())
            nc.sync.dma_start(whh[:], whh_d.ap())
            nc.sync.dma_start(brz[:], brz_d.ap())
            nc.sync.dma_start(brzM[:], brzM_d.ap())
            nc.sync.dma_start(ones1[:], ones_d.ap())
            nc.sync.dma_start(binn[:], binn_d.ap())
            nc.sync.dma_start(bhnn[:], bhnn_d.ap())

            def seg(tile_, t_off, p0, npr, w):
                return tile_[:].rearrange("d (pr u) -> d pr u", u=U)[
                    :, p0:p0 + npr, t_off:t_off + w]

            def seg_m(m2, u, t_off, p0, npr, w):
                return m2[:, u, :].rearrange("d (pr u2) -> d pr u2", u2=U)[
                    :, p0:p0 + npr, t_off:t_off + w]

            def seg2(m2, t_off, p0, npr, w):
                """4D view of the merged [128, 2, NP] message tile:
                [128, u, pair, col]."""
                return m2[:].rearrange("d u (pr u2) -> d u pr u2", u2=U)[
                    :, :, p0:p0 + npr, t_off:t_off + w]

            def piece_stages(args, fast_tail):
                """Stage closures for ONE GRU piece.  Dripping one stage per
                wave keeps each engine's in-order stream free of ops whose
                dependencies resolve late (head-of-line blocking: ACT/DVE have
                no exec-queue lookahead, so a stalled op blocks later ones)."""
                (xc, xn, m2, piece) = args
                t, t_off, p0, npr = piece
                w = caps[t]
                ncols = npr * w
                s = dict(xs=seg(xc, t_off, p0, npr, w),
                         ms=[seg_m(m2, 0, t_off, p0, npr, w),
                             seg_m(m2, 1, t_off, p0, npr, w)],
                         ms2=seg2(m2, t_off, p0, npr, w))
                tail = ("D",) if fast_tail else ("P",)

                def st_mm():
                    s["prz"], s["pn2"] = [], []
                    for u in range(2):
                        prz = psp.tile([128, 2, 256], F32, tag="ps",
                                       name="prz")
                        pool2 = mpsp if fast_tail else psp
                        pn2 = pool2.tile([128, 2, 256], F32,
                                         tag="mps" if fast_tail else "ps",
                                         name="pn2")
                        for gi in range(2):
                            nc.tensor.matmul(prz[:, gi, :ncols],
                                             wih[:, u, t, gi, :], s["xs"],
                                             start=True, stop=False)
                            nc.tensor.matmul(prz[:, gi, :ncols],
                                             whh[:, u, t, gi, :], s["ms"][u],
                                             start=False, stop=False)
                            row = (u * NT + t) * 2 + gi
                            nc.tensor.matmul(prz[:, gi, :ncols],
                                             brzM[0:1, row, :],
                                             ones1[0:1, :ncols],
                                             start=False, stop=True)
                        nc.tensor.matmul(pn2[:, 0, :ncols], wih[:, u, t, 2, :],
                                         s["xs"], start=True, stop=True)
                        nc.tensor.matmul(pn2[:, 1, :ncols], whh[:, u, t, 2, :],
                                         s["ms"][u], start=True, stop=True)
                        s["prz"].append(prz)
                        s["pn2"].append(pn2)
                    s["rzb"] = ggp.tile([128, 2, 2, 256], F16,
                                        tag="gt4", name="rzb", bufs=6)

                def st_sig(u):
                    nc.scalar.activation(s["rzb"][:, u, :, :ncols],
                                         s["prz"][u][:, :, :ncols],
                                         AF.Sigmoid)
                    bal.charge("A", "act", 2 * ncols)

                def st_t1():
                    t12 = ggp.tile([128, 2, 256], F16, tag="gt2", name="t12",
                                   bufs=17)
                    s["t12"] = t12
                    for u in range(2):
                        bal.stt(t12[:, u, :ncols], s["pn2"][u][:, 1, :ncols],
                                bhnn[:, u, t:t + 1], s["rzb"][:, u, 0, :ncols],
                                ALU.add, ALU.mult, ncols)

                def st_na():
                    na2 = ggp.tile([128, 2, 256], F16, tag="gt2", name="na2",
                                   bufs=17)
                    s["na2"] = na2
                    for u in range(2):
                        bal.stt(na2[:, u, :ncols], s["pn2"][u][:, 0, :ncols],
                                binn[:, u, t:t + 1], s["t12"][:, u, :ncols],
                                ALU.add, ALU.add, ncols)

                def st_tanh():
                    n2 = ggp.tile([128, 2, 256], F16, tag="gt2", name="n2",
                                  bufs=17)
                    nc.scalar.activation(n2[:, :, :ncols],
                                         s["na2"][:, :, :ncols], AF.Tanh)
                    bal.charge("A", "act", 2 * ncols, psum_src=False)
                    s["n2"] = n2

                def st_d2():
                    d2 = ggp.tile([128, 2, 256], F16, tag="gt2", name="d2",
                                  bufs=17)
                    n2v = s["n2"][:, :, :ncols].rearrange(
                        "d u (pr w) -> d u pr w", w=w)
                    d2v = d2[:, :, :ncols].rearrange(
                        "d u (pr w) -> d u pr w", w=w)
                    bal.tt("sub", d2v, s["ms2"], n2v, 2 * ncols, allow=tail)
                    s["d2"] = d2

                def st_e2():
                    e2 = ggp.tile([128, 2, 256], F16, tag="gt2", name="e2",
                                  bufs=17)
                    bal.tt("mul", e2[:, :, :ncols],
                           s["rzb"][:, :, 1, :ncols],
                           s["d2"][:, :, :ncols], 2 * ncols, allow=tail)
                    s["e2"] = e2

                def st_hu():
                    hu2 = ggp.tile([128, 2, 256], F16, tag="gt2", name="hu2",
                                   bufs=17)
                    bal.tt("add", hu2[:, :, :ncols], s["n2"][:, :, :ncols],
                           s["e2"][:, :, :ncols], 2 * ncols, allow=tail)
                    hv = hu2[:, :, :ncols].rearrange(
                        "d u (pr w) -> d u pr w", w=w)
                    bal.tt("add", seg(xn, t_off, p0, npr, w),
                           hv[:, 0], hv[:, 1],
                           ncols, allow=("D",) if fast_tail else ("P",))

                return [st_mm, lambda: st_sig(0), lambda: st_sig(1),
                        st_t1, st_na, st_tanh, st_d2, st_e2, st_hu]

            def issue_pieces(batch, fast_tail):
                """Issue whole pieces, stage-interleaved across the batch."""
                stl = [piece_stages(a, fast_tail) for a in batch]
                for i in range(max(len(sl) for sl in stl)):
                    for sl in stl:
                        if i < len(sl):
                            sl[i]()

            def flip_quanta(pr, i, x7t, xb):
                """Per-pair layer-8 flip quanta, chunks 0-1 only (the rem
                chunk of BOTH pairs goes into one shared group tile)."""
                qs = []
                for k0, kn in ((0, 2), (2, 2), (4, 2), (6, 1)):
                    def fq(k0=k0, kn=kn):
                        kk = list(range(k0, k0 + kn))
                        ps3 = mpsp.tile([128, len(kk), 2, 128], F32, tag="mps",
                                        name="ps3")
                        for j, k in enumerate(kk):
                            nc.tensor.matmul(ps3[:, j, 0, :],
                                             x7t[k][:, i, 0:128],
                                             mw8T[:, k, :],
                                             start=True, stop=True)
                            nc.tensor.matmul(ps3[:, j, 1, :],
                                             x7t[k][:, i, 128:256],
                                             mw8T[:, k, :],
                                             start=True, stop=True)
                        bal.copy(xb[:, k0:k0 + len(kk), :, :], ps3[:],
                                 len(kk) * 256)
                    qs.append(fq)
                return qs

            def c2_quantum(grp, x7t, xbc2):
                """Both pairs' rem-chunk flips into ONE PSUM tile (pair 1 at
                partition 32 via PE col-tiling, auto-derived from the out AP's
                base partition) and a single 896-col drain."""
                def cq():
                    ps3c = mpsp.tile([128, NB, 128], F32, tag="mps",
                                     name="ps3c")
                    for i in range(len(grp)):
                        off = 32 * i
                        for k in range(NB):
                            nc.tensor.matmul(ps3c[off:off + rem, k, :],
                                             x7t[k][:, i, 256:U],
                                             mw8T[:, k, :],
                                             start=True, stop=True)
                    bal.copy(xbc2[0:32 + rem, :, :], ps3c[0:32 + rem, :, :],
                             NB * 128)
                return [cq]

            def agg_quanta(pr, i, xb, xbc2, xc, xn, m2, pat,
                           accf=None):
                """Aggregation quanta for one pair; mc=2 reads the shared rem
                tile at base partition 32*i (gtr rows DMA'd to match).  With
                accf (final group only, runs wholly in the epilogue): both
                pairs accumulate into one shared 2-bank tile so pair 7 never
                waits on pair 6's psp slot, and the m2 copies merge."""
                cell = {}

                def ps_():
                    if 'ps' not in cell:
                        if accf is not None:
                            cell['ps'] = (accf[:, 0, :U], accf[:, 1, :U])
                        else:
                            cell['ps'] = (psp.tile([128, U], F32, tag="ps",
                                                   name="ps_n"),
                                          psp.tile([128, U], F32, tag="ps",
                                                   name="ps_u"))
                    return cell['ps']

                off = 32 * i
                qs = []
                steps = [(mc, k) for mc in range(3) for k in range(NB)]
                chunks = [steps[j:j + 4] for j in range(0, len(steps), 4)]
                for ci, ch in enumerate(chunks):
                    def aq(ch=ch, lastq=(ci == len(chunks) - 1)):
                        ps_n, ps_u = ps_()
                        gta, gtr = gtiles[pr]
                        for (mc, k) in ch:
                            dst = ps_u if k == NB - 1 else ps_n
                            start = mc == 0 and k in (0, NB - 1)
                            stop = mc == 2 and k in (NB - 2, NB - 1)
                            if mc < 2:
                                nc.tensor.matmul(dst[:], xb[:, k, mc, :],
                                                 gta[:, mc, k, :],
                                                 start=start, stop=stop)
                            else:
                                nc.tensor.matmul(dst[:],
                                                 xbc2[off:off + rem, k, :],
                                                 gtr[off:off + rem, k, :],
                                                 start=start, stop=stop)
                        if not lastq:
                            return
                        sl = slice(pr * U, (pr + 1) * U)
                        bal.copy(m2[:, 0, sl], ps_n[:], U)
                        bal.copy(m2[:, 1, sl], ps_u[:], U)
                        for piece in pat[pr]:
                            pending.append((xc, xn, m2, piece))
                    qs.append(aq)
                return qs

            from collections import deque
            pending = []        # GRU pieces awaiting issue
            fillers = deque()   # flip/agg quanta awaiting interleave
            stq = deque()       # piece stages dripped one per wave
            GROUPS = ((0, 1), (2, 3), (4, 5), (6, 7))
            for p in range(PASSES):
                last = p == PASSES - 1
                pat = pieces_at
                x_next = xp.tile([128, NP], F16, tag="x")
                m2 = mtp.tile([128, 2, NP], F16, tag="m2")

                for pg, grp in enumerate(GROUPS):
                    G = len(grp)
                    # prefetch next group's adjacency (one group ahead)
                    if pg + 1 < len(GROUPS):
                        nxt = [(p, pn_) for pn_ in GROUPS[pg + 1]]
                    else:
                        nxt = [(p + 1, pn_) for pn_ in GROUPS[0]]
                    for pp, pn in nxt:
                        if pp < PASSES:
                            gta = gtp.tile([128, 2, NB, U], F16, tag="gta")
                            nc.sync.dma_start(gta[:], gPa_d.ap()[:, pn])
                            gtr = gtp.tile([64, NB, U], F16, tag="gtr")
                            o = 32 * (pn % 2)
                            nc.sync.dma_start(gtr[o:o + rem], gPr_d.ap()[:, pn])
                            gtiles[pn] = (gta, gtr)

                    # all still-pending pieces must land before this group's
                    # first wave reads their output columns
                    while pending:
                        issue_pieces([pending.pop(0)], False)

                    # ---- bond MLPs: G pairs per PSUM tile, waves over bonds;
                    # the previous group's flips/aggs and older GRU pieces are
                    # interleaved between waves to keep every engine fed ----
                    curs = [[x_cur[:, pr * U:(pr + 1) * U]] * NB for pr in grp]
                    x7t = [None] * NB
                    per_slot = len(fillers) / ((NL - 1) * NB + 4)
                    credit = 0.0
                    for l in range(NL - 1):
                        outs = [[] for _ in grp]
                        for k in range(NB):
                            if l == NL - 2:
                                nt_ = x7p.tile([128, G, U], F16, tag="x7")
                            else:
                                nt_ = mp.tile([128, G, U], F16, tag="mlp")
                            if l == NL - 2:
                                x7t[k] = nt_
                            ps = mpsp.tile([128, G, 512], F32, tag="mps")
                            for j in range(G):
                                nc.tensor.matmul(ps[:, j, :U],
                                                 mwT06[:, k, l, :],
                                                 curs[j][k],
                                                 start=True, stop=True)
                            bal.relu(nt_[:], ps[:, :, :U], G * U)
                            for j in range(G):
                                outs[j].append(nt_[:, j, :])
                            credit += per_slot
                            for _ in range(2):
                                if stq:
                                    stq.popleft()()
                            while credit >= 1.0 and fillers:
                                fillers.popleft()()
                                credit -= 1.0
                        curs = outs
                        while pending:
                            stq.extend(piece_stages(pending.pop(0), False))

                    while fillers:
                        fillers.popleft()()
                    while stq:
                        stq.popleft()()
                    xbs = [xbp.tile([128, NB, 2, 128], F16, tag="xb",
                                    name="xb") for _ in grp]
                    xbc2 = xbp.tile([64, NB, 128], F16, tag="xbc2", bufs=2,
                                    name="xbc2")
                    for j, pr in enumerate(grp):
                        fillers.extend(flip_quanta(pr, j, x7t, xbs[j]))
                    fillers.extend(c2_quantum(grp, x7t, xbc2))
                    fin2 = last and pg == len(GROUPS) - 1
                    for j, pr in enumerate(grp):
                        accf = None
                        if fin2 and j == 1:
                            accf = mpsp.tile([128, 2, 512], F32, tag="mps",
                                             name="accf")
                        fillers.extend(
                            agg_quanta(pr, j, xbs[j], xbc2, x_cur, x_next,
                                       m2, pat, accf))

                    if last and pg == len(GROUPS) - 1:
                        # pairs 0-5: make sure every piece write is issued
                        # BEFORE the DMA reads those columns (issue order
                        # defines RAW vs WAR for the dependency tracker)
                        while pending:
                            issue_pieces([pending.pop(0)], False)
                        nc.sync.dma_start(y_d.ap()[:, 0:4 * U],
                                          x_next[:, 0:4 * U])

                x_cur = x_next

            while fillers:
                fillers.popleft()()
            while pending:
                issue_pieces(pending[:2], True)
                pending = pending[2:]
            nc.sync.dma_start(y_d.ap()[:, 4 * U:], x_cur[:, 4 * U:])

    nc.compile()
    return nc


def _make_runner(nc):
    import jax
    from jax.experimental.shard_map import shard_map
    from jax.sharding import Mesh, PartitionSpec, NamedSharding
    from concourse.bass2jax import (install_neuronx_cc_hook, _bass_exec_p,
                                    partition_id_tensor)

    install_neuronx_cc_hook()
    partition_name = (nc.partition_id_tensor.name
                      if nc.partition_id_tensor else None)
    in_names, out_names, out_avals, zero_outs = [], [], [], []
    for alloc in nc.m.functions[0].allocations:
        if not isinstance(alloc, mybir.MemoryLocationSet):
            continue
        name = alloc.memorylocations[0].name
        if alloc.kind == "ExternalInput":
            if name != partition_name:
                in_names.append(name)
        elif alloc.kind == "ExternalOutput":
            out_names.append(name)
            shape = tuple(alloc.tensor_shape)
            dtype = mybir.dt.np(alloc.dtype)
            out_avals.append(jax.core.ShapedArray(shape, dtype))
            zero_outs.append(np.zeros(shape, dtype))
    n_params = len(in_names)
    all_names = in_names + out_names
    if partition_name is not None:
        all_names = all_names + [partition_name]

    def _body(*args):
        operands = list(args)
        if partition_name is not None:
            operands.append(partition_id_tensor())
        outs = _bass_exec_p.bind(
            *operands,
            out_avals=tuple(out_avals),
            in_names=tuple(all_names),
            out_names=tuple(out_names),
            lowering_input_output_aliases=(),
            sim_require_finite=True,
            sim_require_nnan=True,
            nc=nc,
        )
        return tuple(outs)

    devices = jax.devices()[:M]
    mesh = Mesh(np.asarray(devices), ("core",))
    specs = (PartitionSpec("core"),) * (n_params + len(out_names))
    fn = jax.jit(shard_map(_body, mesh=mesh,
                           in_specs=specs,
                           out_specs=(PartitionSpec("core"),) * len(out_names)),
                 keep_unused=True)

    def put(in_maps):
        sh = NamedSharding(mesh, PartitionSpec("core"))
        args = []
        for name in in_names:
            cat = np.concatenate([np.asarray(im[name]) for im in in_maps], axis=0)
            args.append(jax.device_put(cat, sh))
        for z in zero_outs:
            cat = np.concatenate([z] * M, axis=0)
            args.append(jax.device_put(cat, sh))
        return args

    def run(args):
        outs = fn(*args)
        outs = [np.asarray(o) for o in outs]
        per_core = []
        for c in range(M):
            per_core.append({
                name: outs[i].reshape(M, *out_avals[i].shape)[c]
                for i, name in enumerate(out_names)})
        return per_core

    return put, run


_CACHE = {}


def _get_runner(meta):
    if meta not in _CACHE:
        nc = _build(meta)
        _CACHE[meta] = (_make_runner(nc), nc)
    return _CACHE[meta]


def _assemble(per_core, placements):
    out = np.empty((B, N, D), np.float32)
    for c in range(M):
        y = np.asarray(per_core[c]["y"], np.float32)   # [D, NP] padded transposed
        gids, pos = placements[c]
        out[gids] = y.T[pos]
    return out


def kernel(g, h, msg_W, gru_Wih, gru_Whh, gru_bih, gru_bhh):
    in_maps, meta, placements = _prepare(g, h, msg_W, gru_Wih, gru_Whh,
                                         gru_bih, gru_bhh)
    (put, run), _nc = _get_runner(meta)
    args = put(in_maps)
    per_core = run(args)
    return _assemble(per_core, placements)


# exposed for test.py
def get_nc_and_runner(g, h, msg_W, gru_Wih, gru_Whh, gru_bih, gru_bhh):
    in_maps, meta, placements = _prepare(g, h, msg_W, gru_Wih, gru_Whh,
                                         gru_bih, gru_bhh)
    (put, run), nc = _get_runner(meta)
    return in_maps, put, run, nc, placements



# revision 84
# speedup vs baseline: 1.0104x; 1.0009x over previous
"""Trainium2 Bass kernel for nn_Big_MPNN (gnn_message_passing).

Self-contained: hardcodes shapes/sharding. Data-parallel over the batch dim
across 8 NeuronCores (16 graphs per core), weights replicated; no collectives.

Node layout: the host pairs graphs to BALANCE per-type counts (local search
minimizing sum of per-type max counts over pairs), then sorts nodes by GRU
atom-type within each pair. Each pair occupies exactly U = sum(caps) columns
(no dead padding); per-type capacities are uniform across all pairs/cores so
every per-type GRU matmul reads a static strided access pattern.

Per-core dataflow (3 passes), transposed activations [D=128 part, cols],
all f16 except PSUM/biases/final cast.  Pairs are processed in groups of two;
per (layer, bond) wave one 2-bank PSUM tile holds both pairs and is drained
by a single ReLU op, load-balanced between ACT and DVE (GPSIMD cannot read
PSUM; it gets the SBUF-only f16 GRU elementwise ops instead).  Each group's
layer-7 flip (chunks 128/128/rem -> normal-layout xb) and aggregation
m^T = xb^T g^T are split into small matmul quanta and paced between the NEXT
group's MLP wave tiles, so the tensor engine never runs long drain-free
stretches.  GRU pieces merge both GRU universes into single wide elementwise
ops (messages in one [128, 2, NP] tile; tanh and the blend each issued once
per piece over [2, ncols]); each piece is expanded into 9 dependency-ordered
stage closures dripped two per wave so no engine's in-order stream blocks on
an op whose inputs resolve late (ACT/DVE have no exec-queue lookahead).  The
final pass drains its last pieces stage-interleaved with a DVE-only tail and
ships y in two DMA halves; pieces must be ISSUED before a DMA that reads
their columns (issue order defines RAW vs WAR for the dependency tracker).
Host unpads/unpermutes the f16 result.
"""

import numpy as np

import concourse.bass as bass
import concourse.bacc as bacc
import concourse.tile as tile
import concourse.mybir as mybir

F32 = mybir.dt.float32
F16 = mybir.dt.float16
AF = mybir.ActivationFunctionType
ALU = mybir.AluOpType

M = 8                      # cores
B, N, FEAT, D = 128, 128, 75, 128
NB, NL, NT = 7, 8, 6       # bonds, mlp layers, gru type slots
PASSES = 3
BG = B // M                # graphs per core
NPAIR = BG // 2            # graph pairs per core (8)
TOP_ATOMS = [6.0, 7.0, 8.0, 9.0, 0.0]


def _pair_graphs(cnt):
    """Pair the B graphs to minimize sum_t max_pairs(count_t).  cnt: [B, NT]."""
    P = B // 2
    order = np.argsort(cnt[:, NT - 1], kind="stable")
    pairs = np.stack([order[:P], order[:P - 1:-1]], 1)
    rng = np.random.default_rng(12345)

    def obj(pr):
        pc = cnt[pr[:, 0]] + cnt[pr[:, 1]]
        s = np.sort(pc, 0)[::-1]
        return s[0].sum() * 1000 + s[1].sum() * 10 + s[2].sum()

    cur = pairs.copy()
    co = obj(cur)
    best, bo = cur.copy(), co
    for _ in range(150000):
        i, j = rng.integers(0, P, 2)
        if i == j:
            continue
        trial = cur.copy()
        a1, b1 = trial[i]
        a2, b2 = trial[j]
        if rng.integers(0, 2) == 0:
            trial[i] = (a1, a2)
            trial[j] = (b1, b2)
        else:
            trial[i] = (a1, b2)
            trial[j] = (a2, b1)
        to = obj(trial)
        if to <= co:
            cur, co = trial, to
            if to < bo:
                best, bo = trial.copy(), to
    return best


def _prepare(g, h, msg_W, gru_Wih, gru_Whh, gru_bih, gru_bhh):
    g = np.ascontiguousarray(np.asarray(g, np.float32))
    h = np.ascontiguousarray(np.asarray(h, np.float32))
    msg_W = np.asarray(msg_W, np.float32)
    gru_Wih = np.asarray(gru_Wih, np.float32).reshape(2, NT, 3, D, D)
    gru_Whh = np.asarray(gru_Whh, np.float32).reshape(2, NT, 3, D, D)
    gru_bih = np.asarray(gru_bih, np.float32).reshape(2, NT, 3, D)
    gru_bhh = np.asarray(gru_bhh, np.float32).reshape(2, NT, 3, D)

    atoms = h[:, :, 0]
    tid = np.full((B, N), NT - 1, np.int32)
    for i, a in enumerate(TOP_ATOMS):
        tid[atoms == np.float32(a)] = i
    cnt = np.stack([(tid == t).sum(1) for t in range(NT)], 1).astype(np.int64)

    pairs = _pair_graphs(cnt)                       # [64, 2] graph ids
    pc = cnt[pairs[:, 0]] + cnt[pairs[:, 1]]
    caps = tuple(int(c) for c in pc.max(axis=0))
    U = sum(caps)
    assert 256 < U <= 384, f"caps {caps} sum {U} out of supported range"
    rem = U - 256
    NP = NPAIR * U
    offs = np.cumsum([0] + list(caps))[:-1]

    # replicated weights, partition-major f16 layouts
    mwT = np.transpose(msg_W, (3, 0, 1, 2))         # [din, k, l, dout]
    mwT06 = np.ascontiguousarray(mwT[:, :, :NL - 1]).astype(np.float16)
    mw8T = np.ascontiguousarray(mwT[:, :, NL - 1]).astype(np.float16)
    wihT = np.ascontiguousarray(
        np.transpose(gru_Wih, (4, 0, 1, 2, 3))).astype(np.float16)
    whhT = np.ascontiguousarray(
        np.transpose(gru_Whh, (4, 0, 1, 2, 3))).astype(np.float16)
    brz = np.ascontiguousarray(
        np.transpose(gru_bih[:, :, :2] + gru_bhh[:, :, :2], (3, 0, 1, 2)))
    brzM = np.ascontiguousarray(
        np.transpose(brz, (1, 2, 3, 0)).reshape(1, 2 * NT * 2, D)
    ).astype(np.float16)
    ones1 = np.ones((1, 512), np.float16)
    binn = np.ascontiguousarray(np.transpose(gru_bih[:, :, 2], (2, 0, 1)))
    bhnn = np.ascontiguousarray(np.transpose(gru_bhh[:, :, 2], (2, 0, 1)))

    h_t = np.concatenate([h, np.zeros((B, N, D - FEAT), np.float32)], axis=2)

    in_maps = []
    placements = []     # per core: (gids [BG], pos [BG, N])
    for c in range(M):
        gids = pairs[c * NPAIR:(c + 1) * NPAIR].reshape(-1)
        pos = np.zeros((BG, N), np.int64)
        x0 = np.zeros((NP, D), np.float32)
        gPa = np.zeros((128, NPAIR, 2, NB, U), np.float32)
        gPr = np.zeros((rem, NPAIR, NB, U), np.float32)
        for p in range(NPAIR):
            ga, gb = gids[2 * p], gids[2 * p + 1]
            tp = np.concatenate([tid[ga], tid[gb]])            # [256]
            hp = np.concatenate([h_t[ga], h_t[gb]], axis=0)    # [256, D]
            ppos = np.zeros(2 * N, np.int64)
            for t in range(NT):
                idx = np.flatnonzero(tp == t)
                ppos[idx] = offs[t] + np.arange(len(idx))
            pos[2 * p] = p * U + ppos[:N]
            pos[2 * p + 1] = p * U + ppos[N:]
            x0[p * U + ppos] = hp
            # dense pair block: big[m_row, k, n_col] = g[graph, k, n, m]
            big = np.zeros((U, NB, U), np.float32)
            for gi, gr in enumerate((ga, gb)):
                lg = ppos[gi * N:(gi + 1) * N]
                blk = np.transpose(g[gr], (2, 0, 1))           # [m, k, n]
                big[np.ix_(lg, np.arange(NB), lg)] = blk
            gPa[:, p, 0] = np.transpose(big[:128], (0, 1, 2))
            gPa[:, p, 1] = big[128:256]
            gPr[:, p] = big[256:U]
        placements.append((gids.copy(), pos))
        in_maps.append(dict(
            gPa=gPa.astype(np.float16),
            gPr=gPr.astype(np.float16),
            x0=np.ascontiguousarray(x0.T).astype(np.float16),
            mwT06=mwT06, mw8T=mw8T, wihT=wihT, whhT=whhT,
            brz=brz, binn=binn, bhnn=bhnn,
            brzM=brzM, ones1=ones1,
        ))
    meta = (caps, U)
    return in_maps, meta, placements


class _Balancer:
    """Greedy per-engine load balancer for drain/elementwise ops."""

    def __init__(self, nc):
        self.nc = nc
        self.load = {"A": 0.0, "D": 0.0, "P": 0.0}

    def _cost(self, e, op, cols, psum_src, f16_sbuf):
        # Exact TimelineSim engine-busy costs: processing = cols*cycle_t +
        # max-over-operands(2*access_cycles)/2 * cycle_t (SBUF dst dominates).
        if e == "A":
            return cols * 0.8333 + 185.0
        if e == "D":
            if f16_sbuf:
                return cols * 0.521 + 60.0
            return cols * 1.0417 + 125.0
        eff = 0.42 if op in ("add", "sub", "mul") else 0.6
        return cols * 0.8333 / eff + 131.0

    def pick(self, op, cols, psum_src=True, f16_sbuf=False, allow=("A", "D")):
        cand = [(self.load[e] + self._cost(e, op, cols, psum_src, f16_sbuf), e)
                for e in allow]
        _, e = min(cand)
        self.load[e] += self._cost(e, op, cols, psum_src, f16_sbuf)
        return e

    def charge(self, e, op, cols, psum_src=True, f16_sbuf=False):
        self.load[e] += self._cost(e, op, cols, psum_src, f16_sbuf)

    # PSUM sources: GPSIMD has no PSUM access -> ACT/DVE only.
    def relu(self, out, ps, cols):
        e = self.pick("relu", cols)
        if e == "A":
            self.nc.scalar.activation(out, ps, AF.Relu)
        else:
            self.nc.vector.tensor_scalar_max(out, ps, 0.0)

    def copy(self, out, ps, cols):
        e = self.pick("copy", cols)
        if e == "A":
            self.nc.scalar.copy(out, ps)
        else:
            self.nc.vector.tensor_copy(out, ps)

    def stt(self, out, in0, scal, in1, op0, op1, cols):
        self.charge("D", "stt", cols)
        self.nc.vector.scalar_tensor_tensor(out, in0, scal, in1,
                                            op0=op0, op1=op1)

    # SBUF-only f16 elementwise: DVE or Pool.
    def tt(self, op, out, a, b, cols, f16_sbuf=True, allow=("P",)):
        e = self.pick(op, cols, psum_src=False, f16_sbuf=f16_sbuf, allow=allow)
        eng = self.nc.vector if e == "D" else self.nc.gpsimd
        getattr(eng, "tensor_" + op)(out, a, b)


def _build(meta):
    caps, U = meta
    rem = U - 256
    NP = NPAIR * U
    nc = bacc.Bacc("TRN2", target_bir_lowering=False, debug=False, num_devices=M)

    gPa_d = nc.dram_tensor("gPa", [128, NPAIR, 2, NB, U], F16, kind="ExternalInput")
    gPr_d = nc.dram_tensor("gPr", [rem, NPAIR, NB, U], F16, kind="ExternalInput")
    x0_d = nc.dram_tensor("x0", [128, NP], F16, kind="ExternalInput")
    mwT06_d = nc.dram_tensor("mwT06", [128, NB, NL - 1, 128], F16, kind="ExternalInput")
    mw8T_d = nc.dram_tensor("mw8T", [128, NB, 128], F16, kind="ExternalInput")
    wih_d = nc.dram_tensor("wihT", [128, 2, NT, 3, 128], F16, kind="ExternalInput")
    whh_d = nc.dram_tensor("whhT", [128, 2, NT, 3, 128], F16, kind="ExternalInput")
    brz_d = nc.dram_tensor("brz", [128, 2, NT, 2], F32, kind="ExternalInput")
    brzM_d = nc.dram_tensor("brzM", [1, 2 * NT * 2, 128], F16, kind="ExternalInput")
    ones_d = nc.dram_tensor("ones1", [1, 512], F16, kind="ExternalInput")
    binn_d = nc.dram_tensor("binn", [128, 2, NT], F32, kind="ExternalInput")
    bhnn_d = nc.dram_tensor("bhnn", [128, 2, NT], F32, kind="ExternalInput")
    y_d = nc.dram_tensor("y", [128, NP], F16, kind="ExternalOutput")

    # GRU pieces: (type, col-offset, pair0, n_pairs); issued after pair p0+npr-1
    # The final pass splits the second half into npr=2 pieces so the tail only
    # waits on the last two pairs' aggregation.
    pieces_at = {pr: [] for pr in range(NPAIR)}
    pieces_at_final = {pr: [] for pr in range(NPAIR)}
    off = 0
    for t in range(NT):
        if caps[t] == 0:
            continue
        npr = min(4, max(1, 256 // caps[t]))
        while NPAIR % npr:
            npr -= 1
        for p0 in range(0, NPAIR, npr):
            pieces_at[p0 + npr - 1].append((t, off, p0, npr))
            if p0 < NPAIR // 2 or npr <= 2:
                pieces_at_final[p0 + npr - 1].append((t, off, p0, npr))
            else:
                for q0 in range(p0, p0 + npr, 2):
                    pieces_at_final[q0 + 1].append((t, off, q0, 2))
        off += caps[t]

    with tile.TileContext(nc) as tc:
        with (
            tc.tile_pool(name="const", bufs=1) as cp,
            tc.tile_pool(name="xp", bufs=2) as xp,
            tc.tile_pool(name="mlp", bufs=24) as mp,
            tc.tile_pool(name="x7p", bufs=10) as x7p,
            tc.tile_pool(name="xbp", bufs=3) as xbp,
            tc.tile_pool(name="gtp", bufs=5) as gtp,
            tc.tile_pool(name="mtp", bufs=2) as mtp,
            tc.tile_pool(name="gates", bufs=32) as ggp,
            tc.tile_pool(name="mps", bufs=3, space="PSUM") as mpsp,
            tc.tile_pool(name="ps", bufs=2, space="PSUM") as psp,
        ):
            bal = _Balancer(nc)

            x_cur = xp.tile([128, NP], F16, tag="x")
            mwT06 = cp.tile([128, NB, NL - 1, 128], F16, tag="mwT06")
            nc.sync.dma_start(x_cur[:, 0:U], x0_d.ap()[:, 0:U])
            nc.sync.dma_start(mwT06[:, :, 0:1, :], mwT06_d.ap()[:, :, 0:1, :])
            nc.sync.dma_start(x_cur[:, U:3 * U], x0_d.ap()[:, U:3 * U])
            nc.sync.dma_start(mwT06[:, :, 1:, :], mwT06_d.ap()[:, :, 1:, :])
            nc.sync.dma_start(x_cur[:, 3 * U:6 * U], x0_d.ap()[:, 3 * U:6 * U])
            nc.sync.dma_start(x_cur[:, 6 * U:], x0_d.ap()[:, 6 * U:])

            gtiles = {}
            for pn in (0, 1):
                gta0 = gtp.tile([128, 2, NB, U], F16, tag="gta")
                nc.sync.dma_start(gta0[:], gPa_d.ap()[:, pn])
                gtr0 = gtp.tile([64, NB, U], F16, tag="gtr")
                o = 32 * (pn % 2)
                nc.sync.dma_start(gtr0[o:o + rem], gPr_d.ap()[:, pn])
                gtiles[pn] = (gta0, gtr0)

            mw8T = cp.tile([128, NB, 128], F16, tag="mw8T")
            wih = cp.tile([128, 2, NT, 3, 128], F16, tag="wih")
            whh = cp.tile([128, 2, NT, 3, 128], F16, tag="whh")
            brz = cp.tile([128, 2, NT, 2], F32, tag="brz")
            brzM = cp.tile([1, 2 * NT * 2, 128], F16, tag="brzM")
            ones1 = cp.tile([1, 512], F16, tag="ones1")
            binn = cp.tile([128, 2, NT], F32, tag="binn")
            bhnn = cp.tile([128, 2, NT], F32, tag="bhnn")
            nc.sync.dma_start(mw8T[:], mw8T_d.ap())
            nc.sync.dma_start(wih[:], wih_d.ap())
            nc.sync.dma_start(whh[:], whh_d.ap())
            nc.sync.dma_start(brz[:], brz_d.ap())
            nc.sync.dma_start(brzM[:], brzM_d.ap())
            nc.sync.dma_start(ones1[:], ones_d.ap())
            nc.sync.dma_start(binn[:], binn_d.ap())
            nc.sync.dma_start(bhnn[:], bhnn_d.ap())

            def seg(tile_, t_off, p0, npr, w):
                return tile_[:].rearrange("d (pr u) -> d pr u", u=U)[
                    :, p0:p0 + npr, t_off:t_off + w]

            def seg_m(m2, u, t_off, p0, npr, w):
                return m2[:, u, :].rearrange("d (pr u2) -> d pr u2", u2=U)[
                    :, p0:p0 + npr, t_off:t_off + w]

            def seg2(m2, t_off, p0, npr, w):
                """4D view of the merged [128, 2, NP] message tile:
                [128, u, pair, col]."""
                return m2[:].rearrange("d u (pr u2) -> d u pr u2", u2=U)[
                    :, :, p0:p0 + npr, t_off:t_off + w]

            def piece_stages(args, fast_tail):
                """Stage closures for ONE GRU piece.  Dripping one stage per
                wave keeps each engine's in-order stream free of ops whose
                dependencies resolve late (head-of-line blocking: ACT/DVE have
                no exec-queue lookahead, so a stalled op blocks later ones)."""
                (xc, xn, m2, piece) = args
                t, t_off, p0, npr = piece
                w = caps[t]
                ncols = npr * w
                s = dict(xs=seg(xc, t_off, p0, npr, w),
                         ms=[seg_m(m2, 0, t_off, p0, npr, w),
                             seg_m(m2, 1, t_off, p0, npr, w)],
                         ms2=seg2(m2, t_off, p0, npr, w))
                tail = ("D",) if fast_tail else ("P",)

                def st_mm():
                    s["prz"], s["pn2"] = [], []
                    for u in range(2):
                        prz = psp.tile([128, 2, 256], F32, tag="ps",
                                       name="prz")
                        pool2 = mpsp if fast_tail else psp
                        pn2 = pool2.tile([128, 2, 256], F32,
                                         tag="mps" if fast_tail else "ps",
                                         name="pn2")
                        for gi in range(2):
                            nc.tensor.matmul(prz[:, gi, :ncols],
                                             wih[:, u, t, gi, :], s["xs"],
                                             start=True, stop=False)
                            nc.tensor.matmul(prz[:, gi, :ncols],
                                             whh[:, u, t, gi, :], s["ms"][u],
                                             start=False, stop=False)
                            row = (u * NT + t) * 2 + gi
                            nc.tensor.matmul(prz[:, gi, :ncols],
                                             brzM[0:1, row, :],
                                             ones1[0:1, :ncols],
                                             start=False, stop=True)
                        nc.tensor.matmul(pn2[:, 0, :ncols], wih[:, u, t, 2, :],
                                         s["xs"], start=True, stop=True)
                        nc.tensor.matmul(pn2[:, 1, :ncols], whh[:, u, t, 2, :],
                                         s["ms"][u], start=True, stop=True)
                        s["prz"].append(prz)
                        s["pn2"].append(pn2)
                    s["rzb"] = ggp.tile([128, 2, 2, 256], F16,
                                        tag="gt4", name="rzb", bufs=6)

                def st_sig(u):
                    nc.scalar.activation(s["rzb"][:, u, :, :ncols],
                                         s["prz"][u][:, :, :ncols],
                                         AF.Sigmoid)
                    bal.charge("A", "act", 2 * ncols)

                def st_t1():
                    t12 = ggp.tile([128, 2, 256], F16, tag="gt2", name="t12",
                                   bufs=17)
                    s["t12"] = t12
                    for u in range(2):
                        bal.stt(t12[:, u, :ncols], s["pn2"][u][:, 1, :ncols],
                                bhnn[:, u, t:t + 1], s["rzb"][:, u, 0, :ncols],
                                ALU.add, ALU.mult, ncols)

                def st_na():
                    na2 = ggp.tile([128, 2, 256], F16, tag="gt2", name="na2",
                                   bufs=17)
                    s["na2"] = na2
                    for u in range(2):
                        bal.stt(na2[:, u, :ncols], s["pn2"][u][:, 0, :ncols],
                                binn[:, u, t:t + 1], s["t12"][:, u, :ncols],
                                ALU.add, ALU.add, ncols)

                def st_tanh():
                    n2 = ggp.tile([128, 2, 256], F16, tag="gt2", name="n2",
                                  bufs=17)
                    nc.scalar.activation(n2[:, :, :ncols],
                                         s["na2"][:, :, :ncols], AF.Tanh)
                    bal.charge("A", "act", 2 * ncols, psum_src=False)
                    s["n2"] = n2

                def st_d2():
                    d2 = ggp.tile([128, 2, 256], F16, tag="gt2", name="d2",
                                  bufs=17)
                    n2v = s["n2"][:, :, :ncols].rearrange(
                        "d u (pr w) -> d u pr w", w=w)
                    d2v = d2[:, :, :ncols].rearrange(
                        "d u (pr w) -> d u pr w", w=w)
                    bal.tt("sub", d2v, s["ms2"], n2v, 2 * ncols, allow=tail)
                    s["d2"] = d2

                def st_e2():
                    e2 = ggp.tile([128, 2, 256], F16, tag="gt2", name="e2",
                                  bufs=17)
                    bal.tt("mul", e2[:, :, :ncols],
                           s["rzb"][:, :, 1, :ncols],
                           s["d2"][:, :, :ncols], 2 * ncols, allow=tail)
                    s["e2"] = e2

                def st_hu():
                    hu2 = ggp.tile([128, 2, 256], F16, tag="gt2", name="hu2",
                                   bufs=17)
                    bal.tt("add", hu2[:, :, :ncols], s["n2"][:, :, :ncols],
                           s["e2"][:, :, :ncols], 2 * ncols, allow=tail)
                    hv = hu2[:, :, :ncols].rearrange(
                        "d u (pr w) -> d u pr w", w=w)
                    bal.tt("add", seg(xn, t_off, p0, npr, w),
                           hv[:, 0], hv[:, 1],
                           ncols, allow=("D",) if fast_tail else ("P",))

                return [st_mm, lambda: st_sig(0), lambda: st_sig(1),
                        st_t1, st_na, st_tanh, st_d2, st_e2, st_hu]

            def issue_pieces(batch, fast_tail):
                """Issue whole pieces, stage-interleaved across the batch."""
                stl = [piece_stages(a, fast_tail) for a in batch]
                for i in range(max(len(sl) for sl in stl)):
                    for sl in stl:
                        if i < len(sl):
                            sl[i]()

            def flip_quanta(pr, i, x7t, xb):
                """Per-pair layer-8 flip quanta, chunks 0-1 only (the rem
                chunk of BOTH pairs goes into one shared group tile)."""
                qs = []
                for k0, kn in ((0, 2), (2, 2), (4, 2), (6, 1)):
                    def fq(k0=k0, kn=kn):
                        kk = list(range(k0, k0 + kn))
                        ps3 = mpsp.tile([128, len(kk), 2, 128], F32, tag="mps",
                                        name="ps3")
                        for j, k in enumerate(kk):
                            nc.tensor.matmul(ps3[:, j, 0, :],
                                             x7t[k][:, i, 0:128],
                                             mw8T[:, k, :],
                                             start=True, stop=True)
                            nc.tensor.matmul(ps3[:, j, 1, :],
                                             x7t[k][:, i, 128:256],
                                             mw8T[:, k, :],
                                             start=True, stop=True)
                        bal.copy(xb[:, k0:k0 + len(kk), :, :], ps3[:],
                                 len(kk) * 256)
                    qs.append(fq)
                return qs

            def c2_quantum(grp, x7t, xbc2):
                """Both pairs' rem-chunk flips into ONE PSUM tile (pair 1 at
                partition 32 via PE col-tiling, auto-derived from the out AP's
                base partition) and a single 896-col drain."""
                def cq():
                    ps3c = mpsp.tile([128, NB, 128], F32, tag="mps",
                                     name="ps3c")
                    for i in range(len(grp)):
                        off = 32 * i
                        for k in range(NB):
                            nc.tensor.matmul(ps3c[off:off + rem, k, :],
                                             x7t[k][:, i, 256:U],
                                             mw8T[:, k, :],
                                             start=True, stop=True)
                    bal.copy(xbc2[0:32 + rem, :, :], ps3c[0:32 + rem, :, :],
                             NB * 128)
                return [cq]

            def agg_quanta(pr, i, xb, xbc2, xc, xn, m2, pat,
                           accf=None):
                """Aggregation quanta for one pair; mc=2 reads the shared rem
                tile at base partition 32*i (gtr rows DMA'd to match).  With
                accf (final group only, runs wholly in the epilogue): both
                pairs accumulate into one shared 2-bank tile so pair 7 never
                waits on pair 6's psp slot, and the m2 copies merge."""
                cell = {}

                def ps_():
                    if 'ps' not in cell:
                        if accf is not None:
                            cell['ps'] = (accf[:, 0, :U], accf[:, 1, :U])
                        else:
                            cell['ps'] = (psp.tile([128, U], F32, tag="ps",
                                                   name="ps_n"),
                                          psp.tile([128, U], F32, tag="ps",
                                                   name="ps_u"))
                    return cell['ps']

                off = 32 * i
                qs = []
                steps = [(mc, k) for mc in range(3) for k in range(NB)]
                chunks = [steps[j:j + 4] for j in range(0, len(steps), 4)]
                for ci, ch in enumerate(chunks):
                    def aq(ch=ch, lastq=(ci == len(chunks) - 1)):
                        ps_n, ps_u = ps_()
                        gta, gtr = gtiles[pr]
                        for (mc, k) in ch:
                            dst = ps_u if k == NB - 1 else ps_n
                            start = mc == 0 and k in (0, NB - 1)
                            stop = mc == 2 and k in (NB - 2, NB - 1)
                            if mc < 2:
                                nc.tensor.matmul(dst[:], xb[:, k, mc, :],
                                                 gta[:, mc, k, :],
                                                 start=start, stop=stop)
                            else:
                                nc.tensor.matmul(dst[:],
                                                 xbc2[off:off + rem, k, :],
                                                 gtr[off:off + rem, k, :],
                                                 start=start, stop=stop)
                        if not lastq:
                            return
                        sl = slice(pr * U, (pr + 1) * U)
                        bal.copy(m2[:, 0, sl], ps_n[:], U)
                        bal.copy(m2[:, 1, sl], ps_u[:], U)
                        for piece in pat[pr]:
                            pending.append((xc, xn, m2, piece))
                    qs.append(aq)
                return qs

            from collections import deque
            pending = []        # GRU pieces awaiting issue
            fillers = deque()   # flip/agg quanta awaiting interleave
            stq = deque()       # piece stages dripped one per wave
            GROUPS = ((0, 1), (2, 3), (4, 5), (6, 7))
            for p in range(PASSES):
                last = p == PASSES - 1
                pat = pieces_at
                x_next = xp.tile([128, NP], F16, tag="x")
                m2 = mtp.tile([128, 2, NP], F16, tag="m2")

                for pg, grp in enumerate(GROUPS):
                    G = len(grp)
                    # prefetch next group's adjacency (one group ahead)
                    if pg + 1 < len(GROUPS):
                        nxt = [(p, pn_) for pn_ in GROUPS[pg + 1]]
                    else:
                        nxt = [(p + 1, pn_) for pn_ in GROUPS[0]]
                    for pp, pn in nxt:
                        if pp < PASSES:
                            gta = gtp.tile([128, 2, NB, U], F16, tag="gta")
                            nc.sync.dma_start(gta[:], gPa_d.ap()[:, pn])
                            gtr = gtp.tile([64, NB, U], F16, tag="gtr")
                            o = 32 * (pn % 2)
                            nc.sync.dma_start(gtr[o:o + rem], gPr_d.ap()[:, pn])
                            gtiles[pn] = (gta, gtr)

                    # all still-pending pieces must land before this group's
                    # first wave reads their output columns
                    while pending:
                        issue_pieces([pending.pop(0)], False)

                    # ---- bond MLPs: G pairs per PSUM tile, waves over bonds;
                    # the previous group's flips/aggs and older GRU pieces are
                    # interleaved between waves to keep every engine fed ----
                    curs = [[x_cur[:, pr * U:(pr + 1) * U]] * NB for pr in grp]
                    x7t = [None] * NB
                    per_slot = len(fillers) / ((NL - 1) * NB + 4)
                    credit = 0.0
                    for l in range(NL - 1):
                        outs = [[] for _ in grp]
                        for k in range(NB):
                            if l == NL - 2:
                                nt_ = x7p.tile([128, G, U], F16, tag="x7")
                            else:
                                nt_ = mp.tile([128, G, U], F16, tag="mlp")
                            if l == NL - 2:
                                x7t[k] = nt_
                            ps = mpsp.tile([128, G, 512], F32, tag="mps")
                            for j in range(G):
                                nc.tensor.matmul(ps[:, j, :U],
                                                 mwT06[:, k, l, :],
                                                 curs[j][k],
                                                 start=True, stop=True)
                            bal.relu(nt_[:], ps[:, :, :U], G * U)
                            for j in range(G):
                                outs[j].append(nt_[:, j, :])
                            credit += per_slot
                            for _ in range(2):
                                if stq:
                                    stq.popleft()()
                            while credit >= 1.0 and fillers:
                                fillers.popleft()()
                                credit -= 1.0
                        curs = outs
                        while pending:
                            stq.extend(piece_stages(pending.pop(0), False))

                    while fillers:
                        fillers.popleft()()
                    while stq:
                        stq.popleft()()
                    xbs = [xbp.tile([128, NB, 2, 128], F16, tag="xb",
                                    name="xb") for _ in grp]
                    xbc2 = xbp.tile([64, NB, 128], F16, tag="xbc2", bufs=2,
                                    name="xbc2")
                    for j, pr in enumerate(grp):
                        fillers.extend(flip_quanta(pr, j, x7t, xbs[j]))
                    fillers.extend(c2_quantum(grp, x7t, xbc2))
                    fin2 = last and pg == len(GROUPS) - 1
                    for j, pr in enumerate(grp):
                        accf = None
                        if fin2 and j == 1:
                            accf = mpsp.tile([128, 2, 512], F32, tag="mps",
                                             name="accf")
                        fillers.extend(
                            agg_quanta(pr, j, xbs[j], xbc2, x_cur, x_next,
                                       m2, pat, accf))

                    if last and pg == len(GROUPS) - 1:
                        # pairs 0-5: make sure every piece write is issued
                        # BEFORE the DMA reads those columns (issue order
                        # defines RAW vs WAR for the dependency tracker)
                        while pending:
                            issue_pieces([pending.pop(0)], False)
                        nc.sync.dma_start(y_d.ap()[:, 0:4 * U],
                                          x_next[:, 0:4 * U])

                x_cur = x_next

            while fillers:
                fillers.popleft()()
            first = True
            while pending:
                nb_ = 1 if first else 2
                first = False
                issue_pieces(pending[:nb_], True)
                pending = pending[nb_:]
            nc.sync.dma_start(y_d.ap()[:, 4 * U:], x_cur[:, 4 * U:])

    nc.compile()
    return nc


def _make_runner(nc):
    import jax
    from jax.experimental.shard_map import shard_map
    from jax.sharding import Mesh, PartitionSpec, NamedSharding
    from concourse.bass2jax import (install_neuronx_cc_hook, _bass_exec_p,
                                    partition_id_tensor)

    install_neuronx_cc_hook()
    partition_name = (nc.partition_id_tensor.name
                      if nc.partition_id_tensor else None)
    in_names, out_names, out_avals, zero_outs = [], [], [], []
    for alloc in nc.m.functions[0].allocations:
        if not isinstance(alloc, mybir.MemoryLocationSet):
            continue
        name = alloc.memorylocations[0].name
        if alloc.kind == "ExternalInput":
            if name != partition_name:
                in_names.append(name)
        elif alloc.kind == "ExternalOutput":
            out_names.append(name)
            shape = tuple(alloc.tensor_shape)
            dtype = mybir.dt.np(alloc.dtype)
            out_avals.append(jax.core.ShapedArray(shape, dtype))
            zero_outs.append(np.zeros(shape, dtype))
    n_params = len(in_names)
    all_names = in_names + out_names
    if partition_name is not None:
        all_names = all_names + [partition_name]

    def _body(*args):
        operands = list(args)
        if partition_name is not None:
            operands.append(partition_id_tensor())
        outs = _bass_exec_p.bind(
            *operands,
            out_avals=tuple(out_avals),
            in_names=tuple(all_names),
            out_names=tuple(out_names),
            lowering_input_output_aliases=(),
            sim_require_finite=True,
            sim_require_nnan=True,
            nc=nc,
        )
        return tuple(outs)

    devices = jax.devices()[:M]
    mesh = Mesh(np.asarray(devices), ("core",))
    specs = (PartitionSpec("core"),) * (n_params + len(out_names))
    fn = jax.jit(shard_map(_body, mesh=mesh,
                           in_specs=specs,
                           out_specs=(PartitionSpec("core"),) * len(out_names)),
                 keep_unused=True)

    def put(in_maps):
        sh = NamedSharding(mesh, PartitionSpec("core"))
        args = []
        for name in in_names:
            cat = np.concatenate([np.asarray(im[name]) for im in in_maps], axis=0)
            args.append(jax.device_put(cat, sh))
        for z in zero_outs:
            cat = np.concatenate([z] * M, axis=0)
            args.append(jax.device_put(cat, sh))
        return args

    def run(args):
        outs = fn(*args)
        outs = [np.asarray(o) for o in outs]
        per_core = []
        for c in range(M):
            per_core.append({
                name: outs[i].reshape(M, *out_avals[i].shape)[c]
                for i, name in enumerate(out_names)})
        return per_core

    return put, run


_CACHE = {}


def _get_runner(meta):
    if meta not in _CACHE:
        nc = _build(meta)
        _CACHE[meta] = (_make_runner(nc), nc)
    return _CACHE[meta]


def _assemble(per_core, placements):
    out = np.empty((B, N, D), np.float32)
    for c in range(M):
        y = np.asarray(per_core[c]["y"], np.float32)   # [D, NP] padded transposed
        gids, pos = placements[c]
        out[gids] = y.T[pos]
    return out


def kernel(g, h, msg_W, gru_Wih, gru_Whh, gru_bih, gru_bhh):
    in_maps, meta, placements = _prepare(g, h, msg_W, gru_Wih, gru_Whh,
                                         gru_bih, gru_bhh)
    (put, run), _nc = _get_runner(meta)
    args = put(in_maps)
    per_core = run(args)
    return _assemble(per_core, placements)


# exposed for test.py
def get_nc_and_runner(g, h, msg_W, gru_Wih, gru_Whh, gru_bih, gru_bhh):
    in_maps, meta, placements = _prepare(g, h, msg_W, gru_Wih, gru_Whh,
                                         gru_bih, gru_bhh)
    (put, run), nc = _get_runner(meta)
    return in_maps, put, run, nc, placements



# revision 87
# speedup vs baseline: 1.0107x; 1.0002x over previous
"""Trainium2 Bass kernel for nn_Big_MPNN (gnn_message_passing).

Self-contained: hardcodes shapes/sharding. Data-parallel over the batch dim
across 8 NeuronCores (16 graphs per core), weights replicated; no collectives.

Node layout: the host pairs graphs to BALANCE per-type counts (local search
minimizing sum of per-type max counts over pairs), then sorts nodes by GRU
atom-type within each pair. Each pair occupies exactly U = sum(caps) columns
(no dead padding); per-type capacities are uniform across all pairs/cores so
every per-type GRU matmul reads a static strided access pattern.

Per-core dataflow (3 passes), transposed activations [D=128 part, cols],
all f16 except PSUM/biases/final cast.  Pairs are processed in groups of two;
per (layer, bond) wave one 2-bank PSUM tile holds both pairs and is drained
by a single ReLU op, load-balanced between ACT and DVE (GPSIMD cannot read
PSUM; it gets the SBUF-only f16 GRU elementwise ops instead).  Each group's
layer-7 flip (chunks 128/128/rem -> normal-layout xb) and aggregation
m^T = xb^T g^T are split into small matmul quanta and paced between the NEXT
group's MLP wave tiles, so the tensor engine never runs long drain-free
stretches.  GRU pieces merge both GRU universes into single wide elementwise
ops (messages in one [128, 2, NP] tile; tanh and the blend each issued once
per piece over [2, ncols]); each piece is expanded into 9 dependency-ordered
stage closures dripped two per wave so no engine's in-order stream blocks on
an op whose inputs resolve late (ACT/DVE have no exec-queue lookahead).  The
final pass drains its last pieces stage-interleaved with a DVE-only tail and
ships y in two DMA halves; pieces must be ISSUED before a DMA that reads
their columns (issue order defines RAW vs WAR for the dependency tracker).
Host unpads/unpermutes the f16 result.
"""

import numpy as np

import concourse.bass as bass
import concourse.bacc as bacc
import concourse.tile as tile
import concourse.mybir as mybir

F32 = mybir.dt.float32
F16 = mybir.dt.float16
AF = mybir.ActivationFunctionType
ALU = mybir.AluOpType

M = 8                      # cores
B, N, FEAT, D = 128, 128, 75, 128
NB, NL, NT = 7, 8, 6       # bonds, mlp layers, gru type slots
PASSES = 3
BG = B // M                # graphs per core
NPAIR = BG // 2            # graph pairs per core (8)
TOP_ATOMS = [6.0, 7.0, 8.0, 9.0, 0.0]


def _pair_graphs(cnt):
    """Pair the B graphs to minimize sum_t max_pairs(count_t).  cnt: [B, NT]."""
    P = B // 2
    order = np.argsort(cnt[:, NT - 1], kind="stable")
    pairs = np.stack([order[:P], order[:P - 1:-1]], 1)
    rng = np.random.default_rng(12345)

    def obj(pr):
        pc = cnt[pr[:, 0]] + cnt[pr[:, 1]]
        s = np.sort(pc, 0)[::-1]
        return s[0].sum() * 1000 + s[1].sum() * 10 + s[2].sum()

    cur = pairs.copy()
    co = obj(cur)
    best, bo = cur.copy(), co
    for _ in range(150000):
        i, j = rng.integers(0, P, 2)
        if i == j:
            continue
        trial = cur.copy()
        a1, b1 = trial[i]
        a2, b2 = trial[j]
        if rng.integers(0, 2) == 0:
            trial[i] = (a1, a2)
            trial[j] = (b1, b2)
        else:
            trial[i] = (a1, b2)
            trial[j] = (a2, b1)
        to = obj(trial)
        if to <= co:
            cur, co = trial, to
            if to < bo:
                best, bo = trial.copy(), to
    return best


def _prepare(g, h, msg_W, gru_Wih, gru_Whh, gru_bih, gru_bhh):
    g = np.ascontiguousarray(np.asarray(g, np.float32))
    h = np.ascontiguousarray(np.asarray(h, np.float32))
    msg_W = np.asarray(msg_W, np.float32)
    gru_Wih = np.asarray(gru_Wih, np.float32).reshape(2, NT, 3, D, D)
    gru_Whh = np.asarray(gru_Whh, np.float32).reshape(2, NT, 3, D, D)
    gru_bih = np.asarray(gru_bih, np.float32).reshape(2, NT, 3, D)
    gru_bhh = np.asarray(gru_bhh, np.float32).reshape(2, NT, 3, D)

    atoms = h[:, :, 0]
    tid = np.full((B, N), NT - 1, np.int32)
    for i, a in enumerate(TOP_ATOMS):
        tid[atoms == np.float32(a)] = i
    cnt = np.stack([(tid == t).sum(1) for t in range(NT)], 1).astype(np.int64)

    pairs = _pair_graphs(cnt)                       # [64, 2] graph ids
    pc = cnt[pairs[:, 0]] + cnt[pairs[:, 1]]
    caps = tuple(int(c) for c in pc.max(axis=0))
    U = sum(caps)
    assert 256 < U <= 384, f"caps {caps} sum {U} out of supported range"
    rem = U - 256
    NP = NPAIR * U
    offs = np.cumsum([0] + list(caps))[:-1]

    # replicated weights, partition-major f16 layouts
    mwT = np.transpose(msg_W, (3, 0, 1, 2))         # [din, k, l, dout]
    mwT06 = np.ascontiguousarray(mwT[:, :, :NL - 1]).astype(np.float16)
    mw8T = np.ascontiguousarray(mwT[:, :, NL - 1]).astype(np.float16)
    wihT = np.ascontiguousarray(
        np.transpose(gru_Wih, (4, 0, 1, 2, 3))).astype(np.float16)
    whhT = np.ascontiguousarray(
        np.transpose(gru_Whh, (4, 0, 1, 2, 3))).astype(np.float16)
    brz = np.ascontiguousarray(
        np.transpose(gru_bih[:, :, :2] + gru_bhh[:, :, :2], (3, 0, 1, 2)))
    brzM = np.ascontiguousarray(
        np.transpose(brz, (1, 2, 3, 0)).reshape(1, 2 * NT * 2, D)
    ).astype(np.float16)
    ones1 = np.ones((1, 512), np.float16)
    binn = np.ascontiguousarray(np.transpose(gru_bih[:, :, 2], (2, 0, 1)))
    bhnn = np.ascontiguousarray(np.transpose(gru_bhh[:, :, 2], (2, 0, 1)))

    h_t = np.concatenate([h, np.zeros((B, N, D - FEAT), np.float32)], axis=2)

    in_maps = []
    placements = []     # per core: (gids [BG], pos [BG, N])
    for c in range(M):
        gids = pairs[c * NPAIR:(c + 1) * NPAIR].reshape(-1)
        pos = np.zeros((BG, N), np.int64)
        x0 = np.zeros((NP, D), np.float32)
        gPa = np.zeros((128, NPAIR, 2, NB, U), np.float32)
        gPr = np.zeros((rem, NPAIR, NB, U), np.float32)
        for p in range(NPAIR):
            ga, gb = gids[2 * p], gids[2 * p + 1]
            tp = np.concatenate([tid[ga], tid[gb]])            # [256]
            hp = np.concatenate([h_t[ga], h_t[gb]], axis=0)    # [256, D]
            ppos = np.zeros(2 * N, np.int64)
            for t in range(NT):
                idx = np.flatnonzero(tp == t)
                ppos[idx] = offs[t] + np.arange(len(idx))
            pos[2 * p] = p * U + ppos[:N]
            pos[2 * p + 1] = p * U + ppos[N:]
            x0[p * U + ppos] = hp
            # dense pair block: big[m_row, k, n_col] = g[graph, k, n, m]
            big = np.zeros((U, NB, U), np.float32)
            for gi, gr in enumerate((ga, gb)):
                lg = ppos[gi * N:(gi + 1) * N]
                blk = np.transpose(g[gr], (2, 0, 1))           # [m, k, n]
                big[np.ix_(lg, np.arange(NB), lg)] = blk
            gPa[:, p, 0] = np.transpose(big[:128], (0, 1, 2))
            gPa[:, p, 1] = big[128:256]
            gPr[:, p] = big[256:U]
        placements.append((gids.copy(), pos))
        in_maps.append(dict(
            gPa=gPa.astype(np.float16),
            gPr=gPr.astype(np.float16),
            x0=np.ascontiguousarray(x0.T).astype(np.float16),
            mwT06=mwT06, mw8T=mw8T, wihT=wihT, whhT=whhT,
            brz=brz, binn=binn, bhnn=bhnn,
            brzM=brzM, ones1=ones1,
        ))
    meta = (caps, U)
    return in_maps, meta, placements


class _Balancer:
    """Greedy per-engine load balancer for drain/elementwise ops."""

    def __init__(self, nc):
        self.nc = nc
        self.load = {"A": 0.0, "D": 0.0, "P": 0.0}

    def _cost(self, e, op, cols, psum_src, f16_sbuf):
        # Exact TimelineSim engine-busy costs: processing = cols*cycle_t +
        # max-over-operands(2*access_cycles)/2 * cycle_t (SBUF dst dominates).
        if e == "A":
            return cols * 0.8333 + 185.0
        if e == "D":
            if f16_sbuf:
                return cols * 0.521 + 60.0
            return cols * 1.0417 + 125.0
        eff = 0.42 if op in ("add", "sub", "mul") else 0.6
        return cols * 0.8333 / eff + 131.0

    def pick(self, op, cols, psum_src=True, f16_sbuf=False, allow=("A", "D")):
        cand = [(self.load[e] + self._cost(e, op, cols, psum_src, f16_sbuf), e)
                for e in allow]
        _, e = min(cand)
        self.load[e] += self._cost(e, op, cols, psum_src, f16_sbuf)
        return e

    def charge(self, e, op, cols, psum_src=True, f16_sbuf=False):
        self.load[e] += self._cost(e, op, cols, psum_src, f16_sbuf)

    # PSUM sources: GPSIMD has no PSUM access -> ACT/DVE only.
    def relu(self, out, ps, cols):
        e = self.pick("relu", cols)
        if e == "A":
            self.nc.scalar.activation(out, ps, AF.Relu)
        else:
            self.nc.vector.tensor_scalar_max(out, ps, 0.0)

    def copy(self, out, ps, cols):
        e = self.pick("copy", cols)
        if e == "A":
            self.nc.scalar.copy(out, ps)
        else:
            self.nc.vector.tensor_copy(out, ps)

    def stt(self, out, in0, scal, in1, op0, op1, cols):
        self.charge("D", "stt", cols)
        self.nc.vector.scalar_tensor_tensor(out, in0, scal, in1,
                                            op0=op0, op1=op1)

    # SBUF-only f16 elementwise: DVE or Pool.
    def tt(self, op, out, a, b, cols, f16_sbuf=True, allow=("P",)):
        e = self.pick(op, cols, psum_src=False, f16_sbuf=f16_sbuf, allow=allow)
        eng = self.nc.vector if e == "D" else self.nc.gpsimd
        getattr(eng, "tensor_" + op)(out, a, b)


def _build(meta):
    caps, U = meta
    rem = U - 256
    NP = NPAIR * U
    nc = bacc.Bacc("TRN2", target_bir_lowering=False, debug=False, num_devices=M)

    gPa_d = nc.dram_tensor("gPa", [128, NPAIR, 2, NB, U], F16, kind="ExternalInput")
    gPr_d = nc.dram_tensor("gPr", [rem, NPAIR, NB, U], F16, kind="ExternalInput")
    x0_d = nc.dram_tensor("x0", [128, NP], F16, kind="ExternalInput")
    mwT06_d = nc.dram_tensor("mwT06", [128, NB, NL - 1, 128], F16, kind="ExternalInput")
    mw8T_d = nc.dram_tensor("mw8T", [128, NB, 128], F16, kind="ExternalInput")
    wih_d = nc.dram_tensor("wihT", [128, 2, NT, 3, 128], F16, kind="ExternalInput")
    whh_d = nc.dram_tensor("whhT", [128, 2, NT, 3, 128], F16, kind="ExternalInput")
    brz_d = nc.dram_tensor("brz", [128, 2, NT, 2], F32, kind="ExternalInput")
    brzM_d = nc.dram_tensor("brzM", [1, 2 * NT * 2, 128], F16, kind="ExternalInput")
    ones_d = nc.dram_tensor("ones1", [1, 512], F16, kind="ExternalInput")
    binn_d = nc.dram_tensor("binn", [128, 2, NT], F32, kind="ExternalInput")
    bhnn_d = nc.dram_tensor("bhnn", [128, 2, NT], F32, kind="ExternalInput")
    y_d = nc.dram_tensor("y", [128, NP], F16, kind="ExternalOutput")

    # GRU pieces: (type, col-offset, pair0, n_pairs); issued after pair p0+npr-1
    # The final pass splits the second half into npr=2 pieces so the tail only
    # waits on the last two pairs' aggregation.
    pieces_at = {pr: [] for pr in range(NPAIR)}
    pieces_at_final = {pr: [] for pr in range(NPAIR)}
    off = 0
    for t in range(NT):
        if caps[t] == 0:
            continue
        npr = min(4, max(1, 256 // caps[t]))
        while NPAIR % npr:
            npr -= 1
        for p0 in range(0, NPAIR, npr):
            pieces_at[p0 + npr - 1].append((t, off, p0, npr))
            if p0 < NPAIR // 2 or npr <= 2:
                pieces_at_final[p0 + npr - 1].append((t, off, p0, npr))
            else:
                for q0 in range(p0, p0 + npr, 2):
                    pieces_at_final[q0 + 1].append((t, off, q0, 2))
        off += caps[t]

    with tile.TileContext(nc) as tc:
        with (
            tc.tile_pool(name="const", bufs=1) as cp,
            tc.tile_pool(name="xp", bufs=2) as xp,
            tc.tile_pool(name="mlp", bufs=24) as mp,
            tc.tile_pool(name="x7p", bufs=10) as x7p,
            tc.tile_pool(name="xbp", bufs=3) as xbp,
            tc.tile_pool(name="gtp", bufs=5) as gtp,
            tc.tile_pool(name="mtp", bufs=2) as mtp,
            tc.tile_pool(name="gates", bufs=32) as ggp,
            tc.tile_pool(name="mps", bufs=3, space="PSUM") as mpsp,
            tc.tile_pool(name="ps", bufs=2, space="PSUM") as psp,
        ):
            bal = _Balancer(nc)

            x_cur = xp.tile([128, NP], F16, tag="x")
            mwT06 = cp.tile([128, NB, NL - 1, 128], F16, tag="mwT06")
            nc.sync.dma_start(x_cur[:, 0:2 * U], x0_d.ap()[:, 0:2 * U])
            nc.sync.dma_start(mwT06[:, 0:1, 0:1, :],
                              mwT06_d.ap()[:, 0:1, 0:1, :])
            nc.sync.dma_start(mwT06[:, 1:, 0:1, :],
                              mwT06_d.ap()[:, 1:, 0:1, :])
            nc.sync.dma_start(mwT06[:, :, 1:2, :], mwT06_d.ap()[:, :, 1:2, :])
            nc.sync.dma_start(x_cur[:, 2 * U:4 * U], x0_d.ap()[:, 2 * U:4 * U])
            nc.sync.dma_start(mwT06[:, :, 2:, :], mwT06_d.ap()[:, :, 2:, :])
            nc.sync.dma_start(x_cur[:, 4 * U:], x0_d.ap()[:, 4 * U:])

            gtiles = {}
            for pn in (0, 1):
                gta0 = gtp.tile([128, 2, NB, U], F16, tag="gta")
                nc.sync.dma_start(gta0[:], gPa_d.ap()[:, pn])
                gtr0 = gtp.tile([64, NB, U], F16, tag="gtr")
                o = 32 * (pn % 2)
                nc.sync.dma_start(gtr0[o:o + rem], gPr_d.ap()[:, pn])
                gtiles[pn] = (gta0, gtr0)

            mw8T = cp.tile([128, NB, 128], F16, tag="mw8T")
            wih = cp.tile([128, 2, NT, 3, 128], F16, tag="wih")
            whh = cp.tile([128, 2, NT, 3, 128], F16, tag="whh")
            brz = cp.tile([128, 2, NT, 2], F32, tag="brz")
            brzM = cp.tile([1, 2 * NT * 2, 128], F16, tag="brzM")
            ones1 = cp.tile([1, 512], F16, tag="ones1")
            binn = cp.tile([128, 2, NT], F32, tag="binn")
            bhnn = cp.tile([128, 2, NT], F32, tag="bhnn")
            nc.sync.dma_start(mw8T[:], mw8T_d.ap())
            nc.sync.dma_start(wih[:], wih_d.ap())
            nc.sync.dma_start(whh[:], whh_d.ap())
            nc.sync.dma_start(brz[:], brz_d.ap())
            nc.sync.dma_start(brzM[:], brzM_d.ap())
            nc.sync.dma_start(ones1[:], ones_d.ap())
            nc.sync.dma_start(binn[:], binn_d.ap())
            nc.sync.dma_start(bhnn[:], bhnn_d.ap())

            def seg(tile_, t_off, p0, npr, w):
                return tile_[:].rearrange("d (pr u) -> d pr u", u=U)[
                    :, p0:p0 + npr, t_off:t_off + w]

            def seg_m(m2, u, t_off, p0, npr, w):
                return m2[:, u, :].rearrange("d (pr u2) -> d pr u2", u2=U)[
                    :, p0:p0 + npr, t_off:t_off + w]

            def seg2(m2, t_off, p0, npr, w):
                """4D view of the merged [128, 2, NP] message tile:
                [128, u, pair, col]."""
                return m2[:].rearrange("d u (pr u2) -> d u pr u2", u2=U)[
                    :, :, p0:p0 + npr, t_off:t_off + w]

            def piece_stages(args, fast_tail):
                """Stage closures for ONE GRU piece.  Dripping one stage per
                wave keeps each engine's in-order stream free of ops whose
                dependencies resolve late (head-of-line blocking: ACT/DVE have
                no exec-queue lookahead, so a stalled op blocks later ones)."""
                (xc, xn, m2, piece) = args
                t, t_off, p0, npr = piece
                w = caps[t]
                ncols = npr * w
                s = dict(xs=seg(xc, t_off, p0, npr, w),
                         ms=[seg_m(m2, 0, t_off, p0, npr, w),
                             seg_m(m2, 1, t_off, p0, npr, w)],
                         ms2=seg2(m2, t_off, p0, npr, w))
                tail = ("D",) if fast_tail else ("P",)

                def st_mm():
                    s["prz"], s["pn2"] = [], []
                    for u in range(2):
                        prz = psp.tile([128, 2, 256], F32, tag="ps",
                                       name="prz")
                        pool2 = mpsp if fast_tail else psp
                        pn2 = pool2.tile([128, 2, 256], F32,
                                         tag="mps" if fast_tail else "ps",
                                         name="pn2")
                        for gi in range(2):
                            nc.tensor.matmul(prz[:, gi, :ncols],
                                             wih[:, u, t, gi, :], s["xs"],
                                             start=True, stop=False)
                            nc.tensor.matmul(prz[:, gi, :ncols],
                                             whh[:, u, t, gi, :], s["ms"][u],
                                             start=False, stop=False)
                            row = (u * NT + t) * 2 + gi
                            nc.tensor.matmul(prz[:, gi, :ncols],
                                             brzM[0:1, row, :],
                                             ones1[0:1, :ncols],
                                             start=False, stop=True)
                        nc.tensor.matmul(pn2[:, 0, :ncols], wih[:, u, t, 2, :],
                                         s["xs"], start=True, stop=True)
                        nc.tensor.matmul(pn2[:, 1, :ncols], whh[:, u, t, 2, :],
                                         s["ms"][u], start=True, stop=True)
                        s["prz"].append(prz)
                        s["pn2"].append(pn2)
                    s["rzb"] = ggp.tile([128, 2, 2, 256], F16,
                                        tag="gt4", name="rzb", bufs=6)

                def st_sig(u):
                    nc.scalar.activation(s["rzb"][:, u, :, :ncols],
                                         s["prz"][u][:, :, :ncols],
                                         AF.Sigmoid)
                    bal.charge("A", "act", 2 * ncols)

                def st_t1():
                    t12 = ggp.tile([128, 2, 256], F16, tag="gt2", name="t12",
                                   bufs=17)
                    s["t12"] = t12
                    for u in range(2):
                        bal.stt(t12[:, u, :ncols], s["pn2"][u][:, 1, :ncols],
                                bhnn[:, u, t:t + 1], s["rzb"][:, u, 0, :ncols],
                                ALU.add, ALU.mult, ncols)

                def st_na():
                    na2 = ggp.tile([128, 2, 256], F16, tag="gt2", name="na2",
                                   bufs=17)
                    s["na2"] = na2
                    for u in range(2):
                        bal.stt(na2[:, u, :ncols], s["pn2"][u][:, 0, :ncols],
                                binn[:, u, t:t + 1], s["t12"][:, u, :ncols],
                                ALU.add, ALU.add, ncols)

                def st_tanh():
                    n2 = ggp.tile([128, 2, 256], F16, tag="gt2", name="n2",
                                  bufs=17)
                    nc.scalar.activation(n2[:, :, :ncols],
                                         s["na2"][:, :, :ncols], AF.Tanh)
                    bal.charge("A", "act", 2 * ncols, psum_src=False)
                    s["n2"] = n2

                def st_d2():
                    d2 = ggp.tile([128, 2, 256], F16, tag="gt2", name="d2",
                                  bufs=17)
                    n2v = s["n2"][:, :, :ncols].rearrange(
                        "d u (pr w) -> d u pr w", w=w)
                    d2v = d2[:, :, :ncols].rearrange(
                        "d u (pr w) -> d u pr w", w=w)
                    bal.tt("sub", d2v, s["ms2"], n2v, 2 * ncols, allow=tail)
                    s["d2"] = d2

                def st_e2():
                    e2 = ggp.tile([128, 2, 256], F16, tag="gt2", name="e2",
                                  bufs=17)
                    bal.tt("mul", e2[:, :, :ncols],
                           s["rzb"][:, :, 1, :ncols],
                           s["d2"][:, :, :ncols], 2 * ncols, allow=tail)
                    s["e2"] = e2

                def st_hu():
                    hu2 = ggp.tile([128, 2, 256], F16, tag="gt2", name="hu2",
                                   bufs=17)
                    bal.tt("add", hu2[:, :, :ncols], s["n2"][:, :, :ncols],
                           s["e2"][:, :, :ncols], 2 * ncols, allow=tail)
                    hv = hu2[:, :, :ncols].rearrange(
                        "d u (pr w) -> d u pr w", w=w)
                    bal.tt("add", seg(xn, t_off, p0, npr, w),
                           hv[:, 0], hv[:, 1],
                           ncols, allow=("D",) if fast_tail else ("P",))

                return [st_mm, lambda: st_sig(0), lambda: st_sig(1),
                        st_t1, st_na, st_tanh, st_d2, st_e2, st_hu]

            def issue_pieces(batch, fast_tail):
                """Issue whole pieces, stage-interleaved across the batch."""
                stl = [piece_stages(a, fast_tail) for a in batch]
                for i in range(max(len(sl) for sl in stl)):
                    for sl in stl:
                        if i < len(sl):
                            sl[i]()

            def flip_quanta(pr, i, x7t, xb):
                """Per-pair layer-8 flip quanta, chunks 0-1 only (the rem
                chunk of BOTH pairs goes into one shared group tile)."""
                qs = []
                for k0, kn in ((0, 2), (2, 2), (4, 2), (6, 1)):
                    def fq(k0=k0, kn=kn):
                        kk = list(range(k0, k0 + kn))
                        ps3 = mpsp.tile([128, len(kk), 2, 128], F32, tag="mps",
                                        name="ps3")
                        for j, k in enumerate(kk):
                            nc.tensor.matmul(ps3[:, j, 0, :],
                                             x7t[k][:, i, 0:128],
                                             mw8T[:, k, :],
                                             start=True, stop=True)
                            nc.tensor.matmul(ps3[:, j, 1, :],
                                             x7t[k][:, i, 128:256],
                                             mw8T[:, k, :],
                                             start=True, stop=True)
                        bal.copy(xb[:, k0:k0 + len(kk), :, :], ps3[:],
                                 len(kk) * 256)
                    qs.append(fq)
                return qs

            def c2_quantum(grp, x7t, xbc2):
                """Both pairs' rem-chunk flips into ONE PSUM tile (pair 1 at
                partition 32 via PE col-tiling, auto-derived from the out AP's
                base partition) and a single 896-col drain."""
                def cq():
                    ps3c = mpsp.tile([128, NB, 128], F32, tag="mps",
                                     name="ps3c")
                    for i in range(len(grp)):
                        off = 32 * i
                        for k in range(NB):
                            nc.tensor.matmul(ps3c[off:off + rem, k, :],
                                             x7t[k][:, i, 256:U],
                                             mw8T[:, k, :],
                                             start=True, stop=True)
                    bal.copy(xbc2[0:32 + rem, :, :], ps3c[0:32 + rem, :, :],
                             NB * 128)
                return [cq]

            def agg_quanta(pr, i, xb, xbc2, xc, xn, m2, pat,
                           accf=None):
                """Aggregation quanta for one pair; mc=2 reads the shared rem
                tile at base partition 32*i (gtr rows DMA'd to match).  With
                accf (final group only, runs wholly in the epilogue): both
                pairs accumulate into one shared 2-bank tile so pair 7 never
                waits on pair 6's psp slot, and the m2 copies merge."""
                cell = {}

                def ps_():
                    if 'ps' not in cell:
                        if accf is not None:
                            cell['ps'] = (accf[:, 0, :U], accf[:, 1, :U])
                        else:
                            cell['ps'] = (psp.tile([128, U], F32, tag="ps",
                                                   name="ps_n"),
                                          psp.tile([128, U], F32, tag="ps",
                                                   name="ps_u"))
                    return cell['ps']

                off = 32 * i
                qs = []
                steps = [(mc, k) for mc in range(3) for k in range(NB)]
                chunks = [steps[j:j + 4] for j in range(0, len(steps), 4)]
                for ci, ch in enumerate(chunks):
                    def aq(ch=ch, lastq=(ci == len(chunks) - 1)):
                        ps_n, ps_u = ps_()
                        gta, gtr = gtiles[pr]
                        for (mc, k) in ch:
                            dst = ps_u if k == NB - 1 else ps_n
                            start = mc == 0 and k in (0, NB - 1)
                            stop = mc == 2 and k in (NB - 2, NB - 1)
                            if mc < 2:
                                nc.tensor.matmul(dst[:], xb[:, k, mc, :],
                                                 gta[:, mc, k, :],
                                                 start=start, stop=stop)
                            else:
                                nc.tensor.matmul(dst[:],
                                                 xbc2[off:off + rem, k, :],
                                                 gtr[off:off + rem, k, :],
                                                 start=start, stop=stop)
                        if not lastq:
                            return
                        sl = slice(pr * U, (pr + 1) * U)
                        bal.copy(m2[:, 0, sl], ps_n[:], U)
                        bal.copy(m2[:, 1, sl], ps_u[:], U)
                        for piece in pat[pr]:
                            pending.append((xc, xn, m2, piece))
                    qs.append(aq)
                return qs

            from collections import deque
            pending = []        # GRU pieces awaiting issue
            fillers = deque()   # flip/agg quanta awaiting interleave
            stq = deque()       # piece stages dripped one per wave
            GROUPS = ((0, 1), (2, 3), (4, 5), (6, 7))
            for p in range(PASSES):
                last = p == PASSES - 1
                pat = pieces_at
                x_next = xp.tile([128, NP], F16, tag="x")
                m2 = mtp.tile([128, 2, NP], F16, tag="m2")

                for pg, grp in enumerate(GROUPS):
                    G = len(grp)
                    # prefetch next group's adjacency (one group ahead)
                    if pg + 1 < len(GROUPS):
                        nxt = [(p, pn_) for pn_ in GROUPS[pg + 1]]
                    else:
                        nxt = [(p + 1, pn_) for pn_ in GROUPS[0]]
                    for pp, pn in nxt:
                        if pp < PASSES:
                            gta = gtp.tile([128, 2, NB, U], F16, tag="gta")
                            nc.sync.dma_start(gta[:], gPa_d.ap()[:, pn])
                            gtr = gtp.tile([64, NB, U], F16, tag="gtr")
                            o = 32 * (pn % 2)
                            nc.sync.dma_start(gtr[o:o + rem], gPr_d.ap()[:, pn])
                            gtiles[pn] = (gta, gtr)

                    # all still-pending pieces must land before this group's
                    # first wave reads their output columns
                    while pending:
                        issue_pieces([pending.pop(0)], False)

                    # ---- bond MLPs: G pairs per PSUM tile, waves over bonds;
                    # the previous group's flips/aggs and older GRU pieces are
                    # interleaved between waves to keep every engine fed ----
                    curs = [[x_cur[:, pr * U:(pr + 1) * U]] * NB for pr in grp]
                    x7t = [None] * NB
                    per_slot = len(fillers) / ((NL - 1) * NB + 4)
                    credit = 0.0
                    for l in range(NL - 1):
                        outs = [[] for _ in grp]
                        for k in range(NB):
                            if l == NL - 2:
                                nt_ = x7p.tile([128, G, U], F16, tag="x7")
                            else:
                                nt_ = mp.tile([128, G, U], F16, tag="mlp")
                            if l == NL - 2:
                                x7t[k] = nt_
                            ps = mpsp.tile([128, G, 512], F32, tag="mps")
                            for j in range(G):
                                nc.tensor.matmul(ps[:, j, :U],
                                                 mwT06[:, k, l, :],
                                                 curs[j][k],
                                                 start=True, stop=True)
                            bal.relu(nt_[:], ps[:, :, :U], G * U)
                            for j in range(G):
                                outs[j].append(nt_[:, j, :])
                            credit += per_slot
                            for _ in range(2):
                                if stq:
                                    stq.popleft()()
                            while credit >= 1.0 and fillers:
                                fillers.popleft()()
                                credit -= 1.0
                        curs = outs
                        while pending:
                            stq.extend(piece_stages(pending.pop(0), False))

                    while fillers:
                        fillers.popleft()()
                    while stq:
                        stq.popleft()()
                    xbs = [xbp.tile([128, NB, 2, 128], F16, tag="xb",
                                    name="xb") for _ in grp]
                    xbc2 = xbp.tile([64, NB, 128], F16, tag="xbc2", bufs=2,
                                    name="xbc2")
                    for j, pr in enumerate(grp):
                        fillers.extend(flip_quanta(pr, j, x7t, xbs[j]))
                    fillers.extend(c2_quantum(grp, x7t, xbc2))
                    fin2 = last and pg == len(GROUPS) - 1
                    for j, pr in enumerate(grp):
                        accf = None
                        if fin2 and j == 1:
                            accf = mpsp.tile([128, 2, 512], F32, tag="mps",
                                             name="accf")
                        fillers.extend(
                            agg_quanta(pr, j, xbs[j], xbc2, x_cur, x_next,
                                       m2, pat, accf))

                    if last and pg == len(GROUPS) - 1:
                        # pairs 0-5: make sure every piece write is issued
                        # BEFORE the DMA reads those columns (issue order
                        # defines RAW vs WAR for the dependency tracker)
                        while pending:
                            issue_pieces([pending.pop(0)], False)
                        nc.sync.dma_start(y_d.ap()[:, 0:4 * U],
                                          x_next[:, 0:4 * U])

                x_cur = x_next

            while fillers:
                fillers.popleft()()
            first = True
            while pending:
                nb_ = 1 if first else 2
                first = False
                issue_pieces(pending[:nb_], True)
                pending = pending[nb_:]
            nc.sync.dma_start(y_d.ap()[:, 4 * U:], x_cur[:, 4 * U:])

    nc.compile()
    return nc


def _make_runner(nc):
    import jax
    from jax.experimental.shard_map import shard_map
    from jax.sharding import Mesh, PartitionSpec, NamedSharding
    from concourse.bass2jax import (install_neuronx_cc_hook, _bass_exec_p,
                                    partition_id_tensor)

    install_neuronx_cc_hook()
    partition_name = (nc.partition_id_tensor.name
                      if nc.partition_id_tensor else None)
    in_names, out_names, out_avals, zero_outs = [], [], [], []
    for alloc in nc.m.functions[0].allocations:
        if not isinstance(alloc, mybir.MemoryLocationSet):
            continue
        name = alloc.memorylocations[0].name
        if alloc.kind == "ExternalInput":
            if name != partition_name:
                in_names.append(name)
        elif alloc.kind == "ExternalOutput":
            out_names.append(name)
            shape = tuple(alloc.tensor_shape)
            dtype = mybir.dt.np(alloc.dtype)
            out_avals.append(jax.core.ShapedArray(shape, dtype))
            zero_outs.append(np.zeros(shape, dtype))
    n_params = len(in_names)
    all_names = in_names + out_names
    if partition_name is not None:
        all_names = all_names + [partition_name]

    def _body(*args):
        operands = list(args)
        if partition_name is not None:
            operands.append(partition_id_tensor())
        outs = _bass_exec_p.bind(
            *operands,
            out_avals=tuple(out_avals),
            in_names=tuple(all_names),
            out_names=tuple(out_names),
            lowering_input_output_aliases=(),
            sim_require_finite=True,
            sim_require_nnan=True,
            nc=nc,
        )
        return tuple(outs)

    devices = jax.devices()[:M]
    mesh = Mesh(np.asarray(devices), ("core",))
    specs = (PartitionSpec("core"),) * (n_params + len(out_names))
    fn = jax.jit(shard_map(_body, mesh=mesh,
                           in_specs=specs,
                           out_specs=(PartitionSpec("core"),) * len(out_names)),
                 keep_unused=True)

    def put(in_maps):
        sh = NamedSharding(mesh, PartitionSpec("core"))
        args = []
        for name in in_names:
            cat = np.concatenate([np.asarray(im[name]) for im in in_maps], axis=0)
            args.append(jax.device_put(cat, sh))
        for z in zero_outs:
            cat = np.concatenate([z] * M, axis=0)
            args.append(jax.device_put(cat, sh))
        return args

    def run(args):
        outs = fn(*args)
        outs = [np.asarray(o) for o in outs]
        per_core = []
        for c in range(M):
            per_core.append({
                name: outs[i].reshape(M, *out_avals[i].shape)[c]
                for i, name in enumerate(out_names)})
        return per_core

    return put, run


_CACHE = {}


def _get_runner(meta):
    if meta not in _CACHE:
        nc = _build(meta)
        _CACHE[meta] = (_make_runner(nc), nc)
    return _CACHE[meta]


def _assemble(per_core, placements):
    out = np.empty((B, N, D), np.float32)
    for c in range(M):
        y = np.asarray(per_core[c]["y"], np.float32)   # [D, NP] padded transposed
        gids, pos = placements[c]
        out[gids] = y.T[pos]
    return out


def kernel(g, h, msg_W, gru_Wih, gru_Whh, gru_bih, gru_bhh):
    in_maps, meta, placements = _prepare(g, h, msg_W, gru_Wih, gru_Whh,
                                         gru_bih, gru_bhh)
    (put, run), _nc = _get_runner(meta)
    args = put(in_maps)
    per_core = run(args)
    return _assemble(per_core, placements)


# exposed for test.py
def get_nc_and_runner(g, h, msg_W, gru_Wih, gru_Whh, gru_bih, gru_bhh):
    in_maps, meta, placements = _prepare(g, h, msg_W, gru_Wih, gru_Whh,
                                         gru_bih, gru_bhh)
    (put, run), nc = _get_runner(meta)
    return in_maps, put, run, nc, placements



# revision 89
# speedup vs baseline: 1.0109x; 1.0002x over previous
"""Trainium2 Bass kernel for nn_Big_MPNN (gnn_message_passing).

Self-contained: hardcodes shapes/sharding. Data-parallel over the batch dim
across 8 NeuronCores (16 graphs per core), weights replicated; no collectives.

Node layout: the host pairs graphs to BALANCE per-type counts (local search
minimizing sum of per-type max counts over pairs), then sorts nodes by GRU
atom-type within each pair. Each pair occupies exactly U = sum(caps) columns
(no dead padding); per-type capacities are uniform across all pairs/cores so
every per-type GRU matmul reads a static strided access pattern.

Per-core dataflow (3 passes), transposed activations [D=128 part, cols],
all f16 except PSUM/biases/final cast.  Pairs are processed in groups of two;
per (layer, bond) wave one 2-bank PSUM tile holds both pairs and is drained
by a single ReLU op, load-balanced between ACT and DVE (GPSIMD cannot read
PSUM; it gets the SBUF-only f16 GRU elementwise ops instead).  Each group's
layer-7 flip (chunks 128/128/rem -> normal-layout xb) and aggregation
m^T = xb^T g^T are split into small matmul quanta and paced between the NEXT
group's MLP wave tiles, so the tensor engine never runs long drain-free
stretches.  GRU pieces merge both GRU universes into single wide elementwise
ops (messages in one [128, 2, NP] tile; tanh and the blend each issued once
per piece over [2, ncols]); each piece is expanded into 9 dependency-ordered
stage closures dripped two per wave so no engine's in-order stream blocks on
an op whose inputs resolve late (ACT/DVE have no exec-queue lookahead).  The
final pass drains its last pieces stage-interleaved with a DVE-only tail and
ships y in two DMA halves; pieces must be ISSUED before a DMA that reads
their columns (issue order defines RAW vs WAR for the dependency tracker).
Host unpads/unpermutes the f16 result.
"""

import numpy as np

import concourse.bass as bass
import concourse.bacc as bacc
import concourse.tile as tile
import concourse.mybir as mybir

F32 = mybir.dt.float32
F16 = mybir.dt.float16
AF = mybir.ActivationFunctionType
ALU = mybir.AluOpType

M = 8                      # cores
B, N, FEAT, D = 128, 128, 75, 128
NB, NL, NT = 7, 8, 6       # bonds, mlp layers, gru type slots
PASSES = 3
BG = B // M                # graphs per core
NPAIR = BG // 2            # graph pairs per core (8)
TOP_ATOMS = [6.0, 7.0, 8.0, 9.0, 0.0]


def _pair_graphs(cnt):
    """Pair the B graphs to minimize sum_t max_pairs(count_t).  cnt: [B, NT]."""
    P = B // 2
    order = np.argsort(cnt[:, NT - 1], kind="stable")
    pairs = np.stack([order[:P], order[:P - 1:-1]], 1)
    rng = np.random.default_rng(12345)

    def obj(pr):
        pc = cnt[pr[:, 0]] + cnt[pr[:, 1]]
        s = np.sort(pc, 0)[::-1]
        return s[0].sum() * 1000 + s[1].sum() * 10 + s[2].sum()

    cur = pairs.copy()
    co = obj(cur)
    best, bo = cur.copy(), co
    for _ in range(150000):
        i, j = rng.integers(0, P, 2)
        if i == j:
            continue
        trial = cur.copy()
        a1, b1 = trial[i]
        a2, b2 = trial[j]
        if rng.integers(0, 2) == 0:
            trial[i] = (a1, a2)
            trial[j] = (b1, b2)
        else:
            trial[i] = (a1, b2)
            trial[j] = (a2, b1)
        to = obj(trial)
        if to <= co:
            cur, co = trial, to
            if to < bo:
                best, bo = trial.copy(), to
    return best


def _prepare(g, h, msg_W, gru_Wih, gru_Whh, gru_bih, gru_bhh):
    g = np.ascontiguousarray(np.asarray(g, np.float32))
    h = np.ascontiguousarray(np.asarray(h, np.float32))
    msg_W = np.asarray(msg_W, np.float32)
    gru_Wih = np.asarray(gru_Wih, np.float32).reshape(2, NT, 3, D, D)
    gru_Whh = np.asarray(gru_Whh, np.float32).reshape(2, NT, 3, D, D)
    gru_bih = np.asarray(gru_bih, np.float32).reshape(2, NT, 3, D)
    gru_bhh = np.asarray(gru_bhh, np.float32).reshape(2, NT, 3, D)

    atoms = h[:, :, 0]
    tid = np.full((B, N), NT - 1, np.int32)
    for i, a in enumerate(TOP_ATOMS):
        tid[atoms == np.float32(a)] = i
    cnt = np.stack([(tid == t).sum(1) for t in range(NT)], 1).astype(np.int64)

    pairs = _pair_graphs(cnt)                       # [64, 2] graph ids
    pc = cnt[pairs[:, 0]] + cnt[pairs[:, 1]]
    caps = tuple(int(c) for c in pc.max(axis=0))
    U = sum(caps)
    assert 256 < U <= 384, f"caps {caps} sum {U} out of supported range"
    rem = U - 256
    NP = NPAIR * U
    offs = np.cumsum([0] + list(caps))[:-1]

    # replicated weights, partition-major f16 layouts
    mwT = np.transpose(msg_W, (3, 0, 1, 2))         # [din, k, l, dout]
    mwT06 = np.ascontiguousarray(mwT[:, :, :NL - 1]).astype(np.float16)
    mw8T = np.ascontiguousarray(mwT[:, :, NL - 1]).astype(np.float16)
    wihT = np.ascontiguousarray(
        np.transpose(gru_Wih, (4, 0, 1, 2, 3))).astype(np.float16)
    whhT = np.ascontiguousarray(
        np.transpose(gru_Whh, (4, 0, 1, 2, 3))).astype(np.float16)
    brz = np.ascontiguousarray(
        np.transpose(gru_bih[:, :, :2] + gru_bhh[:, :, :2], (3, 0, 1, 2)))
    brzM = np.ascontiguousarray(
        np.transpose(brz, (1, 2, 3, 0)).reshape(1, 2 * NT * 2, D)
    ).astype(np.float16)
    ones1 = np.ones((1, 512), np.float16)
    binn = np.ascontiguousarray(np.transpose(gru_bih[:, :, 2], (2, 0, 1)))
    bhnn = np.ascontiguousarray(np.transpose(gru_bhh[:, :, 2], (2, 0, 1)))

    h_t = np.concatenate([h, np.zeros((B, N, D - FEAT), np.float32)], axis=2)

    in_maps = []
    placements = []     # per core: (gids [BG], pos [BG, N])
    for c in range(M):
        gids = pairs[c * NPAIR:(c + 1) * NPAIR].reshape(-1)
        pos = np.zeros((BG, N), np.int64)
        x0 = np.zeros((NP, D), np.float32)
        gPa = np.zeros((128, NPAIR, 2, NB, U), np.float32)
        gPr = np.zeros((rem, NPAIR, NB, U), np.float32)
        for p in range(NPAIR):
            ga, gb = gids[2 * p], gids[2 * p + 1]
            tp = np.concatenate([tid[ga], tid[gb]])            # [256]
            hp = np.concatenate([h_t[ga], h_t[gb]], axis=0)    # [256, D]
            ppos = np.zeros(2 * N, np.int64)
            for t in range(NT):
                idx = np.flatnonzero(tp == t)
                ppos[idx] = offs[t] + np.arange(len(idx))
            pos[2 * p] = p * U + ppos[:N]
            pos[2 * p + 1] = p * U + ppos[N:]
            x0[p * U + ppos] = hp
            # dense pair block: big[m_row, k, n_col] = g[graph, k, n, m]
            big = np.zeros((U, NB, U), np.float32)
            for gi, gr in enumerate((ga, gb)):
                lg = ppos[gi * N:(gi + 1) * N]
                blk = np.transpose(g[gr], (2, 0, 1))           # [m, k, n]
                big[np.ix_(lg, np.arange(NB), lg)] = blk
            gPa[:, p, 0] = np.transpose(big[:128], (0, 1, 2))
            gPa[:, p, 1] = big[128:256]
            gPr[:, p] = big[256:U]
        placements.append((gids.copy(), pos))
        in_maps.append(dict(
            gPa=gPa.astype(np.float16),
            gPr=gPr.astype(np.float16),
            x0=np.ascontiguousarray(x0.T).astype(np.float16),
            mwT06=mwT06, mw8T=mw8T, wihT=wihT, whhT=whhT,
            brz=brz, binn=binn, bhnn=bhnn,
            brzM=brzM, ones1=ones1,
        ))
    meta = (caps, U)
    return in_maps, meta, placements


class _Balancer:
    """Greedy per-engine load balancer for drain/elementwise ops."""

    def __init__(self, nc):
        self.nc = nc
        self.load = {"A": 0.0, "D": 0.0, "P": 0.0}

    def _cost(self, e, op, cols, psum_src, f16_sbuf):
        # Exact TimelineSim engine-busy costs: processing = cols*cycle_t +
        # max-over-operands(2*access_cycles)/2 * cycle_t (SBUF dst dominates).
        if e == "A":
            return cols * 0.8333 + 185.0
        if e == "D":
            if f16_sbuf:
                return cols * 0.521 + 60.0
            return cols * 1.0417 + 125.0
        eff = 0.42 if op in ("add", "sub", "mul") else 0.6
        return cols * 0.8333 / eff + 131.0

    def pick(self, op, cols, psum_src=True, f16_sbuf=False, allow=("A", "D")):
        cand = [(self.load[e] + self._cost(e, op, cols, psum_src, f16_sbuf), e)
                for e in allow]
        _, e = min(cand)
        self.load[e] += self._cost(e, op, cols, psum_src, f16_sbuf)
        return e

    def charge(self, e, op, cols, psum_src=True, f16_sbuf=False):
        self.load[e] += self._cost(e, op, cols, psum_src, f16_sbuf)

    # PSUM sources: GPSIMD has no PSUM access -> ACT/DVE only.
    def relu(self, out, ps, cols):
        e = self.pick("relu", cols)
        if e == "A":
            self.nc.scalar.activation(out, ps, AF.Relu)
        else:
            self.nc.vector.tensor_scalar_max(out, ps, 0.0)

    def copy(self, out, ps, cols):
        e = self.pick("copy", cols)
        if e == "A":
            self.nc.scalar.copy(out, ps)
        else:
            self.nc.vector.tensor_copy(out, ps)

    def stt(self, out, in0, scal, in1, op0, op1, cols):
        self.charge("D", "stt", cols)
        self.nc.vector.scalar_tensor_tensor(out, in0, scal, in1,
                                            op0=op0, op1=op1)

    # SBUF-only f16 elementwise: DVE or Pool.
    def tt(self, op, out, a, b, cols, f16_sbuf=True, allow=("P",)):
        e = self.pick(op, cols, psum_src=False, f16_sbuf=f16_sbuf, allow=allow)
        eng = self.nc.vector if e == "D" else self.nc.gpsimd
        getattr(eng, "tensor_" + op)(out, a, b)


def _build(meta):
    caps, U = meta
    rem = U - 256
    NP = NPAIR * U
    nc = bacc.Bacc("TRN2", target_bir_lowering=False, debug=False, num_devices=M)

    gPa_d = nc.dram_tensor("gPa", [128, NPAIR, 2, NB, U], F16, kind="ExternalInput")
    gPr_d = nc.dram_tensor("gPr", [rem, NPAIR, NB, U], F16, kind="ExternalInput")
    x0_d = nc.dram_tensor("x0", [128, NP], F16, kind="ExternalInput")
    mwT06_d = nc.dram_tensor("mwT06", [128, NB, NL - 1, 128], F16, kind="ExternalInput")
    mw8T_d = nc.dram_tensor("mw8T", [128, NB, 128], F16, kind="ExternalInput")
    wih_d = nc.dram_tensor("wihT", [128, 2, NT, 3, 128], F16, kind="ExternalInput")
    whh_d = nc.dram_tensor("whhT", [128, 2, NT, 3, 128], F16, kind="ExternalInput")
    brz_d = nc.dram_tensor("brz", [128, 2, NT, 2], F32, kind="ExternalInput")
    brzM_d = nc.dram_tensor("brzM", [1, 2 * NT * 2, 128], F16, kind="ExternalInput")
    ones_d = nc.dram_tensor("ones1", [1, 512], F16, kind="ExternalInput")
    binn_d = nc.dram_tensor("binn", [128, 2, NT], F32, kind="ExternalInput")
    bhnn_d = nc.dram_tensor("bhnn", [128, 2, NT], F32, kind="ExternalInput")
    y_d = nc.dram_tensor("y", [128, NP], F16, kind="ExternalOutput")

    # GRU pieces: (type, col-offset, pair0, n_pairs); issued after pair p0+npr-1
    # The final pass splits the second half into npr=2 pieces so the tail only
    # waits on the last two pairs' aggregation.
    pieces_at = {pr: [] for pr in range(NPAIR)}
    pieces_at_final = {pr: [] for pr in range(NPAIR)}
    off = 0
    for t in range(NT):
        if caps[t] == 0:
            continue
        npr = min(4, max(1, 256 // caps[t]))
        while NPAIR % npr:
            npr -= 1
        for p0 in range(0, NPAIR, npr):
            pieces_at[p0 + npr - 1].append((t, off, p0, npr))
            if p0 < NPAIR // 2 or npr <= 2:
                pieces_at_final[p0 + npr - 1].append((t, off, p0, npr))
            else:
                for q0 in range(p0, p0 + npr, 2):
                    pieces_at_final[q0 + 1].append((t, off, q0, 2))
        off += caps[t]

    with tile.TileContext(nc) as tc:
        with (
            tc.tile_pool(name="const", bufs=1) as cp,
            tc.tile_pool(name="xp", bufs=2) as xp,
            tc.tile_pool(name="mlp", bufs=24) as mp,
            tc.tile_pool(name="x7p", bufs=10) as x7p,
            tc.tile_pool(name="xbp", bufs=3) as xbp,
            tc.tile_pool(name="gtp", bufs=5) as gtp,
            tc.tile_pool(name="mtp", bufs=2) as mtp,
            tc.tile_pool(name="gates", bufs=32) as ggp,
            tc.tile_pool(name="mps", bufs=3, space="PSUM") as mpsp,
            tc.tile_pool(name="ps", bufs=2, space="PSUM") as psp,
        ):
            bal = _Balancer(nc)

            x_cur = xp.tile([128, NP], F16, tag="x")
            mwT06 = cp.tile([128, NB, NL - 1, 128], F16, tag="mwT06")
            nc.sync.dma_start(x_cur[:, 0:2 * U], x0_d.ap()[:, 0:2 * U])
            nc.sync.dma_start(mwT06[:, 0:1, 0:1, :],
                              mwT06_d.ap()[:, 0:1, 0:1, :])
            nc.sync.dma_start(mwT06[:, 1:, 0:1, :],
                              mwT06_d.ap()[:, 1:, 0:1, :])
            nc.sync.dma_start(mwT06[:, :, 1:2, :], mwT06_d.ap()[:, :, 1:2, :])
            nc.sync.dma_start(x_cur[:, 2 * U:4 * U], x0_d.ap()[:, 2 * U:4 * U])
            nc.sync.dma_start(mwT06[:, :, 2:, :], mwT06_d.ap()[:, :, 2:, :])
            nc.sync.dma_start(x_cur[:, 4 * U:], x0_d.ap()[:, 4 * U:])

            gtiles = {}
            for pn in (0, 1):
                gta0 = gtp.tile([128, 2, NB, U], F16, tag="gta")
                nc.sync.dma_start(gta0[:], gPa_d.ap()[:, pn])
                gtr0 = gtp.tile([64, NB, U], F16, tag="gtr")
                o = 32 * (pn % 2)
                nc.sync.dma_start(gtr0[o:o + rem], gPr_d.ap()[:, pn])
                gtiles[pn] = (gta0, gtr0)

            mw8T = cp.tile([128, NB, 128], F16, tag="mw8T")
            wih = cp.tile([128, 2, NT, 3, 128], F16, tag="wih")
            whh = cp.tile([128, 2, NT, 3, 128], F16, tag="whh")
            brz = cp.tile([128, 2, NT, 2], F32, tag="brz")
            brzM = cp.tile([1, 2 * NT * 2, 128], F16, tag="brzM")
            ones1 = cp.tile([1, 512], F16, tag="ones1")
            binn = cp.tile([128, 2, NT], F32, tag="binn")
            bhnn = cp.tile([128, 2, NT], F32, tag="bhnn")
            nc.sync.dma_start(mw8T[:], mw8T_d.ap())
            nc.sync.dma_start(wih[:], wih_d.ap())
            nc.sync.dma_start(whh[:], whh_d.ap())
            nc.sync.dma_start(brz[:], brz_d.ap())
            nc.sync.dma_start(brzM[:], brzM_d.ap())
            nc.sync.dma_start(ones1[:], ones_d.ap())
            nc.sync.dma_start(binn[:], binn_d.ap())
            nc.sync.dma_start(bhnn[:], bhnn_d.ap())

            def seg(tile_, t_off, p0, npr, w):
                return tile_[:].rearrange("d (pr u) -> d pr u", u=U)[
                    :, p0:p0 + npr, t_off:t_off + w]

            def seg_m(m2, u, t_off, p0, npr, w):
                return m2[:, u, :].rearrange("d (pr u2) -> d pr u2", u2=U)[
                    :, p0:p0 + npr, t_off:t_off + w]

            def seg2(m2, t_off, p0, npr, w):
                """4D view of the merged [128, 2, NP] message tile:
                [128, u, pair, col]."""
                return m2[:].rearrange("d u (pr u2) -> d u pr u2", u2=U)[
                    :, :, p0:p0 + npr, t_off:t_off + w]

            def piece_stages(args, fast_tail):
                """Stage closures for ONE GRU piece.  Dripping one stage per
                wave keeps each engine's in-order stream free of ops whose
                dependencies resolve late (head-of-line blocking: ACT/DVE have
                no exec-queue lookahead, so a stalled op blocks later ones)."""
                (xc, xn, m2, piece) = args
                t, t_off, p0, npr = piece
                w = caps[t]
                ncols = npr * w
                s = dict(xs=seg(xc, t_off, p0, npr, w),
                         ms=[seg_m(m2, 0, t_off, p0, npr, w),
                             seg_m(m2, 1, t_off, p0, npr, w)],
                         ms2=seg2(m2, t_off, p0, npr, w))
                tail = ("D",) if fast_tail else ("P",)

                def st_mm():
                    s["prz"], s["pn2"] = [], []
                    for u in range(2):
                        prz = psp.tile([128, 2, 256], F32, tag="ps",
                                       name="prz")
                        pool2 = mpsp if fast_tail else psp
                        pn2 = pool2.tile([128, 2, 256], F32,
                                         tag="mps" if fast_tail else "ps",
                                         name="pn2")
                        for gi in range(2):
                            nc.tensor.matmul(prz[:, gi, :ncols],
                                             wih[:, u, t, gi, :], s["xs"],
                                             start=True, stop=False)
                            nc.tensor.matmul(prz[:, gi, :ncols],
                                             whh[:, u, t, gi, :], s["ms"][u],
                                             start=False, stop=False)
                            row = (u * NT + t) * 2 + gi
                            nc.tensor.matmul(prz[:, gi, :ncols],
                                             brzM[0:1, row, :],
                                             ones1[0:1, :ncols],
                                             start=False, stop=True)
                        nc.tensor.matmul(pn2[:, 0, :ncols], wih[:, u, t, 2, :],
                                         s["xs"], start=True, stop=True)
                        nc.tensor.matmul(pn2[:, 1, :ncols], whh[:, u, t, 2, :],
                                         s["ms"][u], start=True, stop=True)
                        s["prz"].append(prz)
                        s["pn2"].append(pn2)
                    s["rzb"] = ggp.tile([128, 2, 2, 256], F16,
                                        tag="gt4", name="rzb", bufs=6)

                def st_sig(u):
                    nc.scalar.activation(s["rzb"][:, u, :, :ncols],
                                         s["prz"][u][:, :, :ncols],
                                         AF.Sigmoid)
                    bal.charge("A", "act", 2 * ncols)

                def st_t1():
                    t12 = ggp.tile([128, 2, 256], F16, tag="gt2", name="t12",
                                   bufs=17)
                    s["t12"] = t12
                    for u in range(2):
                        bal.stt(t12[:, u, :ncols], s["pn2"][u][:, 1, :ncols],
                                bhnn[:, u, t:t + 1], s["rzb"][:, u, 0, :ncols],
                                ALU.add, ALU.mult, ncols)

                def st_na():
                    na2 = ggp.tile([128, 2, 256], F16, tag="gt2", name="na2",
                                   bufs=17)
                    s["na2"] = na2
                    for u in range(2):
                        bal.stt(na2[:, u, :ncols], s["pn2"][u][:, 0, :ncols],
                                binn[:, u, t:t + 1], s["t12"][:, u, :ncols],
                                ALU.add, ALU.add, ncols)

                def st_tanh():
                    n2 = ggp.tile([128, 2, 256], F16, tag="gt2", name="n2",
                                  bufs=17)
                    nc.scalar.activation(n2[:, :, :ncols],
                                         s["na2"][:, :, :ncols], AF.Tanh)
                    bal.charge("A", "act", 2 * ncols, psum_src=False)
                    s["n2"] = n2

                def st_d2():
                    d2 = ggp.tile([128, 2, 256], F16, tag="gt2", name="d2",
                                  bufs=17)
                    n2v = s["n2"][:, :, :ncols].rearrange(
                        "d u (pr w) -> d u pr w", w=w)
                    d2v = d2[:, :, :ncols].rearrange(
                        "d u (pr w) -> d u pr w", w=w)
                    bal.tt("sub", d2v, s["ms2"], n2v, 2 * ncols, allow=tail)
                    s["d2"] = d2

                def st_e2():
                    e2 = ggp.tile([128, 2, 256], F16, tag="gt2", name="e2",
                                  bufs=17)
                    bal.tt("mul", e2[:, :, :ncols],
                           s["rzb"][:, :, 1, :ncols],
                           s["d2"][:, :, :ncols], 2 * ncols, allow=tail)
                    s["e2"] = e2

                def st_hu():
                    hu2 = ggp.tile([128, 2, 256], F16, tag="gt2", name="hu2",
                                   bufs=17)
                    bal.tt("add", hu2[:, :, :ncols], s["n2"][:, :, :ncols],
                           s["e2"][:, :, :ncols], 2 * ncols, allow=tail)
                    hv = hu2[:, :, :ncols].rearrange(
                        "d u (pr w) -> d u pr w", w=w)
                    bal.tt("add", seg(xn, t_off, p0, npr, w),
                           hv[:, 0], hv[:, 1],
                           ncols, allow=("D",) if fast_tail else ("P",))

                return [st_mm, lambda: st_sig(0), lambda: st_sig(1),
                        st_t1, st_na, st_tanh, st_d2, st_e2, st_hu]

            def issue_pieces(batch, fast_tail):
                """Issue whole pieces, stage-interleaved across the batch."""
                stl = [piece_stages(a, fast_tail) for a in batch]
                for i in range(max(len(sl) for sl in stl)):
                    for sl in stl:
                        if i < len(sl):
                            sl[i]()

            def flip_quanta(pr, i, x7t, xb):
                """Per-pair layer-8 flip quanta, chunks 0-1 only (the rem
                chunk of BOTH pairs goes into one shared group tile)."""
                qs = []
                for k0, kn in ((0, 2), (2, 2), (4, 2), (6, 1)):
                    def fq(k0=k0, kn=kn):
                        kk = list(range(k0, k0 + kn))
                        ps3 = mpsp.tile([128, len(kk), 2, 128], F32, tag="mps",
                                        name="ps3")
                        for j, k in enumerate(kk):
                            nc.tensor.matmul(ps3[:, j, 0, :],
                                             x7t[k][:, i, 0:128],
                                             mw8T[:, k, :],
                                             start=True, stop=True)
                            nc.tensor.matmul(ps3[:, j, 1, :],
                                             x7t[k][:, i, 128:256],
                                             mw8T[:, k, :],
                                             start=True, stop=True)
                        bal.copy(xb[:, k0:k0 + len(kk), :, :], ps3[:],
                                 len(kk) * 256)
                    qs.append(fq)
                return qs

            def c2_quantum(grp, x7t, xbc2, split=False):
                """Both pairs' rem-chunk flips into ONE PSUM tile (pair 1 at
                partition 32 via PE col-tiling, auto-derived from the out AP's
                base partition) and a single 896-col drain.  split=True (final
                group) drains in two halves on both engines in parallel since
                the copy gates the epilogue's mc=2 aggregation steps."""
                def cq():
                    ps3c = mpsp.tile([128, NB, 128], F32, tag="mps",
                                     name="ps3c")
                    for i in range(len(grp)):
                        off = 32 * i
                        for k in range(NB):
                            nc.tensor.matmul(ps3c[off:off + rem, k, :],
                                             x7t[k][:, i, 256:U],
                                             mw8T[:, k, :],
                                             start=True, stop=True)
                    if split:
                        bal.copy(xbc2[0:32 + rem, 0:4, :],
                                 ps3c[0:32 + rem, 0:4, :], 4 * 128)
                        bal.copy(xbc2[0:32 + rem, 4:, :],
                                 ps3c[0:32 + rem, 4:, :], 3 * 128)
                    else:
                        bal.copy(xbc2[0:32 + rem, :, :],
                                 ps3c[0:32 + rem, :, :], NB * 128)
                return [cq]

            def agg_quanta(pr, i, xb, xbc2, xc, xn, m2, pat,
                           accf=None):
                """Aggregation quanta for one pair; mc=2 reads the shared rem
                tile at base partition 32*i (gtr rows DMA'd to match).  With
                accf (final group only, runs wholly in the epilogue): both
                pairs accumulate into one shared 2-bank tile so pair 7 never
                waits on pair 6's psp slot, and the m2 copies merge."""
                cell = {}

                def ps_():
                    if 'ps' not in cell:
                        if accf is not None:
                            cell['ps'] = (accf[:, 0, :U], accf[:, 1, :U])
                        else:
                            cell['ps'] = (psp.tile([128, U], F32, tag="ps",
                                                   name="ps_n"),
                                          psp.tile([128, U], F32, tag="ps",
                                                   name="ps_u"))
                    return cell['ps']

                off = 32 * i
                qs = []
                steps = [(mc, k) for mc in range(3) for k in range(NB)]
                chunks = [steps[j:j + 4] for j in range(0, len(steps), 4)]
                for ci, ch in enumerate(chunks):
                    def aq(ch=ch, lastq=(ci == len(chunks) - 1)):
                        ps_n, ps_u = ps_()
                        gta, gtr = gtiles[pr]
                        for (mc, k) in ch:
                            dst = ps_u if k == NB - 1 else ps_n
                            start = mc == 0 and k in (0, NB - 1)
                            stop = mc == 2 and k in (NB - 2, NB - 1)
                            if mc < 2:
                                nc.tensor.matmul(dst[:], xb[:, k, mc, :],
                                                 gta[:, mc, k, :],
                                                 start=start, stop=stop)
                            else:
                                nc.tensor.matmul(dst[:],
                                                 xbc2[off:off + rem, k, :],
                                                 gtr[off:off + rem, k, :],
                                                 start=start, stop=stop)
                        if not lastq:
                            return
                        sl = slice(pr * U, (pr + 1) * U)
                        bal.copy(m2[:, 0, sl], ps_n[:], U)
                        bal.copy(m2[:, 1, sl], ps_u[:], U)
                        for piece in pat[pr]:
                            pending.append((xc, xn, m2, piece))
                    qs.append(aq)
                return qs

            from collections import deque
            pending = []        # GRU pieces awaiting issue
            fillers = deque()   # flip/agg quanta awaiting interleave
            stq = deque()       # piece stages dripped one per wave
            GROUPS = ((0, 1), (2, 3), (4, 5), (6, 7))
            for p in range(PASSES):
                last = p == PASSES - 1
                pat = pieces_at
                x_next = xp.tile([128, NP], F16, tag="x")
                m2 = mtp.tile([128, 2, NP], F16, tag="m2")

                for pg, grp in enumerate(GROUPS):
                    G = len(grp)
                    # prefetch next group's adjacency (one group ahead)
                    if pg + 1 < len(GROUPS):
                        nxt = [(p, pn_) for pn_ in GROUPS[pg + 1]]
                    else:
                        nxt = [(p + 1, pn_) for pn_ in GROUPS[0]]
                    for pp, pn in nxt:
                        if pp < PASSES:
                            gta = gtp.tile([128, 2, NB, U], F16, tag="gta")
                            nc.sync.dma_start(gta[:], gPa_d.ap()[:, pn])
                            gtr = gtp.tile([64, NB, U], F16, tag="gtr")
                            o = 32 * (pn % 2)
                            nc.sync.dma_start(gtr[o:o + rem], gPr_d.ap()[:, pn])
                            gtiles[pn] = (gta, gtr)

                    # all still-pending pieces must land before this group's
                    # first wave reads their output columns
                    while pending:
                        issue_pieces([pending.pop(0)], False)

                    # ---- bond MLPs: G pairs per PSUM tile, waves over bonds;
                    # the previous group's flips/aggs and older GRU pieces are
                    # interleaved between waves to keep every engine fed ----
                    curs = [[x_cur[:, pr * U:(pr + 1) * U]] * NB for pr in grp]
                    x7t = [None] * NB
                    per_slot = len(fillers) / ((NL - 1) * NB + 4)
                    credit = 0.0
                    for l in range(NL - 1):
                        outs = [[] for _ in grp]
                        for k in range(NB):
                            if l == NL - 2:
                                nt_ = x7p.tile([128, G, U], F16, tag="x7")
                            else:
                                nt_ = mp.tile([128, G, U], F16, tag="mlp")
                            if l == NL - 2:
                                x7t[k] = nt_
                            ps = mpsp.tile([128, G, 512], F32, tag="mps")
                            for j in range(G):
                                nc.tensor.matmul(ps[:, j, :U],
                                                 mwT06[:, k, l, :],
                                                 curs[j][k],
                                                 start=True, stop=True)
                            bal.relu(nt_[:], ps[:, :, :U], G * U)
                            for j in range(G):
                                outs[j].append(nt_[:, j, :])
                            credit += per_slot
                            for _ in range(2):
                                if stq:
                                    stq.popleft()()
                            while credit >= 1.0 and fillers:
                                fillers.popleft()()
                                credit -= 1.0
                        curs = outs
                        while pending:
                            stq.extend(piece_stages(pending.pop(0), False))

                    while fillers:
                        fillers.popleft()()
                    while stq:
                        stq.popleft()()
                    xbs = [xbp.tile([128, NB, 2, 128], F16, tag="xb",
                                    name="xb") for _ in grp]
                    xbc2 = xbp.tile([64, NB, 128], F16, tag="xbc2", bufs=2,
                                    name="xbc2")
                    for j, pr in enumerate(grp):
                        fillers.extend(flip_quanta(pr, j, x7t, xbs[j]))
                    fillers.extend(c2_quantum(
                        grp, x7t, xbc2,
                        split=last and pg == len(GROUPS) - 1))
                    fin2 = last and pg == len(GROUPS) - 1
                    for j, pr in enumerate(grp):
                        accf = None
                        if fin2 and j == 1:
                            accf = mpsp.tile([128, 2, 512], F32, tag="mps",
                                             name="accf")
                        fillers.extend(
                            agg_quanta(pr, j, xbs[j], xbc2, x_cur, x_next,
                                       m2, pat, accf))

                    if last and pg == len(GROUPS) - 1:
                        # pairs 0-5: make sure every piece write is issued
                        # BEFORE the DMA reads those columns (issue order
                        # defines RAW vs WAR for the dependency tracker)
                        while pending:
                            issue_pieces([pending.pop(0)], False)
                        nc.sync.dma_start(y_d.ap()[:, 0:4 * U],
                                          x_next[:, 0:4 * U])

                x_cur = x_next

            while fillers:
                fillers.popleft()()
            first = True
            while pending:
                nb_ = 1 if first else 2
                first = False
                issue_pieces(pending[:nb_], True)
                pending = pending[nb_:]
            nc.sync.dma_start(y_d.ap()[:, 4 * U:], x_cur[:, 4 * U:])

    nc.compile()
    return nc


def _make_runner(nc):
    import jax
    from jax.experimental.shard_map import shard_map
    from jax.sharding import Mesh, PartitionSpec, NamedSharding
    from concourse.bass2jax import (install_neuronx_cc_hook, _bass_exec_p,
                                    partition_id_tensor)

    install_neuronx_cc_hook()
    partition_name = (nc.partition_id_tensor.name
                      if nc.partition_id_tensor else None)
    in_names, out_names, out_avals, zero_outs = [], [], [], []
    for alloc in nc.m.functions[0].allocations:
        if not isinstance(alloc, mybir.MemoryLocationSet):
            continue
        name = alloc.memorylocations[0].name
        if alloc.kind == "ExternalInput":
            if name != partition_name:
                in_names.append(name)
        elif alloc.kind == "ExternalOutput":
            out_names.append(name)
            shape = tuple(alloc.tensor_shape)
            dtype = mybir.dt.np(alloc.dtype)
            out_avals.append(jax.core.ShapedArray(shape, dtype))
            zero_outs.append(np.zeros(shape, dtype))
    n_params = len(in_names)
    all_names = in_names + out_names
    if partition_name is not None:
        all_names = all_names + [partition_name]

    def _body(*args):
        operands = list(args)
        if partition_name is not None:
            operands.append(partition_id_tensor())
        outs = _bass_exec_p.bind(
            *operands,
            out_avals=tuple(out_avals),
            in_names=tuple(all_names),
            out_names=tuple(out_names),
            lowering_input_output_aliases=(),
            sim_require_finite=True,
            sim_require_nnan=True,
            nc=nc,
        )
        return tuple(outs)

    devices = jax.devices()[:M]
    mesh = Mesh(np.asarray(devices), ("core",))
    specs = (PartitionSpec("core"),) * (n_params + len(out_names))
    fn = jax.jit(shard_map(_body, mesh=mesh,
                           in_specs=specs,
                           out_specs=(PartitionSpec("core"),) * len(out_names)),
                 keep_unused=True)

    def put(in_maps):
        sh = NamedSharding(mesh, PartitionSpec("core"))
        args = []
        for name in in_names:
            cat = np.concatenate([np.asarray(im[name]) for im in in_maps], axis=0)
            args.append(jax.device_put(cat, sh))
        for z in zero_outs:
            cat = np.concatenate([z] * M, axis=0)
            args.append(jax.device_put(cat, sh))
        return args

    def run(args):
        outs = fn(*args)
        outs = [np.asarray(o) for o in outs]
        per_core = []
        for c in range(M):
            per_core.append({
                name: outs[i].reshape(M, *out_avals[i].shape)[c]
                for i, name in enumerate(out_names)})
        return per_core

    return put, run


_CACHE = {}


def _get_runner(meta):
    if meta not in _CACHE:
        nc = _build(meta)
        _CACHE[meta] = (_make_runner(nc), nc)
    return _CACHE[meta]


def _assemble(per_core, placements):
    out = np.empty((B, N, D), np.float32)
    for c in range(M):
        y = np.asarray(per_core[c]["y"], np.float32)   # [D, NP] padded transposed
        gids, pos = placements[c]
        out[gids] = y.T[pos]
    return out


def kernel(g, h, msg_W, gru_Wih, gru_Whh, gru_bih, gru_bhh):
    in_maps, meta, placements = _prepare(g, h, msg_W, gru_Wih, gru_Whh,
                                         gru_bih, gru_bhh)
    (put, run), _nc = _get_runner(meta)
    args = put(in_maps)
    per_core = run(args)
    return _assemble(per_core, placements)


# exposed for test.py
def get_nc_and_runner(g, h, msg_W, gru_Wih, gru_Whh, gru_bih, gru_bhh):
    in_maps, meta, placements = _prepare(g, h, msg_W, gru_Wih, gru_Whh,
                                         gru_bih, gru_bhh)
    (put, run), nc = _get_runner(meta)
    return in_maps, put, run, nc, placements

